# revision 16
# baseline (speedup 1.0000x reference)
"""Trainium2 Bass kernel for nn_Decoder_42417097016016 (DCTTS-style decoder).

Sharding: pure data parallel over batch. B=16 samples -> 8 NeuronCores x 2
samples each; all weights replicated per core.

Layout: activations live on-chip as (channels, time) so every conv1x1 /
causal conv is a PE matmul with channels on partitions.  Causal convs with
dilation d are 3 accumulating matmuls per output tile with column shifts
(0, d, 2d) - left zero-padding falls out of the shifted PSUM accumulation.

The attention block computes scores (t,s), softmax along free dim (ACT Exp
with accum_out row sums), writes the normalized attn output directly, and
PE-transposes it to (s,t) for the context matmul.  mel and done share one
final matmul by concatenating fc_w as a 401st output column.
"""

import math
import os
import sys

import numpy as np

for _p in ("/opt/trn_rl_repo", "/root/.axon_site/_ro/trn_rl_repo"):
    if os.path.isdir(_p) and _p not in sys.path:
        sys.path.append(_p)

import concourse.bass as bass
import concourse.tile as tile
from concourse import bacc, mybir
from concourse.bass_utils import run_bass_kernel_spmd

AF = mybir.ActivationFunctionType
ALU = mybir.AluOpType
AX = mybir.AxisListType
F32 = mybir.dt.float32
F32R = mybir.dt.float32r

N_CORES = 8
B, T, TE, D, F = 16, 512, 256, 256, 400
BS = B // N_CORES  # samples per core
DIL = [1, 3, 9, 27, 1, 3, 9, 27, 3, 3] + [1, 3, 9, 27, 1, 1]
L = len(DIL)  # 16 highway layers (10 encoder + 6 decoder)
SQ2 = math.sqrt(0.5)

# matmul input precision: "f32r" = full-rate reduced-precision fp32 multiply,
# "f32" = exact fp32 at 1/4 PE rate.
MM_DT = os.environ.get("KBENCH_MM", "f32r")

# bias table column assignments
COL_ENC0, COL_ENC1, COL_ENC2 = 0, 2, 4
COL_QB, COL_OB = 6, 8
COL_DEC0, COL_DEC1, COL_DEC2, COL_DEC3 = 10, 12, 14, 16
NB = 18

LAST_EXEC_NS = None
_BUILD_CACHE = {}


def _mm(nc, out, lhsT, rhs, **kw):
    nc.tensor.matmul(out, lhsT, rhs, **kw)


def _build():
    """Build the per-core Bass program (same program on all 8 cores)."""
    from concourse.masks import make_identity

    nc = bacc.Bacc("TRN2", target_bir_lowering=False, debug=False)
    dt = F32
    # matmul-input storage dtype: walrus requires fp32r matmul operands to be
    # *produced* as fp32r, so every tile feeding the PE carries this dtype.
    dtm = F32R if MM_DT == "f32r" else F32

    # ---- DRAM I/O (per-core shard shapes) ----
    d_x0 = nc.dram_tensor("x0", [BS, F, T], dtm, kind="ExternalInput").ap()
    d_keysT = nc.dram_tensor("keysT", [BS, D, TE], dtm, kind="ExternalInput").ap()
    d_values = nc.dram_tensor("values", [BS, TE, D], dtm, kind="ExternalInput").ap()
    d_hw_w = nc.dram_tensor("hw_w", [L, 128, 2, 3, 4, 128], dtm, kind="ExternalInput").ap()
    d_hw_b = nc.dram_tensor("hw_b", [128, L, 4], dt, kind="ExternalInput").ap()
    d_w_enc0 = nc.dram_tensor("w_enc0", [F, D], dtm, kind="ExternalInput").ap()
    d_w_enc1 = nc.dram_tensor("w_enc1", [D, D], dtm, kind="ExternalInput").ap()
    d_w_enc2 = nc.dram_tensor("w_enc2", [D, D], dtm, kind="ExternalInput").ap()
    d_w_q = nc.dram_tensor("w_q", [D, D], dtm, kind="ExternalInput").ap()
    d_w_o = nc.dram_tensor("w_o", [D, D], dtm, kind="ExternalInput").ap()
    d_w_dec0 = nc.dram_tensor("w_dec0", [2 * D, D], dtm, kind="ExternalInput").ap()
    d_w_dec1 = nc.dram_tensor("w_dec1", [D, D], dtm, kind="ExternalInput").ap()
    d_w_dec2 = nc.dram_tensor("w_dec2", [D, D], dtm, kind="ExternalInput").ap()
    d_w_dec3 = nc.dram_tensor("w_dec3", [D, D], dtm, kind="ExternalInput").ap()
    d_w_last = nc.dram_tensor("w_last", [D, F + 2], dtm, kind="ExternalInput").ap()
    d_b_last = nc.dram_tensor("b_last", [2, F + 2], dtm, kind="ExternalInput").ap()
    d_ones = nc.dram_tensor("ones", [2, 128], dtm, kind="ExternalInput").ap()
    d_bias = nc.dram_tensor("bias_tbl", [128, NB], dt, kind="ExternalInput").ap()

    d_mel = nc.dram_tensor("mel", [BS, T, F], dt, kind="ExternalOutput").ap()
    d_attn = nc.dram_tensor("attn", [BS, T, TE], dt, kind="ExternalOutput").ap()
    d_done = nc.dram_tensor("done", [BS, T, 1], dt, kind="ExternalOutput").ap()

    with tile.TileContext(nc) as tc:
        with (
            tc.tile_pool(name="const", bufs=1) as const,
            tc.tile_pool(name="xpool", bufs=1) as xpool,
            tc.tile_pool(name="persist", bufs=1) as persist,
            tc.tile_pool(name="wstream", bufs=1) as wstream,
            tc.tile_pool(name="temp", bufs=1) as temp,
            tc.tile_pool(name="psum", bufs=1, space="PSUM") as psum,
        ):
            # ---------- constants ----------
            ident = const.tile([128, 128], dt, name="ident")
            make_identity(nc, ident)
            ones_row = const.tile([2, 128], dtm, name="ones_row")
            nc.sync.dma_start(ones_row, d_ones)
            bias_sb = const.tile([128, NB], dt, name="bias_sb")
            nc.sync.dma_start(bias_sb, d_bias)
            hwb_sb = const.tile([128, L, 4], dt, name="hwb_sb")
            nc.sync.dma_start(hwb_sb, d_hw_b)
            blast_sb = const.tile([2, F + 2], dtm, name="blast_sb")
            nc.sync.dma_start(blast_sb, d_b_last)

            def load_w(dram, nm, rows):
                tiles = []
                nchunk = (rows + 127) // 128
                cols = dram.shape[1]
                for c in range(nchunk):
                    cs = min(128, rows - c * 128)
                    t_ = const.tile([cs, cols], dtm, name=f"{nm}_{c}")
                    nc.sync.dma_start(t_, dram[c * 128 : c * 128 + cs, :])
                    tiles.append(t_)
                return tiles

            w_enc0_sb = load_w(d_w_enc0, "wenc0", F)
            w_enc1_sb = load_w(d_w_enc1, "wenc1", D)
            w_enc2_sb = load_w(d_w_enc2, "wenc2", D)
            w_q_sb = load_w(d_w_q, "wq", D)
            w_o_sb = load_w(d_w_o, "wo", D)
            w_dec0_sb = load_w(d_w_dec0, "wdec0", 2 * D)
            w_dec1_sb = load_w(d_w_dec1, "wdec1", D)
            w_dec2_sb = load_w(d_w_dec2, "wdec2", D)
            w_dec3_sb = load_w(d_w_dec3, "wdec3", D)
            w_last_sb = load_w(d_w_last, "wlast", D)

            kT_sb, v_sb = {}, {}
            for s in range(BS):
                for c in range(2):
                    kt = const.tile([128, TE], dtm, name=f"keysT_{s}_{c}")
                    nc.sync.dma_start(kt, d_keysT[s, c * 128 : (c + 1) * 128, :])
                    kT_sb[(s, c)] = kt
                    vt = const.tile([128, D], dtm, name=f"values_{s}_{c}")
                    nc.sync.dma_start(vt, d_values[s, c * 128 : (c + 1) * 128, :])
                    v_sb[(s, c)] = vt

            stat_sb = const.tile([128, 32], F32, name="stat_sb")

            # helper: one conv1x1 block (256 outputs) for all samples,
            # interleaved across samples for engine-FIFO pipelining.
            def conv_block(w_tiles, rhs_per_s, func, bias_col, banks, out_pool,
                           tag_fn, bufs=1, uid=""):
                outs = {s: [None, None] for s in range(BS)}
                pss = {}
                for mt in range(2):
                    for s in range(BS):
                        ps = psum.tile(
                            [128, T], F32,
                            tag=f"bank{banks[s * 2 + mt]}",
                            name=f"ps_{uid}_{s}_{mt}",
                        )
                        nk = len(w_tiles)
                        for c in range(nk):
                            _mm(nc, ps, w_tiles[c][:, mt * 128 : (mt + 1) * 128],
                                rhs_per_s[s][c], start=(c == 0), stop=(c == nk - 1))
                        pss[(s, mt)] = ps
                for mt in range(2):
                    for s in range(BS):
                        ot = out_pool.tile(
                            [128, T], dtm,
                            tag=tag_fn(s, mt), bufs=bufs,
                            name=f"{uid}_{s}_{mt}",
                        )
                        nc.scalar.activation(
                            ot, pss[(s, mt)], func,
                            bias=bias_sb[:, bias_col + mt : bias_col + mt + 1],
                            scale=1.0,
                        )
                        outs[s][mt] = ot
                return outs

            # ---------- encoder head ----------
            xin = {s: [] for s in range(BS)}
            for s in range(BS):
                for c in range(4):
                    cs = min(128, F - c * 128)
                    t_in = temp.tile([cs, T], dtm, tag=f"sm_{s}_{c}", name=f"xin_{s}_{c}")
                    nc.sync.dma_start(t_in, d_x0[s, c * 128 : c * 128 + cs, :])
                    xin[s].append(t_in)

            xtag = lambda s, mt: f"x_{s}_{mt}"
            gatag = lambda s, mt: f"ga_{s}_{mt}"
            gbtag = lambda s, mt: f"gb_{s}_{mt}"
            h1 = conv_block(w_enc0_sb, xin, AF.Relu, COL_ENC0, [0, 1, 4, 5], temp, gatag, uid="h1")
            h2 = conv_block(w_enc1_sb, h1, AF.Relu, COL_ENC1, [2, 3, 6, 7], temp, gbtag, uid="h2")
            xs = conv_block(w_enc2_sb, h2, AF.Identity, COL_ENC2, [0, 1, 4, 5], xpool, xtag,
                            bufs=3, uid="xe")

            # ---------- highway stack helper ----------
            def make_x2(xs_cur, uid):
                # right-shifted copy (col0 = 0): makes the odd-dilation middle
                # conv tap even-aligned for the f32r matmul mode.
                out = {s: [None, None] for s in range(BS)}
                for c in range(2):
                    for s in range(BS):
                        x2 = xpool.tile([128, T], dtm, tag=f"x2_{s}_{c}", bufs=2,
                                        name=f"x2_{uid}_{s}_{c}")
                        nc.vector.tensor_scalar_mul(x2[:, 0:1], xs_cur[s][c][:, 0:1], 0.0)
                        nc.vector.tensor_copy(x2[:, 1:T], xs_cur[s][c][:, 0 : T - 1])
                        out[s][c] = x2
                return out

            def highway_layers(l_lo, l_hi, x2s):
                nonlocal xs
                for l in range(l_lo, l_hi):
                    dil = DIL[l]
                    wt = wstream.tile([128, 2, 3, 4, 128], dtm, tag="hww", bufs=2, name=f"hw_w_{l}")
                    nc.sync.dma_start(wt, d_hw_w[l])
                    last_enc = l == 9
                    ps_all = {}
                    for mt in range(4):
                        for s in range(BS):
                            ps_all[(mt, s)] = psum.tile(
                                [128, T], F32, tag=f"bank{mt * 2 + s}",
                                name=f"hwps_{l}_{mt}_{s}",
                            )
                    # gate halves (mt 2,3) first so sigmoids can start early
                    for mt in (2, 3, 0, 1):
                        for ki, k in enumerate((2, 1, 0)):
                            for kc in range(2):
                                first = ki == 0 and kc == 0
                                last = ki == 2 and kc == 1
                                for s in range(BS):
                                    ps = ps_all[(mt, s)]
                                    if k == 2:
                                        _mm(nc, ps, wt[:, kc, k, mt, :], xs[s][kc],
                                            start=first, stop=last)
                                    elif k == 1:
                                        # y[t] += W1 @ x[t-d] via x2 (x shifted +1)
                                        off = dil - 1
                                        _mm(nc, ps[:, off:T], wt[:, kc, k, mt, :],
                                            x2s[s][kc][:, 0 : T - off],
                                            start=first, stop=last)
                                    else:
                                        sh = 2 * dil
                                        _mm(nc, ps[:, sh:T], wt[:, kc, k, mt, :],
                                            xs[s][kc][:, 0 : T - sh],
                                            start=first, stop=last)
                    # epilogue: x' = x + sigmoid(g) * (a - x)
                    tgs, tmps = {}, {}
                    for c in range(2):
                        for s in range(BS):
                            tg = temp.tile([128, T], dt, tag=f"tg_{s}_{c}", bufs=2,
                                           name=f"tg_{l}_{s}_{c}")
                            nc.scalar.activation(
                                tg, ps_all[(c + 2, s)], AF.Sigmoid,
                                bias=hwb_sb[:, l, c + 2 : c + 3], scale=1.0)
                            tgs[(s, c)] = tg
                    for c in range(2):
                        for s in range(BS):
                            tmp = temp.tile([128, T], dt, tag=f"tmp_{s}_{c}", bufs=2,
                                            name=f"tmp_{l}_{s}_{c}")
                            nc.vector.scalar_tensor_tensor(
                                tmp, ps_all[(c, s)], hwb_sb[:, l, c : c + 1],
                                xs[s][c], op0=ALU.add, op1=ALU.subtract)
                            tmps[(s, c)] = tmp
                    for c in range(2):
                        for s in range(BS):
                            nc.vector.tensor_mul(tmps[(s, c)], tgs[(s, c)], tmps[(s, c)])
                    newxs = {s: [None, None] for s in range(BS)}
                    for c in range(2):
                        for s in range(BS):
                            if last_enc:
                                xn = persist.tile([128, T], dtm, tag=f"q_{s}_{c}",
                                                  name=f"Q_{s}_{c}")
                            else:
                                xn = xpool.tile([128, T], dtm, tag=f"x_{s}_{c}", bufs=3,
                                                name=f"x_{l + 1}_{s}_{c}")
                            nc.gpsimd.tensor_add(xn, tmps[(s, c)], xs[s][c])
                            newxs[s][c] = xn
                    xs = newxs
                    if l + 1 < l_hi:
                        x2s = make_x2(xs, f"l{l + 1}")

            # ---------- encoder highway ----------
            highway_layers(0, 10, make_x2(xs, "e0"))
            Qs = xs  # persisted encoder output (B, D, T) tiles

            # ---------- attention ----------
            Rqs = {s: [] for s in range(BS)}
            q_sb = {s: [] for s in range(BS)}
            for mt in range(2):
                for s in range(BS):
                    ps = psum.tile([128, T], F32, tag=f"bank{s * 4 + mt}",
                                   name=f"qps_{s}_{mt}")
                    for kc in range(2):
                        _mm(nc, ps, w_q_sb[kc][:, mt * 128 : (mt + 1) * 128], Qs[s][kc],
                            start=(kc == 0), stop=(kc == 1))
                    qt = temp.tile([128, T], dtm, tag=f"ga_{s}_{mt}", name=f"q_{s}_{mt}")
                    nc.scalar.activation(
                        qt, ps, AF.Identity,
                        bias=bias_sb[:, COL_QB + mt : COL_QB + mt + 1], scale=1.0)
                    q_sb[s].append(qt)

            att_tiles = {}
            for s in range(BS):
                for tt in range(4):
                    ps = psum.tile([128, TE], F32, tag=f"bank{s * 4 + tt}",
                                   name=f"sps_{s}_{tt}")
                    for dc in range(2):
                        _mm(nc, ps, q_sb[s][dc][:, tt * 128 : (tt + 1) * 128],
                            kT_sb[(s, dc)], start=(dc == 0), stop=(dc == 1))
                    st = stat_sb[:, (s * 4 + tt) * 4 : (s * 4 + tt) * 4 + 4]
                    nc.vector.reduce_max(st[:, 0:1], ps, axis=AX.X, negate=True)
                    at = temp.tile([128, TE], dt, tag=f"sm_{s}_{tt}",
                                   name=f"att_{s}_{tt}")
                    nc.scalar.activation(at, ps, AF.Exp, bias=st[:, 0:1], scale=1.0,
                                         accum_out=st[:, 1:2])
                    nc.vector.reciprocal(st[:, 2:3], st[:, 1:2])
                    nc.vector.tensor_scalar_mul(at, at, st[:, 2:3])
                    nc.sync.dma_start(d_attn[s, tt * 128 : (tt + 1) * 128, :], at)
                    att_tiles[(s, tt)] = at

            aT = {s: [] for s in range(BS)}
            for s in range(BS):
                for sc in range(2):
                    pst = psum.tile([128, T], F32, tag=f"bank{s * 4 + sc}",
                                    name=f"tps_{s}_{sc}")
                    for tt in range(4):
                        nc.tensor.matmul(
                            pst[:, tt * 128 : (tt + 1) * 128],
                            att_tiles[(s, tt)][:, sc * 128 : (sc + 1) * 128],
                            ident, is_transpose=True, start=True, stop=True,
                            skip_group_check=True)
                    a2 = temp.tile([128, T], dtm, tag=f"gb_{s}_{sc}", name=f"aT_{s}_{sc}")
                    nc.vector.tensor_copy(a2, pst)
                    aT[s].append(a2)

            ctx_sb = {s: [] for s in range(BS)}
            for s in range(BS):
                for dc in range(2):
                    ps = psum.tile([128, T], F32, tag=f"bank{s * 4 + 2 + dc}",
                                   name=f"cps_{s}_{dc}")
                    for sc in range(2):
                        _mm(nc, ps, v_sb[(s, sc)][:, dc * 128 : (dc + 1) * 128],
                            aT[s][sc], start=(sc == 0), stop=(sc == 1))
                    ct = temp.tile([128, T], dtm, tag=f"ga_{s}_{dc}", name=f"ctx_{s}_{dc}")
                    nc.vector.tensor_copy(ct, ps)
                    ctx_sb[s].append(ct)

            for mt in range(2):
                for s in range(BS):
                    ps = psum.tile([128, T], F32, tag=f"bank{s * 4 + mt}",
                                   name=f"ops_{s}_{mt}")
                    for dc in range(2):
                        _mm(nc, ps, w_o_sb[dc][:, mt * 128 : (mt + 1) * 128],
                            ctx_sb[s][dc], start=(dc == 0), stop=(dc == 1))
                    tmpo = temp.tile([128, T], dt, tag=f"gb_{s}_{mt}",
                                     name=f"tmpo_{s}_{mt}")
                    nc.scalar.activation(
                        tmpo, ps, AF.Identity,
                        bias=bias_sb[:, COL_OB + mt : COL_OB + mt + 1], scale=1.0)
                    rq = persist.tile([128, T], dtm, tag=f"rq_{s}_{mt}",
                                      name=f"rq_{s}_{mt}")
                    # Rq = sqrt(.5)*query + (sqrt(.5)*(out_proj)) [scales folded into w_o/b_o]
                    nc.vector.scalar_tensor_tensor(
                        rq, Qs[s][mt], SQ2, tmpo, op0=ALU.mult, op1=ALU.add)
                    Rqs[s].append(rq)

            # ---------- decoder ----------
            dec_in = {s: [Rqs[s][0], Rqs[s][1], Qs[s][0], Qs[s][1]] for s in range(BS)}
            xs = conv_block(w_dec0_sb, dec_in, AF.Identity, COL_DEC0, [2, 3, 6, 7],
                            xpool, xtag, bufs=3, uid="xd0")
            highway_layers(10, 16, make_x2(xs, "d0"))
            xs = conv_block(w_dec1_sb, xs, AF.Relu, COL_DEC1, [0, 1, 4, 5], xpool, xtag,
                            bufs=3, uid="xd1")
            xs = conv_block(w_dec2_sb, xs, AF.Relu, COL_DEC2, [2, 3, 6, 7], xpool, xtag,
                            bufs=3, uid="xd2")
            xs = conv_block(w_dec3_sb, xs, AF.Relu, COL_DEC3, [0, 1, 4, 5], xpool, xtag,
                            bufs=3, uid="xd3")

            # ---------- final: mel (sigmoid conv) + done, fused ----------
            for s in range(BS):
                for tt in range(4):
                    ps = psum.tile([128, F + 2], F32, tag=f"bank{s * 4 + tt}",
                                   name=f"fps_{s}_{tt}")
                    for dc in range(2):
                        _mm(nc, ps, xs[s][dc][:, tt * 128 : (tt + 1) * 128],
                            w_last_sb[dc], start=(dc == 0), stop=False)
                    _mm(nc, ps, ones_row, blast_sb, start=False, stop=True)
                    fo = temp.tile([128, F + 2], dt, tag=f"sm_{s}_{tt}",
                                   name=f"fin_{s}_{tt}")
                    nc.scalar.activation(fo, ps, AF.Sigmoid, scale=1.0)
                    nc.sync.dma_start(d_mel[s, tt * 128 : (tt + 1) * 128, :], fo[:, 0:F])
                    nc.sync.dma_start(d_done[s, tt * 128 : (tt + 1) * 128, :],
                                      fo[:, F : F + 1])

    nc.compile()
    return nc


def _prep_host(inputs):
    """Host-side packing: transposes and per-layer weight layout."""
    f32 = np.float32

    def npf(a):
        return np.ascontiguousarray(np.asarray(a), dtype=f32)

    x0 = npf(np.asarray(inputs["inputs"]).transpose(0, 2, 1))  # (B, F, T)
    keysT = npf(np.asarray(inputs["keys"]).transpose(0, 2, 1))  # (B, D, TE)
    values = npf(inputs["values"])  # (B, TE, D)

    w_all = np.concatenate([np.asarray(inputs["enc_hw_w"]),
                            np.asarray(inputs["dec_hw_w"])], axis=0)  # (16, 512, 256, 3)
    wt = w_all.transpose(0, 2, 1, 3)            # (L, ci, co, k)
    wt = wt.reshape(L, 2, 128, 4, 128, 3)       # (L, kc, p, mt, f, k)
    hw_w = npf(wt.transpose(0, 2, 1, 5, 3, 4))  # (L, 128, kc, k, mt, f)

    b_all = np.concatenate([np.asarray(inputs["enc_hw_b"]),
                            np.asarray(inputs["dec_hw_b"])], axis=0)  # (16, 512)
    hw_b = npf(b_all.reshape(L, 4, 128).transpose(2, 0, 1))  # (128, L, 4)

    def t2(w):  # (O, I, 1) -> (I, O)
        return npf(np.asarray(w)[:, :, 0].T)

    w_enc0 = t2(inputs["enc_w0"])
    w_enc1 = t2(inputs["enc_w1"])
    w_enc2 = t2(inputs["enc_w2"])
    w_q = npf(np.asarray(inputs["attn_q_w"]).T)
    w_o = npf(np.asarray(inputs["attn_o_w"]).T * (math.sqrt(TE) * SQ2))
    w_dec0 = t2(inputs["dec_w0"])
    w_dec1 = t2(inputs["dec_w1"])
    w_dec2 = t2(inputs["dec_w2"])
    w_dec3 = t2(inputs["dec_w3"])
    w_last = npf(np.concatenate(
        [np.asarray(inputs["last_w"])[:, :, 0].T, np.asarray(inputs["fc_w"]).T,
         np.zeros((D, 1), np.float32)], axis=1))
    b_last = np.zeros((2, F + 2), np.float32)
    b_last[0, :F] = np.asarray(inputs["last_b"])
    b_last[0, F] = np.asarray(inputs["fc_b"])[0]
    b_last = npf(b_last)

    def cols(v):  # (256,) -> (128, 2)
        return np.asarray(v, dtype=f32).reshape(2, 128).T

    bias_tbl = np.zeros((128, NB), dtype=f32)
    bias_tbl[:, COL_ENC0:COL_ENC0 + 2] = cols(inputs["enc_b0"])
    bias_tbl[:, COL_ENC1:COL_ENC1 + 2] = cols(inputs["enc_b1"])
    bias_tbl[:, COL_ENC2:COL_ENC2 + 2] = cols(inputs["enc_b2"])
    bias_tbl[:, COL_QB:COL_QB + 2] = cols(inputs["attn_q_b"])
    bias_tbl[:, COL_OB:COL_OB + 2] = cols(np.asarray(inputs["attn_o_b"]) * SQ2)
    bias_tbl[:, COL_DEC0:COL_DEC0 + 2] = cols(inputs["dec_b0"])
    bias_tbl[:, COL_DEC1:COL_DEC1 + 2] = cols(inputs["dec_b1"])
    bias_tbl[:, COL_DEC2:COL_DEC2 + 2] = cols(inputs["dec_b2"])
    bias_tbl[:, COL_DEC3:COL_DEC3 + 2] = cols(inputs["dec_b3"])
    bias_tbl = npf(bias_tbl)

    shared = dict(ones=np.stack([np.ones(128, f32), np.zeros(128, f32)]), hw_w=hw_w, hw_b=hw_b, w_enc0=w_enc0, w_enc1=w_enc1, w_enc2=w_enc2,
                  w_q=w_q, w_o=w_o, w_dec0=w_dec0, w_dec1=w_dec1, w_dec2=w_dec2,
                  w_dec3=w_dec3, w_last=w_last, b_last=b_last, bias_tbl=bias_tbl)

    in_maps = []
    for i in range(N_CORES):
        sl = slice(i * BS, (i + 1) * BS)
        m = dict(shared)
        m["x0"] = npf(x0[sl])
        m["keysT"] = npf(keysT[sl])
        m["values"] = npf(values[sl])
        in_maps.append(m)
    return in_maps


def kernel(**inputs):
    global LAST_EXEC_NS
    if "nc" not in _BUILD_CACHE:
        _BUILD_CACHE["nc"] = _build()
    nc = _BUILD_CACHE["nc"]

    in_maps = _prep_host(inputs)

    trace = os.environ.get("KBENCH_TRACE", "0") == "1"
    if trace:
        _install_ntff_hook()
    res = run_bass_kernel_spmd(nc, in_maps, core_ids=list(range(N_CORES)), trace=trace)
    LAST_EXEC_NS = res.exec_time_ns

    mel = np.concatenate([r["mel"] for r in res.results], axis=0)
    attn = np.concatenate([r["attn"] for r in res.results], axis=0)
    done = np.concatenate([r["done"] for r in res.results], axis=0)
    return mel, attn, done


def _install_ntff_hook():
    """Register the axon NTFF profiling hook (missing from this image's antenv)."""
    import types

    if "antenv.axon_hooks" in sys.modules:
        return
    m = types.ModuleType("antenv.axon_hooks")
    m._h = None
    m.set_axon_ntff_profile_hook = lambda h: setattr(m, "_h", h)
    m.get_axon_ntff_profile_hook = lambda: m._h
    sys.modules["antenv.axon_hooks"] = m
    try:
        import antenv

        antenv.axon_hooks = m
        from trn_agent_boot.trn_boot import _ntff_profile_via_ctypes

        m._h = _ntff_profile_via_ctypes("/opt/axon/libaxon_pjrt.so")
    except Exception:
        m._h = None


# revision 17
# speedup vs baseline: 1.2419x; 1.2419x over previous
"""Trainium2 Bass kernel for nn_Decoder_42417097016016 (DCTTS-style decoder).

Sharding: pure data parallel over batch. B=16 samples -> 8 NeuronCores x 2
samples each; all weights replicated per core.

Layout: activations live on-chip as (channels, time) so every conv1x1 /
causal conv is a PE matmul with channels on partitions.  Causal convs with
dilation d are 3 accumulating matmuls per output tile with column shifts
(0, d, 2d) - left zero-padding falls out of the shifted PSUM accumulation.

The attention block computes scores (t,s), softmax along free dim (ACT Exp
with accum_out row sums), writes the normalized attn output directly, and
PE-transposes it to (s,t) for the context matmul.  mel and done share one
final matmul by concatenating fc_w as a 401st output column (padded to 402).

Matmul precision knob KBENCH_MM: "f16" (default, full PE rate), "f32r"
(reduced-precision fp32, half rate, needs even matmul geometry -> shifted x2
copies for odd-dilation taps), "f32" (exact, quarter rate).
"""

import math
import os
import sys

import numpy as np

for _p in ("/opt/trn_rl_repo", "/root/.axon_site/_ro/trn_rl_repo"):
    if os.path.isdir(_p) and _p not in sys.path:
        sys.path.append(_p)

import concourse.bass as bass
import concourse.tile as tile
from concourse import bacc, mybir
from concourse.bass_utils import run_bass_kernel_spmd

AF = mybir.ActivationFunctionType
ALU = mybir.AluOpType
AX = mybir.AxisListType
F32 = mybir.dt.float32
F32R = mybir.dt.float32r
F16 = mybir.dt.float16

N_CORES = 8
B, T, TE, D, F = 16, 512, 256, 256, 400
BS = B // N_CORES  # samples per core
DIL = [1, 3, 9, 27, 1, 3, 9, 27, 3, 3] + [1, 3, 9, 27, 1, 1]
L = len(DIL)  # 16 highway layers (10 encoder + 6 decoder)
SQ2 = math.sqrt(0.5)

MM_DT = os.environ.get("KBENCH_MM", "f16")

# bias table column assignments
COL_ENC0, COL_ENC1, COL_ENC2 = 0, 2, 4
COL_QB, COL_OB = 6, 8
COL_DEC0, COL_DEC1, COL_DEC2, COL_DEC3 = 10, 12, 14, 16
NB = 18

LAST_EXEC_NS = None
_BUILD_CACHE = {}


def _mm(nc, out, lhsT, rhs, **kw):
    nc.tensor.matmul(out, lhsT, rhs, **kw)


def _build():
    """Build the per-core Bass program (same program on all 8 cores)."""
    from concourse.masks import make_identity

    nc = bacc.Bacc("TRN2", target_bir_lowering=False, debug=False)
    dt = F32
    dtm = {"f16": F16, "f32r": F32R, "f32": F32}[MM_DT]
    use_x2 = MM_DT == "f32r"  # f32r matmuls need even offsets/N

    # ---- DRAM I/O (per-core shard shapes) ----
    d_x0 = nc.dram_tensor("x0", [BS, F, T], dtm, kind="ExternalInput").ap()
    d_keysT = nc.dram_tensor("keysT", [BS, D, TE], dtm, kind="ExternalInput").ap()
    d_values = nc.dram_tensor("values", [BS, TE, D], dtm, kind="ExternalInput").ap()
    d_hw_w = nc.dram_tensor("hw_w", [L, 128, 2, 3, 4, 128], dtm, kind="ExternalInput").ap()
    d_hw_b = nc.dram_tensor("hw_b", [128, L, 4], dt, kind="ExternalInput").ap()
    d_w_enc0 = nc.dram_tensor("w_enc0", [F, D], dtm, kind="ExternalInput").ap()
    d_w_enc1 = nc.dram_tensor("w_enc1", [D, D], dtm, kind="ExternalInput").ap()
    d_w_enc2 = nc.dram_tensor("w_enc2", [D, D], dtm, kind="ExternalInput").ap()
    d_w_q = nc.dram_tensor("w_q", [D, D], dtm, kind="ExternalInput").ap()
    d_w_o = nc.dram_tensor("w_o", [D, D], dtm, kind="ExternalInput").ap()
    d_w_dec0 = nc.dram_tensor("w_dec0", [2 * D, D], dtm, kind="ExternalInput").ap()
    d_w_dec1 = nc.dram_tensor("w_dec1", [D, D], dtm, kind="ExternalInput").ap()
    d_w_dec2 = nc.dram_tensor("w_dec2", [D, D], dtm, kind="ExternalInput").ap()
    d_w_dec3 = nc.dram_tensor("w_dec3", [D, D], dtm, kind="ExternalInput").ap()
    d_w_last = nc.dram_tensor("w_last", [D, F + 2], dtm, kind="ExternalInput").ap()
    d_b_last = nc.dram_tensor("b_last", [2, F + 2], dtm, kind="ExternalInput").ap()
    d_ones = nc.dram_tensor("ones", [2, 128], dtm, kind="ExternalInput").ap()
    d_bias = nc.dram_tensor("bias_tbl", [128, NB], dt, kind="ExternalInput").ap()

    d_mel = nc.dram_tensor("mel", [BS, T, F], dt, kind="ExternalOutput").ap()
    d_attn = nc.dram_tensor("attn", [BS, T, TE], dt, kind="ExternalOutput").ap()
    d_done = nc.dram_tensor("done", [BS, T, 1], dt, kind="ExternalOutput").ap()

    with tile.TileContext(nc) as tc:
        with (
            tc.tile_pool(name="const", bufs=1) as const,
            tc.tile_pool(name="xpool", bufs=1) as xpool,
            tc.tile_pool(name="persist", bufs=1) as persist,
            tc.tile_pool(name="wstream", bufs=1) as wstream,
            tc.tile_pool(name="temp", bufs=1) as temp,
            tc.tile_pool(name="psum", bufs=1, space="PSUM") as psum,
        ):
            # ---------- startup-critical loads first (DMA queue is FIFO) ----
            w_enc0_sb = []
            for c in range(4):
                cs = min(128, F - c * 128)
                t_ = const.tile([cs, D], dtm, name=f"wenc0_{c}")
                nc.sync.dma_start(t_, d_w_enc0[c * 128 : c * 128 + cs, :])
                w_enc0_sb.append(t_)
            xin = {s: [] for s in range(BS)}
            for s in range(BS):
                for c in range(4):
                    cs = min(128, F - c * 128)
                    t_in = temp.tile([cs, T], dtm, tag=f"sm_{s}_{c}", name=f"xin_{s}_{c}")
                    nc.sync.dma_start(t_in, d_x0[s, c * 128 : c * 128 + cs, :])
                    xin[s].append(t_in)

            def load_w(dram, nm, rows):
                tiles = []
                nchunk = (rows + 127) // 128
                cols = dram.shape[1]
                for c in range(nchunk):
                    cs = min(128, rows - c * 128)
                    t_ = const.tile([cs, cols], dtm, name=f"{nm}_{c}")
                    nc.sync.dma_start(t_, dram[c * 128 : c * 128 + cs, :])
                    tiles.append(t_)
                return tiles

            w_enc1_sb = load_w(d_w_enc1, "wenc1", D)
            w_enc2_sb = load_w(d_w_enc2, "wenc2", D)
            bias_sb = const.tile([128, NB], dt, name="bias_sb")
            nc.sync.dma_start(bias_sb, d_bias)
            hwb_sb = const.tile([128, L, 4], dt, name="hwb_sb")
            nc.sync.dma_start(hwb_sb, d_hw_b)

            ident = const.tile([128, 128], dt, name="ident")
            make_identity(nc, ident)
            stat_sb = const.tile([128, 32], F32, name="stat_sb")

            # helper: one conv1x1 block (256 outputs) for all samples.
            # Epilogue on DVE: out = relu?(psum + bias)
            def conv_block(w_tiles, rhs_per_s, relu, bias_col, banks, out_pool,
                           tag_fn, bufs=1, uid=""):
                outs = {s: [None, None] for s in range(BS)}
                pss = {}
                for mt in range(2):
                    for s in range(BS):
                        ps = psum.tile(
                            [128, T], F32,
                            tag=f"bank{banks[s * 2 + mt]}",
                            name=f"ps_{uid}_{s}_{mt}",
                        )
                        nk = len(w_tiles)
                        for c in range(nk):
                            _mm(nc, ps, w_tiles[c][:, mt * 128 : (mt + 1) * 128],
                                rhs_per_s[s][c], start=(c == 0), stop=(c == nk - 1))
                        pss[(s, mt)] = ps
                for mt in range(2):
                    for s in range(BS):
                        ot = out_pool.tile(
                            [128, T], dtm,
                            tag=tag_fn(s, mt), bufs=bufs,
                            name=f"{uid}_{s}_{mt}",
                        )
                        b_ap = bias_sb[:, bias_col + mt : bias_col + mt + 1]
                        if relu:
                            nc.vector.tensor_scalar(ot, pss[(s, mt)], b_ap, 0.0,
                                                    op0=ALU.add, op1=ALU.max)
                        else:
                            nc.vector.tensor_scalar_add(ot, pss[(s, mt)], b_ap)
                        outs[s][mt] = ot
                return outs

            xtag = lambda s, mt: f"x_{s}_{mt}"
            gatag = lambda s, mt: f"ga_{s}_{mt}"
            gbtag = lambda s, mt: f"gb_{s}_{mt}"

            # ---------- encoder head ----------
            h1 = conv_block(w_enc0_sb, xin, True, COL_ENC0, [0, 1, 4, 5], temp, gatag, uid="h1")
            h2 = conv_block(w_enc1_sb, h1, True, COL_ENC1, [2, 3, 6, 7], temp, gbtag, uid="h2")
            xs = conv_block(w_enc2_sb, h2, False, COL_ENC2, [0, 1, 4, 5], xpool, xtag,
                            bufs=3, uid="xe")

            # ---------- remaining const loads (drain behind encoder work) ----
            w_q_sb = load_w(d_w_q, "wq", D)
            w_o_sb = load_w(d_w_o, "wo", D)
            w_dec0_sb = load_w(d_w_dec0, "wdec0", 2 * D)
            w_dec1_sb = load_w(d_w_dec1, "wdec1", D)
            w_dec2_sb = load_w(d_w_dec2, "wdec2", D)
            w_dec3_sb = load_w(d_w_dec3, "wdec3", D)
            w_last_sb = load_w(d_w_last, "wlast", D)
            ones_row = const.tile([2, 128], dtm, name="ones_row")
            nc.sync.dma_start(ones_row, d_ones)
            blast_sb = const.tile([2, F + 2], dtm, name="blast_sb")
            nc.sync.dma_start(blast_sb, d_b_last)
            kT_sb, v_sb = {}, {}
            for s in range(BS):
                for c in range(2):
                    kt = const.tile([128, TE], dtm, name=f"keysT_{s}_{c}")
                    nc.sync.dma_start(kt, d_keysT[s, c * 128 : (c + 1) * 128, :])
                    kT_sb[(s, c)] = kt
                    vt = const.tile([128, D], dtm, name=f"values_{s}_{c}")
                    nc.sync.dma_start(vt, d_values[s, c * 128 : (c + 1) * 128, :])
                    v_sb[(s, c)] = vt

            # ---------- highway stack ----------
            def make_x2(xs_cur, uid):
                # right-shifted copy (col0 = 0): makes the odd-dilation middle
                # conv tap even-aligned for the f32r matmul mode.
                out = {s: [None, None] for s in range(BS)}
                for c in range(2):
                    for s in range(BS):
                        x2 = xpool.tile([128, T], dtm, tag=f"x2_{s}_{c}", bufs=2,
                                        name=f"x2_{uid}_{s}_{c}")
                        nc.gpsimd.tensor_scalar_mul(x2[:, 0:1], xs_cur[s][c][:, 0:1], 0.0)
                        nc.gpsimd.tensor_copy(x2[:, 1:T], xs_cur[s][c][:, 0 : T - 1])
                        out[s][c] = x2
                return out

            def highway_layers(l_lo, l_hi, x2s):
                nonlocal xs
                for l in range(l_lo, l_hi):
                    dil = DIL[l]
                    wt = wstream.tile([128, 2, 3, 4, 128], dtm, tag="hww", bufs=2,
                                      name=f"hw_w_{l}")
                    nc.sync.dma_start(wt, d_hw_w[l])
                    last_enc = l == 9
                    ps_all = {}
                    for mt in range(4):
                        for s in range(BS):
                            ps_all[(mt, s)] = psum.tile(
                                [128, T], F32, tag=f"bank{mt * 2 + s}",
                                name=f"hwps_{l}_{mt}_{s}",
                            )
                    # mt order (2,0,3,1): gate+input for chunk 0 finish first so
                    # its epilogue chain overlaps the remaining matmuls.
                    # kc0 taps before kc1 so the next layer can start on x'[0].
                    for mt in (2, 0, 3, 1):
                        idx = 0
                        for kc in range(2):
                            for k in (2, 1, 0):
                                first = idx == 0
                                last = idx == 5
                                idx += 1
                                for s in range(BS):
                                    ps = ps_all[(mt, s)]
                                    if k == 2:
                                        _mm(nc, ps, wt[:, kc, k, mt, :], xs[s][kc],
                                            start=first, stop=last)
                                    elif k == 1:
                                        if use_x2:
                                            off = dil - 1
                                            _mm(nc, ps[:, off:T], wt[:, kc, k, mt, :],
                                                x2s[s][kc][:, 0 : T - off],
                                                start=first, stop=last)
                                        else:
                                            _mm(nc, ps[:, dil:T], wt[:, kc, k, mt, :],
                                                xs[s][kc][:, 0 : T - dil],
                                                start=first, stop=last)
                                    else:
                                        sh = 2 * dil
                                        _mm(nc, ps[:, sh:T], wt[:, kc, k, mt, :],
                                            xs[s][kc][:, 0 : T - sh],
                                            start=first, stop=last)
                    # epilogue: x' = x + sigmoid(g) * (a - x)
                    # per chunk c: g = bank (c+2), a = bank c
                    newxs = {s: [None, None] for s in range(BS)}
                    for c in range(2):
                        tgs = {}
                        for s in range(BS):
                            tg = temp.tile([128, T], dtm, tag=f"tg_{s}_{c}", bufs=2,
                                           name=f"tg_{l}_{s}_{c}")
                            nc.scalar.activation(
                                tg, ps_all[(c + 2, s)], AF.Sigmoid,
                                bias=hwb_sb[:, l, c + 2 : c + 3], scale=1.0)
                            tgs[s] = tg
                        tmps = {}
                        for s in range(BS):
                            tmp = temp.tile([128, T], dtm, tag=f"tmp_{s}_{c}", bufs=2,
                                            name=f"tmp_{l}_{s}_{c}")
                            nc.vector.scalar_tensor_tensor(
                                tmp, ps_all[(c, s)], hwb_sb[:, l, c : c + 1],
                                xs[s][c], op0=ALU.add, op1=ALU.subtract)
                            tmps[s] = tmp
                        for s in range(BS):
                            nc.vector.tensor_mul(tmps[s], tgs[s], tmps[s])
                        for s in range(BS):
                            if last_enc:
                                xn = persist.tile([128, T], dtm, tag=f"q_{s}_{c}",
                                                  name=f"Q_{s}_{c}")
                            else:
                                xn = xpool.tile([128, T], dtm, tag=f"x_{s}_{c}", bufs=3,
                                                name=f"x_{l + 1}_{s}_{c}")
                            nc.gpsimd.tensor_add(xn, tmps[s], xs[s][c])
                            newxs[s][c] = xn
                    xs = newxs
                    if use_x2 and l + 1 < l_hi:
                        x2s = make_x2(xs, f"l{l + 1}")

            # ---------- encoder highway ----------
            highway_layers(0, 10, make_x2(xs, "e0") if use_x2 else None)
            Qs = xs  # persisted encoder output (D, T) tiles

            # ---------- attention ----------
            Rqs = {s: [] for s in range(BS)}
            q_sb = {s: [] for s in range(BS)}
            for mt in range(2):
                for s in range(BS):
                    ps = psum.tile([128, T], F32, tag=f"bank{s * 4 + mt}",
                                   name=f"qps_{s}_{mt}")
                    for kc in range(2):
                        _mm(nc, ps, w_q_sb[kc][:, mt * 128 : (mt + 1) * 128], Qs[s][kc],
                            start=(kc == 0), stop=(kc == 1))
                    qt = temp.tile([128, T], dtm, tag=f"ga_{s}_{mt}", name=f"q_{s}_{mt}")
                    nc.vector.tensor_scalar_add(
                        qt, ps, bias_sb[:, COL_QB + mt : COL_QB + mt + 1])
                    q_sb[s].append(qt)

            att_tiles = {}
            for s in range(BS):
                for tt in range(4):
                    ps = psum.tile([128, TE], F32, tag=f"bank{s * 4 + tt}",
                                   name=f"sps_{s}_{tt}")
                    for dc in range(2):
                        _mm(nc, ps, q_sb[s][dc][:, tt * 128 : (tt + 1) * 128],
                            kT_sb[(s, dc)], start=(dc == 0), stop=(dc == 1))
                    st = stat_sb[:, (s * 4 + tt) * 4 : (s * 4 + tt) * 4 + 4]
                    nc.vector.reduce_max(st[:, 0:1], ps, axis=AX.X, negate=True)
                    at = temp.tile([128, TE], dt, tag=f"sm_{s}_{tt}",
                                   name=f"att_{s}_{tt}")
                    nc.scalar.activation(at, ps, AF.Exp, bias=st[:, 0:1], scale=1.0,
                                         accum_out=st[:, 1:2])
                    nc.vector.reciprocal(st[:, 2:3], st[:, 1:2])
                    nc.vector.tensor_scalar_mul(at, at, st[:, 2:3])
                    nc.sync.dma_start(d_attn[s, tt * 128 : (tt + 1) * 128, :], at)
                    att_tiles[(s, tt)] = at

            aT = {s: [] for s in range(BS)}
            for s in range(BS):
                for sc in range(2):
                    pst = psum.tile([128, T], F32, tag=f"bank{s * 4 + sc}",
                                    name=f"tps_{s}_{sc}")
                    for tt in range(4):
                        nc.tensor.matmul(
                            pst[:, tt * 128 : (tt + 1) * 128],
                            att_tiles[(s, tt)][:, sc * 128 : (sc + 1) * 128],
                            ident, is_transpose=True, start=True, stop=True,
                            skip_group_check=True)
                    a2 = temp.tile([128, T], dtm, tag=f"gb_{s}_{sc}", name=f"aT_{s}_{sc}")
                    nc.vector.tensor_copy(a2, pst)
                    aT[s].append(a2)

            ctx_sb = {s: [] for s in range(BS)}
            for s in range(BS):
                for dc in range(2):
                    ps = psum.tile([128, T], F32, tag=f"bank{s * 4 + 2 + dc}",
                                   name=f"cps_{s}_{dc}")
                    for sc in range(2):
                        _mm(nc, ps, v_sb[(s, sc)][:, dc * 128 : (dc + 1) * 128],
                            aT[s][sc], start=(sc == 0), stop=(sc == 1))
                    ct = temp.tile([128, T], dtm, tag=f"ga_{s}_{dc}", name=f"ctx_{s}_{dc}")
                    nc.vector.tensor_copy(ct, ps)
                    ctx_sb[s].append(ct)

            for mt in range(2):
                for s in range(BS):
                    ps = psum.tile([128, T], F32, tag=f"bank{s * 4 + mt}",
                                   name=f"ops_{s}_{mt}")
                    for dc in range(2):
                        _mm(nc, ps, w_o_sb[dc][:, mt * 128 : (mt + 1) * 128],
                            ctx_sb[s][dc], start=(dc == 0), stop=(dc == 1))
                    tmpo = temp.tile([128, T], dt, tag=f"gb_{s}_{mt}",
                                     name=f"tmpo_{s}_{mt}")
                    nc.vector.tensor_scalar_add(
                        tmpo, ps, bias_sb[:, COL_OB + mt : COL_OB + mt + 1])
                    rq = persist.tile([128, T], dtm, tag=f"rq_{s}_{mt}",
                                      name=f"rq_{s}_{mt}")
                    # Rq = sqrt(.5)*query + out_proj  [scales folded into w_o/b_o]
                    nc.vector.scalar_tensor_tensor(
                        rq, Qs[s][mt], SQ2, tmpo, op0=ALU.mult, op1=ALU.add)
                    Rqs[s].append(rq)

            # ---------- decoder ----------
            dec_in = {s: [Rqs[s][0], Rqs[s][1], Qs[s][0], Qs[s][1]] for s in range(BS)}
            xs = conv_block(w_dec0_sb, dec_in, False, COL_DEC0, [2, 3, 6, 7],
                            xpool, xtag, bufs=3, uid="xd0")
            highway_layers(10, 16, make_x2(xs, "d0") if use_x2 else None)
            xs = conv_block(w_dec1_sb, xs, True, COL_DEC1, [0, 1, 4, 5], xpool, xtag,
                            bufs=3, uid="xd1")
            xs = conv_block(w_dec2_sb, xs, True, COL_DEC2, [2, 3, 6, 7], xpool, xtag,
                            bufs=3, uid="xd2")
            xs = conv_block(w_dec3_sb, xs, True, COL_DEC3, [0, 1, 4, 5], xpool, xtag,
                            bufs=3, uid="xd3")

            # ---------- final: mel (sigmoid conv) + done, fused ----------
            for s in range(BS):
                for tt in range(4):
                    ps = psum.tile([128, F + 2], F32, tag=f"bank{s * 4 + tt}",
                                   name=f"fps_{s}_{tt}")
                    for dc in range(2):
                        _mm(nc, ps, xs[s][dc][:, tt * 128 : (tt + 1) * 128],
                            w_last_sb[dc], start=(dc == 0), stop=False)
                    _mm(nc, ps, ones_row, blast_sb, start=False, stop=True)
                    fo = temp.tile([128, F + 2], dt, tag=f"sm_{s}_{tt}",
                                   name=f"fin_{s}_{tt}")
                    nc.scalar.activation(fo, ps, AF.Sigmoid, scale=1.0)
                    nc.sync.dma_start(d_mel[s, tt * 128 : (tt + 1) * 128, :], fo[:, 0:F])
                    nc.sync.dma_start(d_done[s, tt * 128 : (tt + 1) * 128, :],
                                      fo[:, F : F + 1])

    nc.compile()
    return nc


def _prep_host(inputs):
    """Host-side packing: transposes and per-layer weight layout."""
    f32 = np.float32
    mm_np = np.float16 if MM_DT == "f16" else np.float32

    def npm(a):
        return np.ascontiguousarray(np.asarray(a, dtype=f32)).astype(mm_np)

    x0 = npm(np.asarray(inputs["inputs"]).transpose(0, 2, 1))  # (B, F, T)
    keysT = npm(np.asarray(inputs["keys"]).transpose(0, 2, 1))  # (B, D, TE)
    values = npm(inputs["values"])  # (B, TE, D)

    w_all = np.concatenate([np.asarray(inputs["enc_hw_w"]),
                            np.asarray(inputs["dec_hw_w"])], axis=0)  # (16, 512, 256, 3)
    wt = w_all.transpose(0, 2, 1, 3)            # (L, ci, co, k)
    wt = wt.reshape(L, 2, 128, 4, 128, 3)       # (L, kc, p, mt, f, k)
    hw_w = npm(wt.transpose(0, 2, 1, 5, 3, 4))  # (L, 128, kc, k, mt, f)

    b_all = np.concatenate([np.asarray(inputs["enc_hw_b"]),
                            np.asarray(inputs["dec_hw_b"])], axis=0)  # (16, 512)
    hw_b = np.ascontiguousarray(
        np.asarray(b_all, f32).reshape(L, 4, 128).transpose(2, 0, 1))  # (128, L, 4)

    def t2(w):  # (O, I, 1) -> (I, O)
        return npm(np.asarray(w)[:, :, 0].T)

    w_enc0 = t2(inputs["enc_w0"])
    w_enc1 = t2(inputs["enc_w1"])
    w_enc2 = t2(inputs["enc_w2"])
    w_q = npm(np.asarray(inputs["attn_q_w"]).T)
    w_o = npm(np.asarray(inputs["attn_o_w"], f32).T * (math.sqrt(TE) * SQ2))
    w_dec0 = t2(inputs["dec_w0"])
    w_dec1 = t2(inputs["dec_w1"])
    w_dec2 = t2(inputs["dec_w2"])
    w_dec3 = t2(inputs["dec_w3"])
    w_last = npm(np.concatenate(
        [np.asarray(inputs["last_w"], f32)[:, :, 0].T,
         np.asarray(inputs["fc_w"], f32).T,
         np.zeros((D, 1), f32)], axis=1))
    b_last = np.zeros((2, F + 2), f32)
    b_last[0, :F] = np.asarray(inputs["last_b"])
    b_last[0, F] = np.asarray(inputs["fc_b"])[0]
    b_last = npm(b_last)

    def cols(v):  # (256,) -> (128, 2)
        return np.asarray(v, dtype=f32).reshape(2, 128).T

    bias_tbl = np.zeros((128, NB), dtype=f32)
    bias_tbl[:, COL_ENC0:COL_ENC0 + 2] = cols(inputs["enc_b0"])
    bias_tbl[:, COL_ENC1:COL_ENC1 + 2] = cols(inputs["enc_b1"])
    bias_tbl[:, COL_ENC2:COL_ENC2 + 2] = cols(inputs["enc_b2"])
    bias_tbl[:, COL_QB:COL_QB + 2] = cols(inputs["attn_q_b"])
    bias_tbl[:, COL_OB:COL_OB + 2] = cols(np.asarray(inputs["attn_o_b"], f32) * SQ2)
    bias_tbl[:, COL_DEC0:COL_DEC0 + 2] = cols(inputs["dec_b0"])
    bias_tbl[:, COL_DEC1:COL_DEC1 + 2] = cols(inputs["dec_b1"])
    bias_tbl[:, COL_DEC2:COL_DEC2 + 2] = cols(inputs["dec_b2"])
    bias_tbl[:, COL_DEC3:COL_DEC3 + 2] = cols(inputs["dec_b3"])

    shared = dict(ones=np.stack([np.ones(128, f32), np.zeros(128, f32)]).astype(mm_np),
                  hw_w=hw_w, hw_b=hw_b, w_enc0=w_enc0, w_enc1=w_enc1, w_enc2=w_enc2,
                  w_q=w_q, w_o=w_o, w_dec0=w_dec0, w_dec1=w_dec1, w_dec2=w_dec2,
                  w_dec3=w_dec3, w_last=w_last, b_last=b_last, bias_tbl=bias_tbl)

    in_maps = []
    for i in range(N_CORES):
        sl = slice(i * BS, (i + 1) * BS)
        m = dict(shared)
        m["x0"] = np.ascontiguousarray(x0[sl])
        m["keysT"] = np.ascontiguousarray(keysT[sl])
        m["values"] = np.ascontiguousarray(values[sl])
        in_maps.append(m)
    return in_maps


def kernel(**inputs):
    global LAST_EXEC_NS
    if "nc" not in _BUILD_CACHE:
        _BUILD_CACHE["nc"] = _build()
    nc = _BUILD_CACHE["nc"]

    in_maps = _prep_host(inputs)

    trace = os.environ.get("KBENCH_TRACE", "0") == "1"
    if trace:
        _install_ntff_hook()
    res = run_bass_kernel_spmd(nc, in_maps, core_ids=list(range(N_CORES)), trace=trace)
    LAST_EXEC_NS = res.exec_time_ns

    mel = np.concatenate([r["mel"] for r in res.results], axis=0)
    attn = np.concatenate([r["attn"] for r in res.results], axis=0)
    done = np.concatenate([r["done"] for r in res.results], axis=0)
    return mel, attn, done


def _install_ntff_hook():
    """Register the axon NTFF profiling hook (missing from this image's antenv)."""
    import types

    if "antenv.axon_hooks" in sys.modules:
        return
    m = types.ModuleType("antenv.axon_hooks")
    m._h = None
    m.set_axon_ntff_profile_hook = lambda h: setattr(m, "_h", h)
    m.get_axon_ntff_profile_hook = lambda: m._h
    sys.modules["antenv.axon_hooks"] = m
    try:
        import antenv

        antenv.axon_hooks = m
        from trn_agent_boot.trn_boot import _ntff_profile_via_ctypes

        m._h = _ntff_profile_via_ctypes("/opt/axon/libaxon_pjrt.so")
    except Exception:
        m._h = None


# revision 18
# speedup vs baseline: 1.3104x; 1.0552x over previous
"""Trainium2 Bass kernel for nn_Decoder_42417097016016 (DCTTS-style decoder).

Sharding: pure data parallel over batch. B=16 samples -> 8 NeuronCores x 2
samples each; all weights replicated per core.

Layout: activations live on-chip as (channels, time) so every conv1x1 /
causal conv is a PE matmul with channels on partitions.  Causal convs with
dilation d are 3 accumulating matmuls per output tile with column shifts
(0, d, 2d) - left zero-padding falls out of the shifted PSUM accumulation.

The attention block computes scores (t,s), softmax along free dim (ACT Exp
with accum_out row sums), writes the normalized attn output directly, and
PE-transposes it to (s,t) for the context matmul.  mel and done share one
final matmul by concatenating fc_w as a 401st output column (padded to 402).

Matmul precision knob KBENCH_MM: "f16" (default, full PE rate), "f32r"
(reduced-precision fp32, half rate, needs even matmul geometry -> shifted x2
copies for odd-dilation taps), "f32" (exact, quarter rate).
"""

import math
import os
import sys

import numpy as np

for _p in ("/opt/trn_rl_repo", "/root/.axon_site/_ro/trn_rl_repo"):
    if os.path.isdir(_p) and _p not in sys.path:
        sys.path.append(_p)

import concourse.bass as bass
import concourse.tile as tile
from concourse import bacc, mybir
from concourse.bass_utils import run_bass_kernel_spmd

AF = mybir.ActivationFunctionType
ALU = mybir.AluOpType
AX = mybir.AxisListType
F32 = mybir.dt.float32
F32R = mybir.dt.float32r
F16 = mybir.dt.float16

N_CORES = 8
B, T, TE, D, F = 16, 512, 256, 256, 400
BS = B // N_CORES  # samples per core
DIL = [1, 3, 9, 27, 1, 3, 9, 27, 3, 3] + [1, 3, 9, 27, 1, 1]
L = len(DIL)  # 16 highway layers (10 encoder + 6 decoder)
SQ2 = math.sqrt(0.5)

MM_DT = os.environ.get("KBENCH_MM", "f16")

# bias table column assignments
COL_ENC0, COL_ENC1, COL_ENC2 = 0, 2, 4
COL_QB, COL_OB = 6, 8
COL_DEC0, COL_DEC1, COL_DEC2, COL_DEC3 = 10, 12, 14, 16
NB = 18

LAST_EXEC_NS = None
_BUILD_CACHE = {}


def _mm(nc, out, lhsT, rhs, **kw):
    nc.tensor.matmul(out, lhsT, rhs, **kw)


def _build():
    """Build the per-core Bass program (same program on all 8 cores)."""
    from concourse.masks import make_identity

    nc = bacc.Bacc("TRN2", target_bir_lowering=False, debug=False)
    dt = F32
    dtm = {"f16": F16, "f32r": F32R, "f32": F32}[MM_DT]
    use_x2 = MM_DT == "f32r"  # f32r matmuls need even offsets/N

    # ---- DRAM I/O (per-core shard shapes) ----
    d_x0 = nc.dram_tensor("x0", [BS, F, T], dtm, kind="ExternalInput").ap()
    d_keysT = nc.dram_tensor("keysT", [BS, D, TE], dtm, kind="ExternalInput").ap()
    d_values = nc.dram_tensor("values", [BS, TE, D], dtm, kind="ExternalInput").ap()
    d_hw_w = nc.dram_tensor("hw_w", [L, 128, 2, 3, 4, 128], dtm, kind="ExternalInput").ap()
    d_hw_b = nc.dram_tensor("hw_b", [128, L, 4], dt, kind="ExternalInput").ap()
    d_w_enc0 = nc.dram_tensor("w_enc0", [F, D], dtm, kind="ExternalInput").ap()
    d_w_enc1 = nc.dram_tensor("w_enc1", [D, D], dtm, kind="ExternalInput").ap()
    d_w_enc2 = nc.dram_tensor("w_enc2", [D, D], dtm, kind="ExternalInput").ap()
    d_w_q = nc.dram_tensor("w_q", [D, D], dtm, kind="ExternalInput").ap()
    d_w_o = nc.dram_tensor("w_o", [D, D], dtm, kind="ExternalInput").ap()
    d_w_dec0 = nc.dram_tensor("w_dec0", [2 * D, D], dtm, kind="ExternalInput").ap()
    d_w_dec1 = nc.dram_tensor("w_dec1", [D, D], dtm, kind="ExternalInput").ap()
    d_w_dec2 = nc.dram_tensor("w_dec2", [D, D], dtm, kind="ExternalInput").ap()
    d_w_dec3 = nc.dram_tensor("w_dec3", [D, D], dtm, kind="ExternalInput").ap()
    d_w_last = nc.dram_tensor("w_last", [D, F + 2], dtm, kind="ExternalInput").ap()
    d_b_last = nc.dram_tensor("b_last", [2, F + 2], dtm, kind="ExternalInput").ap()
    d_ones = nc.dram_tensor("ones", [2, 128], dtm, kind="ExternalInput").ap()
    d_bias = nc.dram_tensor("bias_tbl", [128, NB], dt, kind="ExternalInput").ap()

    d_mel = nc.dram_tensor("mel", [BS, T, F], dt, kind="ExternalOutput").ap()
    d_attn = nc.dram_tensor("attn", [BS, T, TE], dt, kind="ExternalOutput").ap()
    d_done = nc.dram_tensor("done", [BS, T, 1], dt, kind="ExternalOutput").ap()

    with tile.TileContext(nc) as tc:
        with (
            tc.tile_pool(name="const", bufs=1) as const,
            tc.tile_pool(name="xpool", bufs=1) as xpool,
            tc.tile_pool(name="persist", bufs=1) as persist,
            tc.tile_pool(name="wstream", bufs=1) as wstream,
            tc.tile_pool(name="temp", bufs=1) as temp,
            tc.tile_pool(name="psum", bufs=1, space="PSUM") as psum,
        ):
            # ---------- startup-critical loads first (DMA queue is FIFO) ----
            w_enc0_sb = []
            for c in range(4):
                cs = min(128, F - c * 128)
                t_ = const.tile([cs, D], dtm, name=f"wenc0_{c}")
                nc.sync.dma_start(t_, d_w_enc0[c * 128 : c * 128 + cs, :])
                w_enc0_sb.append(t_)
            xin = {s: [] for s in range(BS)}
            for s in range(BS):
                for c in range(4):
                    cs = min(128, F - c * 128)
                    t_in = temp.tile([cs, T], dtm, tag=f"sm_{s}_{c}", name=f"xin_{s}_{c}")
                    nc.sync.dma_start(t_in, d_x0[s, c * 128 : c * 128 + cs, :])
                    xin[s].append(t_in)

            def load_w(dram, nm, rows):
                tiles = []
                nchunk = (rows + 127) // 128
                cols = dram.shape[1]
                for c in range(nchunk):
                    cs = min(128, rows - c * 128)
                    t_ = const.tile([cs, cols], dtm, name=f"{nm}_{c}")
                    nc.sync.dma_start(t_, dram[c * 128 : c * 128 + cs, :])
                    tiles.append(t_)
                return tiles

            w_enc1_sb = load_w(d_w_enc1, "wenc1", D)
            w_enc2_sb = load_w(d_w_enc2, "wenc2", D)
            bias_sb = const.tile([128, NB], dt, name="bias_sb")
            nc.sync.dma_start(bias_sb, d_bias)
            hwb_sb = const.tile([128, L, 4], dt, name="hwb_sb")
            nc.sync.dma_start(hwb_sb, d_hw_b)

            ident = const.tile([128, 128], dt, name="ident")
            make_identity(nc, ident)
            stat_sb = const.tile([128, 32], F32, name="stat_sb")

            # helper: one conv1x1 block (256 outputs) for all samples.
            # Epilogue on DVE: out = relu?(psum + bias)
            def conv_block(w_tiles, rhs_per_s, relu, bias_col, banks, out_pool,
                           tag_fn, bufs=1, uid=""):
                outs = {s: [None, None] for s in range(BS)}
                pss = {}
                for mt in range(2):
                    for s in range(BS):
                        ps = psum.tile(
                            [128, T], F32,
                            tag=f"bank{banks[s * 2 + mt]}",
                            name=f"ps_{uid}_{s}_{mt}",
                        )
                        nk = len(w_tiles)
                        for c in range(nk):
                            _mm(nc, ps, w_tiles[c][:, mt * 128 : (mt + 1) * 128],
                                rhs_per_s[s][c], start=(c == 0), stop=(c == nk - 1))
                        pss[(s, mt)] = ps
                for mt in range(2):
                    for s in range(BS):
                        ot = out_pool.tile(
                            [128, T], dtm,
                            tag=tag_fn(s, mt), bufs=bufs,
                            name=f"{uid}_{s}_{mt}",
                        )
                        b_ap = bias_sb[:, bias_col + mt : bias_col + mt + 1]
                        if relu:
                            nc.vector.tensor_scalar(ot, pss[(s, mt)], b_ap, 0.0,
                                                    op0=ALU.add, op1=ALU.max)
                        else:
                            nc.vector.tensor_scalar_add(ot, pss[(s, mt)], b_ap)
                        outs[s][mt] = ot
                return outs

            xtag = lambda s, mt: f"x_{s}_{mt}"
            gatag = lambda s, mt: f"ga_{s}_{mt}"
            gbtag = lambda s, mt: f"gb_{s}_{mt}"

            # ---------- encoder head ----------
            h1 = conv_block(w_enc0_sb, xin, True, COL_ENC0, [0, 1, 4, 5], temp, gatag, uid="h1")
            h2 = conv_block(w_enc1_sb, h1, True, COL_ENC1, [2, 3, 6, 7], temp, gbtag, uid="h2")
            xs = conv_block(w_enc2_sb, h2, False, COL_ENC2, [0, 1, 4, 5], xpool, xtag,
                            bufs=3, uid="xe")

            # remaining const loads are deferred (emitted mid-encoder so the
            # first highway-layer weight DMAs aren't stuck behind them)
            misc = {}

            def load_misc_consts():
                misc["w_q"] = load_w(d_w_q, "wq", D)
                misc["w_o"] = load_w(d_w_o, "wo", D)
                misc["w_dec0"] = load_w(d_w_dec0, "wdec0", 2 * D)
                misc["w_dec1"] = load_w(d_w_dec1, "wdec1", D)
                misc["w_dec2"] = load_w(d_w_dec2, "wdec2", D)
                misc["w_dec3"] = load_w(d_w_dec3, "wdec3", D)
                misc["w_last"] = load_w(d_w_last, "wlast", D)
                ones_row = const.tile([2, 128], dtm, name="ones_row")
                nc.sync.dma_start(ones_row, d_ones)
                misc["ones"] = ones_row
                blast_sb = const.tile([2, F + 2], dtm, name="blast_sb")
                nc.sync.dma_start(blast_sb, d_b_last)
                misc["blast"] = blast_sb
                kT_sb, v_sb = {}, {}
                for s in range(BS):
                    for c in range(2):
                        kt = const.tile([128, TE], dtm, name=f"keysT_{s}_{c}")
                        nc.sync.dma_start(kt, d_keysT[s, c * 128 : (c + 1) * 128, :])
                        kT_sb[(s, c)] = kt
                        vt = const.tile([128, D], dtm, name=f"values_{s}_{c}")
                        nc.sync.dma_start(vt, d_values[s, c * 128 : (c + 1) * 128, :])
                        v_sb[(s, c)] = vt
                misc["kT"] = kT_sb
                misc["v"] = v_sb

            # ---------- highway stack ----------
            def make_x2(xs_cur, uid):
                # right-shifted copy (col0 = 0): makes the odd-dilation middle
                # conv tap even-aligned for the f32r matmul mode.
                out = {s: [None, None] for s in range(BS)}
                for c in range(2):
                    for s in range(BS):
                        x2 = xpool.tile([128, T], dtm, tag=f"x2_{s}_{c}", bufs=2,
                                        name=f"x2_{uid}_{s}_{c}")
                        nc.gpsimd.tensor_scalar_mul(x2[:, 0:1], xs_cur[s][c][:, 0:1], 0.0)
                        nc.gpsimd.tensor_copy(x2[:, 1:T], xs_cur[s][c][:, 0 : T - 1])
                        out[s][c] = x2
                return out

            def highway_layers(l_lo, l_hi, x2s):
                nonlocal xs
                for l in range(l_lo, l_hi):
                    dil = DIL[l]
                    wt = wstream.tile([128, 2, 3, 4, 128], dtm, tag="hww", bufs=2,
                                      name=f"hw_w_{l}")
                    nc.sync.dma_start(wt, d_hw_w[l])
                    last_enc = l == 9
                    ps_all = {}
                    for mt in range(4):
                        for s in range(BS):
                            ps_all[(mt, s)] = psum.tile(
                                [128, T], F32, tag=f"bank{mt * 2 + s}",
                                name=f"hwps_{l}_{mt}_{s}",
                            )
                    # kc-major: ALL kc0 taps (24 matmuls) before any kc1 tap,
                    # giving the previous layer's x'[1] epilogue a ~5us runway.
                    # mt order (2,0,3,1): chunk-0 gate+input banks finish first
                    # so its epilogue chain overlaps the remaining matmuls.
                    seen = {}
                    for kc in range(2):
                        for mt in (2, 0, 3, 1):
                            for k in (2, 1, 0):
                                first = kc == 0 and k == 2
                                last = kc == 1 and k == 0
                                for s in range(BS):
                                    ps = ps_all[(mt, s)]
                                    if k == 2:
                                        _mm(nc, ps, wt[:, kc, k, mt, :], xs[s][kc],
                                            start=first, stop=last)
                                    elif k == 1:
                                        if use_x2:
                                            off = dil - 1
                                            _mm(nc, ps[:, off:T], wt[:, kc, k, mt, :],
                                                x2s[s][kc][:, 0 : T - off],
                                                start=first, stop=last)
                                        else:
                                            _mm(nc, ps[:, dil:T], wt[:, kc, k, mt, :],
                                                xs[s][kc][:, 0 : T - dil],
                                                start=first, stop=last)
                                    else:
                                        sh = 2 * dil
                                        _mm(nc, ps[:, sh:T], wt[:, kc, k, mt, :],
                                            xs[s][kc][:, 0 : T - sh],
                                            start=first, stop=last)
                    # epilogue: x' = x + sigmoid(g) * (a - x)
                    # per chunk c: g = bank (c+2), a = bank c
                    newxs = {s: [None, None] for s in range(BS)}
                    for c in range(2):
                        tgs = {}
                        for s in range(BS):
                            tg = temp.tile([128, T], dtm, tag=f"tg_{s}_{c}", bufs=2,
                                           name=f"tg_{l}_{s}_{c}")
                            nc.scalar.activation(
                                tg, ps_all[(c + 2, s)], AF.Sigmoid,
                                bias=hwb_sb[:, l, c + 2 : c + 3], scale=1.0)
                            tgs[s] = tg
                        tmps = {}
                        for s in range(BS):
                            tmp = temp.tile([128, T], dtm, tag=f"tmp_{s}_{c}", bufs=2,
                                            name=f"tmp_{l}_{s}_{c}")
                            nc.vector.scalar_tensor_tensor(
                                tmp, ps_all[(c, s)], hwb_sb[:, l, c : c + 1],
                                xs[s][c], op0=ALU.add, op1=ALU.subtract)
                            tmps[s] = tmp
                        for s in range(BS):
                            nc.gpsimd.tensor_mul(tmps[s], tgs[s], tmps[s])
                        for s in range(BS):
                            if last_enc:
                                xn = persist.tile([128, T], dtm, tag=f"q_{s}_{c}",
                                                  name=f"Q_{s}_{c}")
                            else:
                                xn = xpool.tile([128, T], dtm, tag=f"x_{s}_{c}", bufs=3,
                                                name=f"x_{l + 1}_{s}_{c}")
                            nc.vector.tensor_add(xn, tmps[s], xs[s][c])
                            newxs[s][c] = xn
                    xs = newxs
                    if use_x2 and l + 1 < l_hi:
                        x2s = make_x2(xs, f"l{l + 1}")
                    if l == 1:
                        load_misc_consts()

            # ---------- encoder highway ----------
            highway_layers(0, 10, make_x2(xs, "e0") if use_x2 else None)
            w_q_sb, w_o_sb = misc["w_q"], misc["w_o"]
            w_dec0_sb, w_dec1_sb = misc["w_dec0"], misc["w_dec1"]
            w_dec2_sb, w_dec3_sb = misc["w_dec2"], misc["w_dec3"]
            w_last_sb, ones_row, blast_sb = misc["w_last"], misc["ones"], misc["blast"]
            kT_sb, v_sb = misc["kT"], misc["v"]
            Qs = xs  # persisted encoder output (D, T) tiles

            # ---------- attention ----------
            Rqs = {s: [] for s in range(BS)}
            q_sb = {s: [] for s in range(BS)}
            for mt in range(2):
                for s in range(BS):
                    ps = psum.tile([128, T], F32, tag=f"bank{s * 4 + mt}",
                                   name=f"qps_{s}_{mt}")
                    for kc in range(2):
                        _mm(nc, ps, w_q_sb[kc][:, mt * 128 : (mt + 1) * 128], Qs[s][kc],
                            start=(kc == 0), stop=(kc == 1))
                    qt = temp.tile([128, T], dtm, tag=f"ga_{s}_{mt}", name=f"q_{s}_{mt}")
                    nc.vector.tensor_scalar_add(
                        qt, ps, bias_sb[:, COL_QB + mt : COL_QB + mt + 1])
                    q_sb[s].append(qt)

            att_tiles = {}
            for s in range(BS):
                for tt in range(4):
                    ps = psum.tile([128, TE], F32, tag=f"bank{s * 4 + tt}",
                                   name=f"sps_{s}_{tt}")
                    for dc in range(2):
                        _mm(nc, ps, q_sb[s][dc][:, tt * 128 : (tt + 1) * 128],
                            kT_sb[(s, dc)], start=(dc == 0), stop=(dc == 1))
                    st = stat_sb[:, (s * 4 + tt) * 4 : (s * 4 + tt) * 4 + 4]
                    nc.vector.reduce_max(st[:, 0:1], ps, axis=AX.X, negate=True)
                    at = temp.tile([128, TE], dt, tag=f"sm_{s}_{tt}",
                                   name=f"att_{s}_{tt}")
                    nc.scalar.activation(at, ps, AF.Exp, bias=st[:, 0:1], scale=1.0,
                                         accum_out=st[:, 1:2])
                    nc.vector.reciprocal(st[:, 2:3], st[:, 1:2])
                    nc.vector.tensor_scalar_mul(at, at, st[:, 2:3])
                    nc.sync.dma_start(d_attn[s, tt * 128 : (tt + 1) * 128, :], at)
                    att_tiles[(s, tt)] = at

            aT = {s: [] for s in range(BS)}
            for s in range(BS):
                for sc in range(2):
                    pst = psum.tile([128, T], F32, tag=f"bank{s * 4 + sc}",
                                    name=f"tps_{s}_{sc}")
                    for tt in range(4):
                        nc.tensor.matmul(
                            pst[:, tt * 128 : (tt + 1) * 128],
                            att_tiles[(s, tt)][:, sc * 128 : (sc + 1) * 128],
                            ident, is_transpose=True, start=True, stop=True,
                            skip_group_check=True)
                    a2 = temp.tile([128, T], dtm, tag=f"gb_{s}_{sc}", name=f"aT_{s}_{sc}")
                    nc.vector.tensor_copy(a2, pst)
                    aT[s].append(a2)

            ctx_sb = {s: [] for s in range(BS)}
            for s in range(BS):
                for dc in range(2):
                    ps = psum.tile([128, T], F32, tag=f"bank{s * 4 + 2 + dc}",
                                   name=f"cps_{s}_{dc}")
                    for sc in range(2):
                        _mm(nc, ps, v_sb[(s, sc)][:, dc * 128 : (dc + 1) * 128],
                            aT[s][sc], start=(sc == 0), stop=(sc == 1))
                    ct = temp.tile([128, T], dtm, tag=f"ga_{s}_{dc}", name=f"ctx_{s}_{dc}")
                    nc.vector.tensor_copy(ct, ps)
                    ctx_sb[s].append(ct)

            for mt in range(2):
                for s in range(BS):
                    ps = psum.tile([128, T], F32, tag=f"bank{s * 4 + mt}",
                                   name=f"ops_{s}_{mt}")
                    for dc in range(2):
                        _mm(nc, ps, w_o_sb[dc][:, mt * 128 : (mt + 1) * 128],
                            ctx_sb[s][dc], start=(dc == 0), stop=(dc == 1))
                    tmpo = temp.tile([128, T], dt, tag=f"gb_{s}_{mt}",
                                     name=f"tmpo_{s}_{mt}")
                    nc.vector.tensor_scalar_add(
                        tmpo, ps, bias_sb[:, COL_OB + mt : COL_OB + mt + 1])
                    rq = persist.tile([128, T], dtm, tag=f"rq_{s}_{mt}",
                                      name=f"rq_{s}_{mt}")
                    # Rq = sqrt(.5)*query + out_proj  [scales folded into w_o/b_o]
                    nc.vector.scalar_tensor_tensor(
                        rq, Qs[s][mt], SQ2, tmpo, op0=ALU.mult, op1=ALU.add)
                    Rqs[s].append(rq)

            # ---------- decoder ----------
            dec_in = {s: [Rqs[s][0], Rqs[s][1], Qs[s][0], Qs[s][1]] for s in range(BS)}
            xs = conv_block(w_dec0_sb, dec_in, False, COL_DEC0, [2, 3, 6, 7],
                            xpool, xtag, bufs=3, uid="xd0")
            highway_layers(10, 16, make_x2(xs, "d0") if use_x2 else None)
            xs = conv_block(w_dec1_sb, xs, True, COL_DEC1, [0, 1, 4, 5], xpool, xtag,
                            bufs=3, uid="xd1")
            xs = conv_block(w_dec2_sb, xs, True, COL_DEC2, [2, 3, 6, 7], xpool, xtag,
                            bufs=3, uid="xd2")
            xs = conv_block(w_dec3_sb, xs, True, COL_DEC3, [0, 1, 4, 5], xpool, xtag,
                            bufs=3, uid="xd3")

            # ---------- final: mel (sigmoid conv) + done, fused ----------
            for s in range(BS):
                for tt in range(4):
                    ps = psum.tile([128, F + 2], F32, tag=f"bank{s * 4 + tt}",
                                   name=f"fps_{s}_{tt}")
                    for dc in range(2):
                        _mm(nc, ps, xs[s][dc][:, tt * 128 : (tt + 1) * 128],
                            w_last_sb[dc], start=(dc == 0), stop=False)
                    _mm(nc, ps, ones_row, blast_sb, start=False, stop=True)
                    fo = temp.tile([128, F + 2], dt, tag=f"sm_{s}_{tt}",
                                   name=f"fin_{s}_{tt}")
                    nc.scalar.activation(fo, ps, AF.Sigmoid, scale=1.0)
                    nc.sync.dma_start(d_mel[s, tt * 128 : (tt + 1) * 128, :], fo[:, 0:F])
                    nc.sync.dma_start(d_done[s, tt * 128 : (tt + 1) * 128, :],
                                      fo[:, F : F + 1])

    nc.compile()
    return nc


def _prep_host(inputs):
    """Host-side packing: transposes and per-layer weight layout."""
    f32 = np.float32
    mm_np = np.float16 if MM_DT == "f16" else np.float32

    def npm(a):
        return np.ascontiguousarray(np.asarray(a, dtype=f32)).astype(mm_np)

    x0 = npm(np.asarray(inputs["inputs"]).transpose(0, 2, 1))  # (B, F, T)
    keysT = npm(np.asarray(inputs["keys"]).transpose(0, 2, 1))  # (B, D, TE)
    values = npm(inputs["values"])  # (B, TE, D)

    w_all = np.concatenate([np.asarray(inputs["enc_hw_w"]),
                            np.asarray(inputs["dec_hw_w"])], axis=0)  # (16, 512, 256, 3)
    wt = w_all.transpose(0, 2, 1, 3)            # (L, ci, co, k)
    wt = wt.reshape(L, 2, 128, 4, 128, 3)       # (L, kc, p, mt, f, k)
    hw_w = npm(wt.transpose(0, 2, 1, 5, 3, 4))  # (L, 128, kc, k, mt, f)

    b_all = np.concatenate([np.asarray(inputs["enc_hw_b"]),
                            np.asarray(inputs["dec_hw_b"])], axis=0)  # (16, 512)
    hw_b = np.ascontiguousarray(
        np.asarray(b_all, f32).reshape(L, 4, 128).transpose(2, 0, 1))  # (128, L, 4)

    def t2(w):  # (O, I, 1) -> (I, O)
        return npm(np.asarray(w)[:, :, 0].T)

    w_enc0 = t2(inputs["enc_w0"])
    w_enc1 = t2(inputs["enc_w1"])
    w_enc2 = t2(inputs["enc_w2"])
    w_q = npm(np.asarray(inputs["attn_q_w"]).T)
    w_o = npm(np.asarray(inputs["attn_o_w"], f32).T * (math.sqrt(TE) * SQ2))
    w_dec0 = t2(inputs["dec_w0"])
    w_dec1 = t2(inputs["dec_w1"])
    w_dec2 = t2(inputs["dec_w2"])
    w_dec3 = t2(inputs["dec_w3"])
    w_last = npm(np.concatenate(
        [np.asarray(inputs["last_w"], f32)[:, :, 0].T,
         np.asarray(inputs["fc_w"], f32).T,
         np.zeros((D, 1), f32)], axis=1))
    b_last = np.zeros((2, F + 2), f32)
    b_last[0, :F] = np.asarray(inputs["last_b"])
    b_last[0, F] = np.asarray(inputs["fc_b"])[0]
    b_last = npm(b_last)

    def cols(v):  # (256,) -> (128, 2)
        return np.asarray(v, dtype=f32).reshape(2, 128).T

    bias_tbl = np.zeros((128, NB), dtype=f32)
    bias_tbl[:, COL_ENC0:COL_ENC0 + 2] = cols(inputs["enc_b0"])
    bias_tbl[:, COL_ENC1:COL_ENC1 + 2] = cols(inputs["enc_b1"])
    bias_tbl[:, COL_ENC2:COL_ENC2 + 2] = cols(inputs["enc_b2"])
    bias_tbl[:, COL_QB:COL_QB + 2] = cols(inputs["attn_q_b"])
    bias_tbl[:, COL_OB:COL_OB + 2] = cols(np.asarray(inputs["attn_o_b"], f32) * SQ2)
    bias_tbl[:, COL_DEC0:COL_DEC0 + 2] = cols(inputs["dec_b0"])
    bias_tbl[:, COL_DEC1:COL_DEC1 + 2] = cols(inputs["dec_b1"])
    bias_tbl[:, COL_DEC2:COL_DEC2 + 2] = cols(inputs["dec_b2"])
    bias_tbl[:, COL_DEC3:COL_DEC3 + 2] = cols(inputs["dec_b3"])

    shared = dict(ones=np.stack([np.ones(128, f32), np.zeros(128, f32)]).astype(mm_np),
                  hw_w=hw_w, hw_b=hw_b, w_enc0=w_enc0, w_enc1=w_enc1, w_enc2=w_enc2,
                  w_q=w_q, w_o=w_o, w_dec0=w_dec0, w_dec1=w_dec1, w_dec2=w_dec2,
                  w_dec3=w_dec3, w_last=w_last, b_last=b_last, bias_tbl=bias_tbl)

    in_maps = []
    for i in range(N_CORES):
        sl = slice(i * BS, (i + 1) * BS)
        m = dict(shared)
        m["x0"] = np.ascontiguousarray(x0[sl])
        m["keysT"] = np.ascontiguousarray(keysT[sl])
        m["values"] = np.ascontiguousarray(values[sl])
        in_maps.append(m)
    return in_maps


def kernel(**inputs):
    global LAST_EXEC_NS
    if "nc" not in _BUILD_CACHE:
        _BUILD_CACHE["nc"] = _build()
    nc = _BUILD_CACHE["nc"]

    in_maps = _prep_host(inputs)

    trace = os.environ.get("KBENCH_TRACE", "0") == "1"
    if trace:
        _install_ntff_hook()
    res = run_bass_kernel_spmd(nc, in_maps, core_ids=list(range(N_CORES)), trace=trace)
    LAST_EXEC_NS = res.exec_time_ns

    mel = np.concatenate([r["mel"] for r in res.results], axis=0)
    attn = np.concatenate([r["attn"] for r in res.results], axis=0)
    done = np.concatenate([r["done"] for r in res.results], axis=0)
    return mel, attn, done


def _install_ntff_hook():
    """Register the axon NTFF profiling hook (missing from this image's antenv)."""
    import types

    if "antenv.axon_hooks" in sys.modules:
        return
    m = types.ModuleType("antenv.axon_hooks")
    m._h = None
    m.set_axon_ntff_profile_hook = lambda h: setattr(m, "_h", h)
    m.get_axon_ntff_profile_hook = lambda: m._h
    sys.modules["antenv.axon_hooks"] = m
    try:
        import antenv

        antenv.axon_hooks = m
        from trn_agent_boot.trn_boot import _ntff_profile_via_ctypes

        m._h = _ntff_profile_via_ctypes("/opt/axon/libaxon_pjrt.so")
    except Exception:
        m._h = None


# revision 21
# speedup vs baseline: 1.3418x; 1.0240x over previous
"""Trainium2 Bass kernel for nn_Decoder_42417097016016 (DCTTS-style decoder).

Sharding: pure data parallel over batch. B=16 samples -> 8 NeuronCores x 2
samples each; all weights replicated per core.

Layout: activations live on-chip as (channels, time) so every conv1x1 /
causal conv is a PE matmul with channels on partitions.  Causal convs with
dilation d are 3 accumulating matmuls per output tile with column shifts
(0, d, 2d) - left zero-padding falls out of the shifted PSUM accumulation.

The attention block computes scores (t,s), softmax along free dim (ACT Exp
with accum_out row sums), writes the normalized attn output directly, and
PE-transposes it to (s,t) for the context matmul.  mel and done share one
final matmul by concatenating fc_w as a 401st output column (padded to 402).

Matmul precision knob KBENCH_MM: "f16" (default, full PE rate), "f32r"
(reduced-precision fp32, half rate, needs even matmul geometry -> shifted x2
copies for odd-dilation taps), "f32" (exact, quarter rate).
"""

import math
import os
import sys

import numpy as np

for _p in ("/opt/trn_rl_repo", "/root/.axon_site/_ro/trn_rl_repo"):
    if os.path.isdir(_p) and _p not in sys.path:
        sys.path.append(_p)

import concourse.bass as bass
import concourse.tile as tile
from concourse import bacc, mybir
from concourse.bass_utils import run_bass_kernel_spmd

AF = mybir.ActivationFunctionType
ALU = mybir.AluOpType
AX = mybir.AxisListType
F32 = mybir.dt.float32
F32R = mybir.dt.float32r
F16 = mybir.dt.float16

N_CORES = 8
B, T, TE, D, F = 16, 512, 256, 256, 400
BS = B // N_CORES  # samples per core
DIL = [1, 3, 9, 27, 1, 3, 9, 27, 3, 3] + [1, 3, 9, 27, 1, 1]
L = len(DIL)  # 16 highway layers (10 encoder + 6 decoder)
SQ2 = math.sqrt(0.5)

MM_DT = os.environ.get("KBENCH_MM", "f16")

# bias table column assignments
COL_ENC0, COL_ENC1, COL_ENC2 = 0, 2, 4
COL_QB, COL_OB = 6, 8
COL_DEC0, COL_DEC1, COL_DEC2, COL_DEC3 = 10, 12, 14, 16
NB = 18

LAST_EXEC_NS = None
_BUILD_CACHE = {}


def _mm(nc, out, lhsT, rhs, **kw):
    nc.tensor.matmul(out, lhsT, rhs, **kw)


def _build():
    """Build the per-core Bass program (same program on all 8 cores)."""
    from concourse.masks import make_identity

    nc = bacc.Bacc("TRN2", target_bir_lowering=False, debug=False)
    dt = F32
    dtm = {"f16": F16, "f32r": F32R, "f32": F32}[MM_DT]
    use_x2 = MM_DT == "f32r"  # f32r matmuls need even offsets/N

    # ---- DRAM I/O (per-core shard shapes) ----
    # x0: (s, p, c, t) pre-chunked+padded on host -> one DMA per sample
    d_x0 = nc.dram_tensor("x0", [BS, 128, 4, T], dtm, kind="ExternalInput").ap()
    d_hw_w = nc.dram_tensor("hw_w", [L, 128, 2, 3, 4, 128], dtm, kind="ExternalInput").ap()
    # wenc0p: enc_w0^T padded 400->512 rows, chunked (128, 4, 256)
    d_wenc0 = nc.dram_tensor("wenc0p", [128, 4, D], dtm, kind="ExternalInput").ap()
    # pack1: [enc1 c0|c1, enc2 c0|c1] as (128, 4*256)
    d_pack1 = nc.dram_tensor("pack1", [128, 4 * D], dtm, kind="ExternalInput").ap()
    # bias2: [bias_tbl (18) | hw_b (L*4)] fp32
    d_bias2 = nc.dram_tensor("bias2", [128, NB + L * 4], dt, kind="ExternalInput").ap()
    # wpack: all attention/decoder weights + keys/values + ones/blast rows
    WQ_O, WO_O = 0, 512
    WD0_O, WD1_O, WD2_O, WD3_O = 1024, 2048, 2560, 3072
    WLAST_O = 3584
    KT_O = WLAST_O + 2 * (F + 2)          # 4388
    V_O = KT_O + 4 * TE                   # 5412
    ONES_O = V_O + 4 * D                  # 6436
    BLAST_O = ONES_O + 128                # 6564
    WPACK_COLS = BLAST_O + (F + 2)        # 6966
    d_wpack = nc.dram_tensor("wpack", [128, WPACK_COLS], dtm, kind="ExternalInput").ap()

    d_mel = nc.dram_tensor("mel", [BS, T, F], dt, kind="ExternalOutput").ap()
    d_attn = nc.dram_tensor("attn", [BS, T, TE], dt, kind="ExternalOutput").ap()
    d_done = nc.dram_tensor("done", [BS, T, 1], dt, kind="ExternalOutput").ap()

    with tile.TileContext(nc) as tc:
        with (
            tc.tile_pool(name="const", bufs=1) as const,
            tc.tile_pool(name="xpool", bufs=1) as xpool,
            tc.tile_pool(name="persist", bufs=1) as persist,
            tc.tile_pool(name="wstream", bufs=1) as wstream,
            tc.tile_pool(name="temp", bufs=1) as temp,
            tc.tile_pool(name="psum", bufs=1, space="PSUM") as psum,
        ):
            # ---------- startup-critical loads first (DMA queue is FIFO) ----
            wenc0_sb = const.tile([128, 4, D], dtm, name="wenc0_sb")
            nc.sync.dma_start(wenc0_sb, d_wenc0)
            w_enc0_sb = [wenc0_sb[:, c, :] for c in range(4)]
            xin = {}
            for s in range(BS):
                xt = temp.tile([128, 4, T], dtm, tag=f"xin_{s}", name=f"xin_{s}")
                nc.sync.dma_start(xt, d_x0[s])
                xin[s] = [xt[:, c, :] for c in range(4)]
            pack1_sb = const.tile([128, 4 * D], dtm, name="pack1_sb")
            nc.sync.dma_start(pack1_sb, d_pack1)
            w_enc1_sb = [pack1_sb[:, 0:D], pack1_sb[:, D : 2 * D]]
            w_enc2_sb = [pack1_sb[:, 2 * D : 3 * D], pack1_sb[:, 3 * D : 4 * D]]
            bias2_sb = const.tile([128, NB + L * 4], dt, name="bias2_sb")
            nc.sync.dma_start(bias2_sb, d_bias2)
            bias_sb = bias2_sb[:, 0:NB]

            def hwb_ap(l, j):
                c = NB + l * 4 + j
                return bias2_sb[:, c : c + 1]

            ident = const.tile([128, 128], dt, name="ident")
            make_identity(nc, ident)
            stat_sb = const.tile([128, 32], F32, name="stat_sb")

            # helper: one conv1x1 block (256 outputs) for all samples.
            # Epilogue on DVE: out = relu?(psum + bias)
            def conv_block(w_tiles, rhs_per_s, relu, bias_col, banks, out_pool,
                           tag_fn, bufs=1, uid=""):
                outs = {s: [None, None] for s in range(BS)}
                pss = {}
                for mt in range(2):
                    for s in range(BS):
                        ps = psum.tile(
                            [128, T], F32,
                            tag=f"bank{banks[s * 2 + mt]}",
                            name=f"ps_{uid}_{s}_{mt}",
                        )
                        nk = len(w_tiles)
                        for c in range(nk):
                            _mm(nc, ps, w_tiles[c][:, mt * 128 : (mt + 1) * 128],
                                rhs_per_s[s][c], start=(c == 0), stop=(c == nk - 1))
                        pss[(s, mt)] = ps
                for mt in range(2):
                    for s in range(BS):
                        ot = out_pool.tile(
                            [128, T], dtm,
                            tag=tag_fn(s, mt), bufs=bufs,
                            name=f"{uid}_{s}_{mt}",
                        )
                        b_ap = bias_sb[:, bias_col + mt : bias_col + mt + 1]
                        if relu:
                            nc.vector.tensor_scalar(ot, pss[(s, mt)], b_ap, 0.0,
                                                    op0=ALU.add, op1=ALU.max)
                        else:
                            nc.vector.tensor_scalar_add(ot, pss[(s, mt)], b_ap)
                        outs[s][mt] = ot
                return outs

            xtag = lambda s, mt: f"x_{s}_{mt}"
            gatag = lambda s, mt: f"ga_{s}_{mt}"
            gbtag = lambda s, mt: f"gb_{s}_{mt}"

            # ---------- encoder head ----------
            h1 = conv_block(w_enc0_sb, xin, True, COL_ENC0, [0, 1, 4, 5], temp, gatag, uid="h1")
            h2 = conv_block(w_enc1_sb, h1, True, COL_ENC1, [2, 3, 6, 7], temp, gbtag, uid="h2")
            xs = conv_block(w_enc2_sb, h2, False, COL_ENC2, [0, 1, 4, 5], xpool, xtag,
                            bufs=3, uid="xe")

            # remaining const loads are deferred (emitted mid-encoder so the
            # first highway-layer weight DMAs aren't stuck behind them)
            misc = {}

            def load_misc_consts():
                wp = const.tile([128, WPACK_COLS], dtm, name="wpack_sb")
                nc.sync.dma_start(wp, d_wpack)
                misc["w_q"] = [wp[:, WQ_O : WQ_O + D], wp[:, WQ_O + D : WQ_O + 2 * D]]
                misc["w_o"] = [wp[:, WO_O : WO_O + D], wp[:, WO_O + D : WO_O + 2 * D]]
                misc["w_dec0"] = [wp[:, WD0_O + c * D : WD0_O + (c + 1) * D] for c in range(4)]
                misc["w_dec1"] = [wp[:, WD1_O : WD1_O + D], wp[:, WD1_O + D : WD1_O + 2 * D]]
                misc["w_dec2"] = [wp[:, WD2_O : WD2_O + D], wp[:, WD2_O + D : WD2_O + 2 * D]]
                misc["w_dec3"] = [wp[:, WD3_O : WD3_O + D], wp[:, WD3_O + D : WD3_O + 2 * D]]
                misc["w_last"] = [wp[:, WLAST_O : WLAST_O + F + 2],
                                  wp[:, WLAST_O + F + 2 : WLAST_O + 2 * (F + 2)]]
                misc["ones"] = wp[0:2, ONES_O : ONES_O + 128]
                misc["blast"] = wp[0:2, BLAST_O : BLAST_O + F + 2]
                kT_sb, v_sb = {}, {}
                for s in range(BS):
                    for c in range(2):
                        i = s * 2 + c
                        kT_sb[(s, c)] = wp[:, KT_O + i * TE : KT_O + (i + 1) * TE]
                        v_sb[(s, c)] = wp[:, V_O + i * D : V_O + (i + 1) * D]
                misc["kT"] = kT_sb
                misc["v"] = v_sb

            # ---------- highway stack ----------
            def make_x2(xs_cur, uid):
                # right-shifted copy (col0 = 0): makes the odd-dilation middle
                # conv tap even-aligned for the f32r matmul mode.
                out = {s: [None, None] for s in range(BS)}
                for c in range(2):
                    for s in range(BS):
                        x2 = xpool.tile([128, T], dtm, tag=f"x2_{s}_{c}", bufs=2,
                                        name=f"x2_{uid}_{s}_{c}")
                        nc.gpsimd.tensor_scalar_mul(x2[:, 0:1], xs_cur[s][c][:, 0:1], 0.0)
                        nc.gpsimd.tensor_copy(x2[:, 1:T], xs_cur[s][c][:, 0 : T - 1])
                        out[s][c] = x2
                return out

            def highway_layers(l_lo, l_hi, x2s):
                nonlocal xs
                for l in range(l_lo, l_hi):
                    dil = DIL[l]
                    wt = wstream.tile([128, 2, 3, 4, 128], dtm, tag="hww", bufs=2,
                                      name=f"hw_w_{l}")
                    nc.sync.dma_start(wt, d_hw_w[l])
                    last_enc = l == 9
                    ps_all = {}
                    for mt in range(4):
                        for s in range(BS):
                            ps_all[(mt, s)] = psum.tile(
                                [128, T], F32, tag=f"bank{mt * 2 + s}",
                                name=f"hwps_{l}_{mt}_{s}",
                            )
                    # kc-major: ALL kc0 taps (24 matmuls) before any kc1 tap,
                    # giving the previous layer's x'[1] epilogue a ~5us runway.
                    # mt order (2,0,3,1): chunk-0 gate+input banks finish first
                    # so its epilogue chain overlaps the remaining matmuls.
                    seen = {}
                    for kc in range(2):
                        for mt in (2, 0, 3, 1):
                            for k in (2, 1, 0):
                                first = kc == 0 and k == 2
                                last = kc == 1 and k == 0
                                for s in range(BS):
                                    ps = ps_all[(mt, s)]
                                    if k == 2:
                                        _mm(nc, ps, wt[:, kc, k, mt, :], xs[s][kc],
                                            start=first, stop=last)
                                    elif k == 1:
                                        if use_x2:
                                            off = dil - 1
                                            _mm(nc, ps[:, off:T], wt[:, kc, k, mt, :],
                                                x2s[s][kc][:, 0 : T - off],
                                                start=first, stop=last)
                                        else:
                                            _mm(nc, ps[:, dil:T], wt[:, kc, k, mt, :],
                                                xs[s][kc][:, 0 : T - dil],
                                                start=first, stop=last)
                                    else:
                                        sh = 2 * dil
                                        _mm(nc, ps[:, sh:T], wt[:, kc, k, mt, :],
                                            xs[s][kc][:, 0 : T - sh],
                                            start=first, stop=last)
                    # epilogue: x' = x + sigmoid(g) * (a - x)
                    # per chunk c: g = bank (c+2), a = bank c
                    newxs = {s: [None, None] for s in range(BS)}
                    for c in range(2):
                        tgs = {}
                        for s in range(BS):
                            tg = temp.tile([128, T], dtm, tag=f"tg_{s}_{c}", bufs=2,
                                           name=f"tg_{l}_{s}_{c}")
                            nc.scalar.activation(
                                tg, ps_all[(c + 2, s)], AF.Sigmoid,
                                bias=hwb_ap(l, c + 2), scale=1.0)
                            tgs[s] = tg
                        tmps = {}
                        for s in range(BS):
                            tmp = temp.tile([128, T], dtm, tag=f"tmp_{s}_{c}", bufs=2,
                                            name=f"tmp_{l}_{s}_{c}")
                            nc.vector.scalar_tensor_tensor(
                                tmp, ps_all[(c, s)], hwb_ap(l, c),
                                xs[s][c], op0=ALU.add, op1=ALU.subtract)
                            tmps[s] = tmp
                        for s in range(BS):
                            nc.gpsimd.tensor_mul(tmps[s], tgs[s], tmps[s])
                        for s in range(BS):
                            if last_enc:
                                xn = persist.tile([128, T], dtm, tag=f"q_{s}_{c}",
                                                  name=f"Q_{s}_{c}")
                            else:
                                xn = xpool.tile([128, T], dtm, tag=f"x_{s}_{c}", bufs=3,
                                                name=f"x_{l + 1}_{s}_{c}")
                            nc.vector.tensor_add(xn, tmps[s], xs[s][c])
                            newxs[s][c] = xn
                    xs = newxs
                    if use_x2 and l + 1 < l_hi:
                        x2s = make_x2(xs, f"l{l + 1}")
                    if l == 1:
                        load_misc_consts()

            # ---------- encoder highway ----------
            highway_layers(0, 10, make_x2(xs, "e0") if use_x2 else None)
            w_q_sb, w_o_sb = misc["w_q"], misc["w_o"]
            w_dec0_sb, w_dec1_sb = misc["w_dec0"], misc["w_dec1"]
            w_dec2_sb, w_dec3_sb = misc["w_dec2"], misc["w_dec3"]
            w_last_sb, ones_row, blast_sb = misc["w_last"], misc["ones"], misc["blast"]
            kT_sb, v_sb = misc["kT"], misc["v"]
            Qs = xs  # persisted encoder output (D, T) tiles

            # ---------- attention ----------
            Rqs = {s: [] for s in range(BS)}
            q_sb = {s: [] for s in range(BS)}
            for mt in range(2):
                for s in range(BS):
                    ps = psum.tile([128, T], F32, tag=f"bank{s * 4 + mt}",
                                   name=f"qps_{s}_{mt}")
                    for kc in range(2):
                        _mm(nc, ps, w_q_sb[kc][:, mt * 128 : (mt + 1) * 128], Qs[s][kc],
                            start=(kc == 0), stop=(kc == 1))
                    qt = temp.tile([128, T], dtm, tag=f"ga_{s}_{mt}", name=f"q_{s}_{mt}")
                    nc.vector.tensor_scalar_add(
                        qt, ps, bias_sb[:, COL_QB + mt : COL_QB + mt + 1])
                    q_sb[s].append(qt)

            att_tiles = {}
            for s in range(BS):
                at_s = temp.tile([128, 4, TE], dt, tag=f"att_{s}", name=f"att_{s}")
                for tt in range(4):
                    ps = psum.tile([128, TE], F32, tag=f"bank{s * 4 + tt}",
                                   name=f"sps_{s}_{tt}")
                    for dc in range(2):
                        _mm(nc, ps, q_sb[s][dc][:, tt * 128 : (tt + 1) * 128],
                            kT_sb[(s, dc)], start=(dc == 0), stop=(dc == 1))
                    st = stat_sb[:, (s * 4 + tt) * 4 : (s * 4 + tt) * 4 + 4]
                    nc.vector.reduce_max(st[:, 0:1], ps, axis=AX.X, negate=True)
                    at = at_s[:, tt, :]
                    nc.scalar.activation(at, ps, AF.Exp, bias=st[:, 0:1], scale=1.0,
                                         accum_out=st[:, 1:2])
                    nc.vector.reciprocal(st[:, 2:3], st[:, 1:2])
                    nc.vector.tensor_scalar_mul(at, at, st[:, 2:3])
                    att_tiles[(s, tt)] = at
                nc.sync.dma_start(
                    d_attn[s].rearrange("(tt p) e -> p tt e", p=128), at_s)

            aT = {s: [] for s in range(BS)}
            for s in range(BS):
                for sc in range(2):
                    pst = psum.tile([128, T], F32, tag=f"bank{s * 4 + sc}",
                                    name=f"tps_{s}_{sc}")
                    for tt in range(4):
                        nc.tensor.matmul(
                            pst[:, tt * 128 : (tt + 1) * 128],
                            att_tiles[(s, tt)][:, sc * 128 : (sc + 1) * 128],
                            ident, is_transpose=True, start=True, stop=True,
                            skip_group_check=True)
                    a2 = temp.tile([128, T], dtm, tag=f"gb_{s}_{sc}", name=f"aT_{s}_{sc}")
                    nc.vector.tensor_copy(a2, pst)
                    aT[s].append(a2)

            ctx_sb = {s: [] for s in range(BS)}
            for s in range(BS):
                for dc in range(2):
                    ps = psum.tile([128, T], F32, tag=f"bank{s * 4 + 2 + dc}",
                                   name=f"cps_{s}_{dc}")
                    for sc in range(2):
                        _mm(nc, ps, v_sb[(s, sc)][:, dc * 128 : (dc + 1) * 128],
                            aT[s][sc], start=(sc == 0), stop=(sc == 1))
                    ct = temp.tile([128, T], dtm, tag=f"ga_{s}_{dc}", name=f"ctx_{s}_{dc}")
                    nc.vector.tensor_copy(ct, ps)
                    ctx_sb[s].append(ct)

            for mt in range(2):
                for s in range(BS):
                    ps = psum.tile([128, T], F32, tag=f"bank{s * 4 + mt}",
                                   name=f"ops_{s}_{mt}")
                    for dc in range(2):
                        _mm(nc, ps, w_o_sb[dc][:, mt * 128 : (mt + 1) * 128],
                            ctx_sb[s][dc], start=(dc == 0), stop=(dc == 1))
                    tmpo = temp.tile([128, T], dt, tag=f"gb_{s}_{mt}",
                                     name=f"tmpo_{s}_{mt}")
                    nc.vector.tensor_scalar_add(
                        tmpo, ps, bias_sb[:, COL_OB + mt : COL_OB + mt + 1])
                    rq = persist.tile([128, T], dtm, tag=f"rq_{s}_{mt}",
                                      name=f"rq_{s}_{mt}")
                    # Rq = sqrt(.5)*query + out_proj  [scales folded into w_o/b_o]
                    nc.vector.scalar_tensor_tensor(
                        rq, Qs[s][mt], SQ2, tmpo, op0=ALU.mult, op1=ALU.add)
                    Rqs[s].append(rq)

            # ---------- decoder ----------
            dec_in = {s: [Rqs[s][0], Rqs[s][1], Qs[s][0], Qs[s][1]] for s in range(BS)}
            xs = conv_block(w_dec0_sb, dec_in, False, COL_DEC0, [2, 3, 6, 7],
                            xpool, xtag, bufs=3, uid="xd0")
            highway_layers(10, 16, make_x2(xs, "d0") if use_x2 else None)
            xs = conv_block(w_dec1_sb, xs, True, COL_DEC1, [0, 1, 4, 5], xpool, xtag,
                            bufs=3, uid="xd1")
            xs = conv_block(w_dec2_sb, xs, True, COL_DEC2, [2, 3, 6, 7], xpool, xtag,
                            bufs=3, uid="xd2")
            xs = conv_block(w_dec3_sb, xs, True, COL_DEC3, [0, 1, 4, 5], xpool, xtag,
                            bufs=3, uid="xd3")

            # ---------- final: mel (sigmoid conv) + done, fused ----------
            for s in range(BS):
                fo = temp.tile([128, 4, F + 2], dt, tag=f"fin_{s}", name=f"fin_{s}")
                for tt in range(4):
                    ps = psum.tile([128, F + 2], F32, tag=f"bank{s * 4 + tt}",
                                   name=f"fps_{s}_{tt}")
                    for dc in range(2):
                        _mm(nc, ps, xs[s][dc][:, tt * 128 : (tt + 1) * 128],
                            w_last_sb[dc], start=(dc == 0), stop=False)
                    _mm(nc, ps, ones_row, blast_sb, start=False, stop=True)
                    nc.scalar.activation(fo[:, tt, :], ps, AF.Sigmoid, scale=1.0)
                nc.sync.dma_start(
                    d_mel[s].rearrange("(tt p) f -> p tt f", p=128), fo[:, :, 0:F])
                nc.sync.dma_start(
                    d_done[s].rearrange("(tt p) o -> p tt o", p=128),
                    fo[:, :, F : F + 1])

    nc.compile()
    return nc


def _prep_host(inputs):
    """Host-side packing: transposes, chunking, and packed const blocks."""
    f32 = np.float32
    mm_np = np.float16 if MM_DT == "f16" else np.float32

    def npm(a):
        return np.ascontiguousarray(np.asarray(a, dtype=f32)).astype(mm_np)

    # x0: (B, T, F) -> pad F to 512 -> (B, 128, 4, T)
    x_t = np.zeros((B, 512, T), f32)
    x_t[:, :F, :] = np.asarray(inputs["inputs"], f32).transpose(0, 2, 1)
    x0 = npm(x_t.reshape(B, 4, 128, T).transpose(0, 2, 1, 3))

    keysT = np.asarray(inputs["keys"], f32).transpose(0, 2, 1)  # (B, D, TE)
    values = np.asarray(inputs["values"], f32)  # (B, TE, D)

    w_all = np.concatenate([np.asarray(inputs["enc_hw_w"]),
                            np.asarray(inputs["dec_hw_w"])], axis=0)  # (16, 512, 256, 3)
    wt = w_all.transpose(0, 2, 1, 3)            # (L, ci, co, k)
    wt = wt.reshape(L, 2, 128, 4, 128, 3)       # (L, kc, p, mt, f, k)
    hw_w = npm(wt.transpose(0, 2, 1, 5, 3, 4))  # (L, 128, kc, k, mt, f)

    def t2(w):  # (O, I, 1) -> (I, O) fp32
        return np.asarray(w, f32)[:, :, 0].T

    # wenc0p: (400, 256) -> pad rows to 512 -> (128, 4, 256)
    we0 = np.zeros((512, D), f32)
    we0[:F] = t2(inputs["enc_w0"])
    wenc0p = npm(we0.reshape(4, 128, D).transpose(1, 0, 2))

    def chunks(w):  # (rows, cols) -> list of (128, cols)
        return [w[c * 128 : (c + 1) * 128] for c in range(w.shape[0] // 128)]

    pack1 = npm(np.concatenate(
        chunks(t2(inputs["enc_w1"])) + chunks(t2(inputs["enc_w2"])), axis=1))

    # wpack: [wq | wo | wdec0 | wdec1 | wdec2 | wdec3 | wlast | kT | v | ones | blast]
    w_q = np.asarray(inputs["attn_q_w"], f32).T
    w_o = np.asarray(inputs["attn_o_w"], f32).T * (math.sqrt(TE) * SQ2)
    w_last = np.concatenate(
        [np.asarray(inputs["last_w"], f32)[:, :, 0].T,
         np.asarray(inputs["fc_w"], f32).T,
         np.zeros((D, 1), f32)], axis=1)  # (256, 402)
    blocks = (chunks(w_q) + chunks(w_o) + chunks(t2(inputs["dec_w0"]))
              + chunks(t2(inputs["dec_w1"])) + chunks(t2(inputs["dec_w2"]))
              + chunks(t2(inputs["dec_w3"])) + chunks(w_last))
    # keys/values are per-core; build the shared prefix once
    prefix = np.concatenate(blocks, axis=1)  # (128, 4388)
    ones_blk = np.zeros((128, 128), f32)
    ones_blk[0, :] = 1.0
    blast_blk = np.zeros((128, F + 2), f32)
    blast_blk[0, :F] = np.asarray(inputs["last_b"], f32)
    blast_blk[0, F] = np.asarray(inputs["fc_b"], f32)[0]

    b_all = np.concatenate([np.asarray(inputs["enc_hw_b"]),
                            np.asarray(inputs["dec_hw_b"])], axis=0)  # (16, 512)
    hw_b = np.asarray(b_all, f32).reshape(L, 4, 128).transpose(2, 0, 1).reshape(128, L * 4)

    def cols(v):  # (256,) -> (128, 2)
        return np.asarray(v, dtype=f32).reshape(2, 128).T

    bias_tbl = np.zeros((128, NB), dtype=f32)
    bias_tbl[:, COL_ENC0:COL_ENC0 + 2] = cols(inputs["enc_b0"])
    bias_tbl[:, COL_ENC1:COL_ENC1 + 2] = cols(inputs["enc_b1"])
    bias_tbl[:, COL_ENC2:COL_ENC2 + 2] = cols(inputs["enc_b2"])
    bias_tbl[:, COL_QB:COL_QB + 2] = cols(inputs["attn_q_b"])
    bias_tbl[:, COL_OB:COL_OB + 2] = cols(np.asarray(inputs["attn_o_b"], f32) * SQ2)
    bias_tbl[:, COL_DEC0:COL_DEC0 + 2] = cols(inputs["dec_b0"])
    bias_tbl[:, COL_DEC1:COL_DEC1 + 2] = cols(inputs["dec_b1"])
    bias_tbl[:, COL_DEC2:COL_DEC2 + 2] = cols(inputs["dec_b2"])
    bias_tbl[:, COL_DEC3:COL_DEC3 + 2] = cols(inputs["dec_b3"])
    bias2 = np.ascontiguousarray(np.concatenate([bias_tbl, hw_b], axis=1))

    shared = dict(hw_w=hw_w, bias2=bias2, wenc0p=wenc0p, pack1=pack1)

    in_maps = []
    for i in range(N_CORES):
        sl = slice(i * BS, (i + 1) * BS)
        kv_blocks = []
        for s in range(BS):
            for c in range(2):
                kv_blocks.append(keysT[i * BS + s, c * 128 : (c + 1) * 128, :])
        for s in range(BS):
            for c in range(2):
                kv_blocks.append(values[i * BS + s, c * 128 : (c + 1) * 128, :])
        wpack = npm(np.concatenate(
            [prefix] + kv_blocks[:4] + kv_blocks[4:] + [ones_blk, blast_blk], axis=1))
        m = dict(shared)
        m["x0"] = np.ascontiguousarray(x0[sl])
        m["wpack"] = wpack
        in_maps.append(m)
    return in_maps


def kernel(**inputs):
    global LAST_EXEC_NS
    if "nc" not in _BUILD_CACHE:
        _BUILD_CACHE["nc"] = _build()
    nc = _BUILD_CACHE["nc"]

    in_maps = _prep_host(inputs)

    trace = os.environ.get("KBENCH_TRACE", "0") == "1"
    if trace:
        _install_ntff_hook()
    res = run_bass_kernel_spmd(nc, in_maps, core_ids=list(range(N_CORES)), trace=trace)
    LAST_EXEC_NS = res.exec_time_ns

    mel = np.concatenate([r["mel"] for r in res.results], axis=0)
    attn = np.concatenate([r["attn"] for r in res.results], axis=0)
    done = np.concatenate([r["done"] for r in res.results], axis=0)
    return mel, attn, done


def _install_ntff_hook():
    """Register the axon NTFF profiling hook (missing from this image's antenv)."""
    import types

    if "antenv.axon_hooks" in sys.modules:
        return
    m = types.ModuleType("antenv.axon_hooks")
    m._h = None
    m.set_axon_ntff_profile_hook = lambda h: setattr(m, "_h", h)
    m.get_axon_ntff_profile_hook = lambda: m._h
    sys.modules["antenv.axon_hooks"] = m
    try:
        import antenv

        antenv.axon_hooks = m
        from trn_agent_boot.trn_boot import _ntff_profile_via_ctypes

        m._h = _ntff_profile_via_ctypes("/opt/axon/libaxon_pjrt.so")
    except Exception:
        m._h = None


# revision 23
# speedup vs baseline: 1.4301x; 1.0657x over previous
"""Trainium2 Bass kernel for nn_Decoder_42417097016016 (DCTTS-style decoder).

Sharding: pure data parallel over batch. B=16 samples -> 8 NeuronCores x 2
samples each; all weights replicated per core.

Layout: activations live on-chip as (channels, time) so every conv1x1 /
causal conv is a PE matmul with channels on partitions.  Causal convs with
dilation d are 3 accumulating matmuls per output tile with column shifts
(0, d, 2d) - left zero-padding falls out of the shifted PSUM accumulation.

The attention block computes scores (t,s), softmax along free dim (ACT Exp
with accum_out row sums), writes the normalized attn output directly, and
PE-transposes it to (s,t) for the context matmul.  mel and done share one
final matmul by concatenating fc_w as a 401st output column (padded to 402).

Matmul precision knob KBENCH_MM: "f16" (default, full PE rate), "f32r"
(reduced-precision fp32, half rate, needs even matmul geometry -> shifted x2
copies for odd-dilation taps), "f32" (exact, quarter rate).
"""

import math
import os
import sys

import numpy as np

for _p in ("/opt/trn_rl_repo", "/root/.axon_site/_ro/trn_rl_repo"):
    if os.path.isdir(_p) and _p not in sys.path:
        sys.path.append(_p)

import concourse.bass as bass
import concourse.tile as tile
from concourse import bacc, mybir
from concourse.bass_utils import run_bass_kernel_spmd

AF = mybir.ActivationFunctionType
ALU = mybir.AluOpType
AX = mybir.AxisListType
F32 = mybir.dt.float32
F32R = mybir.dt.float32r
F16 = mybir.dt.float16

N_CORES = 8
B, T, TE, D, F = 16, 512, 256, 256, 400
BS = B // N_CORES  # samples per core
DIL = [1, 3, 9, 27, 1, 3, 9, 27, 3, 3] + [1, 3, 9, 27, 1, 1]
L = len(DIL)  # 16 highway layers (10 encoder + 6 decoder)
SQ2 = math.sqrt(0.5)

MM_DT = os.environ.get("KBENCH_MM", "f16")

# bias table column assignments
COL_ENC0, COL_ENC1, COL_ENC2 = 0, 2, 4
COL_QB, COL_OB = 6, 8
COL_DEC0, COL_DEC1, COL_DEC2, COL_DEC3 = 10, 12, 14, 16
NB = 18

LAST_EXEC_NS = None
_BUILD_CACHE = {}


def _mm(nc, out, lhsT, rhs, **kw):
    nc.tensor.matmul(out, lhsT, rhs, **kw)


def _build():
    """Build the per-core Bass program (same program on all 8 cores)."""
    from concourse.masks import make_identity

    nc = bacc.Bacc("TRN2", target_bir_lowering=False, debug=False)
    dt = F32
    dtm = {"f16": F16, "f32r": F32R, "f32": F32}[MM_DT]
    use_x2 = MM_DT == "f32r"  # f32r matmuls need even offsets/N

    # ---- DRAM I/O (per-core shard shapes) ----
    # x0: (s, p, c, t) pre-chunked+padded on host -> one DMA per sample
    d_x0 = nc.dram_tensor("x0", [BS, 128, 4, T], dtm, kind="ExternalInput").ap()
    d_hw_w = nc.dram_tensor("hw_w", [L, 128, 2, 3, 4, 128], dtm, kind="ExternalInput").ap()
    # wenc0p: enc_w0^T padded 400->512 rows, chunked (128, 4, 256)
    d_wenc0 = nc.dram_tensor("wenc0p", [128, 4, D], dtm, kind="ExternalInput").ap()
    # pack1: [enc1 c0|c1, enc2 c0|c1] as (128, 4*256)
    d_pack1 = nc.dram_tensor("pack1", [128, 4 * D], dtm, kind="ExternalInput").ap()
    # bias2: [bias_tbl (18) | hw_b (L*4)] fp32
    d_bias2 = nc.dram_tensor("bias2", [128, NB + L * 4], dt, kind="ExternalInput").ap()
    # wpack: all attention/decoder weights + keys/values + ones/blast rows
    WQ_O, WO_O = 0, 512
    WD0_O, WD1_O, WD2_O, WD3_O = 1024, 2048, 2560, 3072
    WLAST_O = 3584
    KT_O = WLAST_O + 2 * (F + 2)          # 4388
    V_O = KT_O + 4 * TE                   # 5412
    ONES_O = V_O + 4 * D                  # 6436
    BLAST_O = ONES_O + 128                # 6564
    WPACK_COLS = BLAST_O + (F + 2)        # 6966
    d_wpack = nc.dram_tensor("wpack", [128, WPACK_COLS], dtm, kind="ExternalInput").ap()

    d_mel = nc.dram_tensor("mel", [BS, T, F], dt, kind="ExternalOutput").ap()
    d_attn = nc.dram_tensor("attn", [BS, T, TE], dt, kind="ExternalOutput").ap()
    d_done = nc.dram_tensor("done", [BS, T, 1], dt, kind="ExternalOutput").ap()

    with tile.TileContext(nc) as tc:
        with (
            tc.tile_pool(name="const", bufs=1) as const,
            tc.tile_pool(name="xpool", bufs=1) as xpool,
            tc.tile_pool(name="persist", bufs=1) as persist,
            tc.tile_pool(name="wstream", bufs=1) as wstream,
            tc.tile_pool(name="temp", bufs=1) as temp,
            tc.tile_pool(name="psum", bufs=1, space="PSUM") as psum,
        ):
            # ---------- startup-critical loads first (DMA queue is FIFO) ----
            wenc0_sb = const.tile([128, 4, D], dtm, name="wenc0_sb")
            nc.sync.dma_start(wenc0_sb, d_wenc0)
            w_enc0_sb = [wenc0_sb[:, c, :] for c in range(4)]
            xin = {}
            for s in range(BS):
                xt = temp.tile([128, 4, T], dtm, tag=f"xin_{s}", name=f"xin_{s}")
                nc.sync.dma_start(xt, d_x0[s])
                xin[s] = [xt[:, c, :] for c in range(4)]
            pack1_sb = const.tile([128, 4 * D], dtm, name="pack1_sb")
            nc.sync.dma_start(pack1_sb, d_pack1)
            w_enc1_sb = [pack1_sb[:, 0:D], pack1_sb[:, D : 2 * D]]
            w_enc2_sb = [pack1_sb[:, 2 * D : 3 * D], pack1_sb[:, 3 * D : 4 * D]]
            bias2_sb = const.tile([128, NB + L * 4], dt, name="bias2_sb")
            nc.sync.dma_start(bias2_sb, d_bias2)
            bias_sb = bias2_sb[:, 0:NB]

            def hwb_ap(l, j):
                c = NB + l * 4 + j
                return bias2_sb[:, c : c + 1]

            ident = const.tile([128, 128], dt, name="ident")
            make_identity(nc, ident)
            stat_sb = const.tile([128, 32], F32, name="stat_sb")
            # HAM warm-up: keep the PE busy during the input DMA wait so the
            # clock gate is already at 8/8 when real matmuls arrive.
            warm_ps = psum.tile([128, 128], F32, tag="bank7", name="warm_ps")
            for _w in range(24):
                nc.tensor.matmul(warm_ps, ident, ident, is_transpose=True,
                                 start=True, stop=True, skip_group_check=True)

            # helper: one conv1x1 block (256 outputs) for all samples.
            # Epilogue on DVE: out = relu?(psum + bias)
            def conv_block(w_tiles, rhs_per_s, relu, bias_col, banks, out_pool,
                           tag_fn, bufs=1, uid=""):
                outs = {s: [None, None] for s in range(BS)}
                pss = {}
                for mt in range(2):
                    for s in range(BS):
                        ps = psum.tile(
                            [128, T], F32,
                            tag=f"bank{banks[s * 2 + mt]}",
                            name=f"ps_{uid}_{s}_{mt}",
                        )
                        nk = len(w_tiles)
                        for c in range(nk):
                            _mm(nc, ps, w_tiles[c][:, mt * 128 : (mt + 1) * 128],
                                rhs_per_s[s][c], start=(c == 0), stop=(c == nk - 1))
                        pss[(s, mt)] = ps
                for mt in range(2):
                    for s in range(BS):
                        ot = out_pool.tile(
                            [128, T], dtm,
                            tag=tag_fn(s, mt), bufs=bufs,
                            name=f"{uid}_{s}_{mt}",
                        )
                        b_ap = bias_sb[:, bias_col + mt : bias_col + mt + 1]
                        if relu:
                            nc.vector.tensor_scalar(ot, pss[(s, mt)], b_ap, 0.0,
                                                    op0=ALU.add, op1=ALU.max)
                        else:
                            nc.vector.tensor_scalar_add(ot, pss[(s, mt)], b_ap)
                        outs[s][mt] = ot
                return outs

            xtag = lambda s, mt: f"x_{s}_{mt}"
            gatag = lambda s, mt: f"ga_{s}_{mt}"
            gbtag = lambda s, mt: f"gb_{s}_{mt}"

            # ---------- encoder head ----------
            h1 = conv_block(w_enc0_sb, xin, True, COL_ENC0, [0, 1, 4, 5], temp, gatag, uid="h1")
            h2 = conv_block(w_enc1_sb, h1, True, COL_ENC1, [2, 3, 6, 7], temp, gbtag, uid="h2")
            xs = conv_block(w_enc2_sb, h2, False, COL_ENC2, [0, 1, 4, 5], xpool, xtag,
                            bufs=3, uid="xe")

            # remaining const loads are deferred (emitted mid-encoder so the
            # first highway-layer weight DMAs aren't stuck behind them)
            misc = {}

            def load_misc_consts():
                wp = const.tile([128, WPACK_COLS], dtm, name="wpack_sb")
                nc.sync.dma_start(wp, d_wpack)
                misc["w_q"] = [wp[:, WQ_O : WQ_O + D], wp[:, WQ_O + D : WQ_O + 2 * D]]
                misc["w_o"] = [wp[:, WO_O : WO_O + D], wp[:, WO_O + D : WO_O + 2 * D]]
                misc["w_dec0"] = [wp[:, WD0_O + c * D : WD0_O + (c + 1) * D] for c in range(4)]
                misc["w_dec1"] = [wp[:, WD1_O : WD1_O + D], wp[:, WD1_O + D : WD1_O + 2 * D]]
                misc["w_dec2"] = [wp[:, WD2_O : WD2_O + D], wp[:, WD2_O + D : WD2_O + 2 * D]]
                misc["w_dec3"] = [wp[:, WD3_O : WD3_O + D], wp[:, WD3_O + D : WD3_O + 2 * D]]
                misc["w_last"] = [wp[:, WLAST_O : WLAST_O + F + 2],
                                  wp[:, WLAST_O + F + 2 : WLAST_O + 2 * (F + 2)]]
                misc["ones"] = wp[0:2, ONES_O : ONES_O + 128]
                misc["blast"] = wp[0:2, BLAST_O : BLAST_O + F + 2]
                kT_sb, v_sb = {}, {}
                for s in range(BS):
                    for c in range(2):
                        i = s * 2 + c
                        kT_sb[(s, c)] = wp[:, KT_O + i * TE : KT_O + (i + 1) * TE]
                        v_sb[(s, c)] = wp[:, V_O + i * D : V_O + (i + 1) * D]
                misc["kT"] = kT_sb
                misc["v"] = v_sb

            # ---------- highway stack ----------
            def make_x2(xs_cur, uid):
                # right-shifted copy (col0 = 0): makes the odd-dilation middle
                # conv tap even-aligned for the f32r matmul mode.
                out = {s: [None, None] for s in range(BS)}
                for c in range(2):
                    for s in range(BS):
                        x2 = xpool.tile([128, T], dtm, tag=f"x2_{s}_{c}", bufs=2,
                                        name=f"x2_{uid}_{s}_{c}")
                        nc.gpsimd.tensor_scalar_mul(x2[:, 0:1], xs_cur[s][c][:, 0:1], 0.0)
                        nc.gpsimd.tensor_copy(x2[:, 1:T], xs_cur[s][c][:, 0 : T - 1])
                        out[s][c] = x2
                return out

            def highway_layers(l_lo, l_hi, x2s):
                nonlocal xs
                for l in range(l_lo, l_hi):
                    dil = DIL[l]
                    wt = wstream.tile([128, 2, 3, 4, 128], dtm, tag="hww", bufs=2,
                                      name=f"hw_w_{l}")
                    nc.sync.dma_start(wt, d_hw_w[l])
                    last_enc = l == 9
                    ps_all = {}
                    for mt in range(4):
                        for s in range(BS):
                            ps_all[(mt, s)] = psum.tile(
                                [128, T], F32, tag=f"bank{mt * 2 + s}",
                                name=f"hwps_{l}_{mt}_{s}",
                            )
                    # kc-major: ALL kc0 taps (24 matmuls) before any kc1 tap,
                    # giving the previous layer's x'[1] epilogue a ~5us runway.
                    # mt order (2,0,3,1): chunk-0 gate+input banks finish first
                    # so its epilogue chain overlaps the remaining matmuls.
                    seen = {}
                    for kc in range(2):
                        for mt in (2, 0, 3, 1):
                            for k in (2, 1, 0):
                                first = kc == 0 and k == 2
                                last = kc == 1 and k == 0
                                for s in range(BS):
                                    ps = ps_all[(mt, s)]
                                    if k == 2:
                                        _mm(nc, ps, wt[:, kc, k, mt, :], xs[s][kc],
                                            start=first, stop=last)
                                    elif k == 1:
                                        if use_x2:
                                            off = dil - 1
                                            _mm(nc, ps[:, off:T], wt[:, kc, k, mt, :],
                                                x2s[s][kc][:, 0 : T - off],
                                                start=first, stop=last)
                                        else:
                                            _mm(nc, ps[:, dil:T], wt[:, kc, k, mt, :],
                                                xs[s][kc][:, 0 : T - dil],
                                                start=first, stop=last)
                                    else:
                                        sh = 2 * dil
                                        _mm(nc, ps[:, sh:T], wt[:, kc, k, mt, :],
                                            xs[s][kc][:, 0 : T - sh],
                                            start=first, stop=last)
                    # epilogue: x' = x + sigmoid(g) * (a - x)
                    # per chunk c: g = bank (c+2), a = bank c
                    newxs = {s: [None, None] for s in range(BS)}
                    for c in range(2):
                        tgs = {}
                        for s in range(BS):
                            tg = temp.tile([128, T], dtm, tag=f"tg_{s}_{c}", bufs=2,
                                           name=f"tg_{l}_{s}_{c}")
                            nc.scalar.activation(
                                tg, ps_all[(c + 2, s)], AF.Sigmoid,
                                bias=hwb_ap(l, c + 2), scale=1.0)
                            tgs[s] = tg
                        tmps = {}
                        for s in range(BS):
                            tmp = temp.tile([128, T], dtm, tag=f"tmp_{s}_{c}", bufs=2,
                                            name=f"tmp_{l}_{s}_{c}")
                            nc.vector.scalar_tensor_tensor(
                                tmp, ps_all[(c, s)], hwb_ap(l, c),
                                xs[s][c], op0=ALU.add, op1=ALU.subtract)
                            tmps[s] = tmp
                        for s in range(BS):
                            nc.gpsimd.tensor_mul(tmps[s], tgs[s], tmps[s])
                        for s in range(BS):
                            if last_enc:
                                xn = persist.tile([128, T], dtm, tag=f"q_{s}_{c}",
                                                  name=f"Q_{s}_{c}")
                            else:
                                xn = xpool.tile([128, T], dtm, tag=f"x_{s}_{c}", bufs=3,
                                                name=f"x_{l + 1}_{s}_{c}")
                            nc.vector.tensor_add(xn, tmps[s], xs[s][c])
                            newxs[s][c] = xn
                    xs = newxs
                    if use_x2 and l + 1 < l_hi:
                        x2s = make_x2(xs, f"l{l + 1}")
                    if l == 1:
                        load_misc_consts()

            # ---------- encoder highway ----------
            highway_layers(0, 10, make_x2(xs, "e0") if use_x2 else None)
            w_q_sb, w_o_sb = misc["w_q"], misc["w_o"]
            w_dec0_sb, w_dec1_sb = misc["w_dec0"], misc["w_dec1"]
            w_dec2_sb, w_dec3_sb = misc["w_dec2"], misc["w_dec3"]
            w_last_sb, ones_row, blast_sb = misc["w_last"], misc["ones"], misc["blast"]
            kT_sb, v_sb = misc["kT"], misc["v"]
            Qs = xs  # persisted encoder output (D, T) tiles

            # ---------- attention ----------
            Rqs = {s: [] for s in range(BS)}
            q_sb = {s: [] for s in range(BS)}
            for mt in range(2):
                for s in range(BS):
                    ps = psum.tile([128, T], F32, tag=f"bank{s * 4 + mt}",
                                   name=f"qps_{s}_{mt}")
                    for kc in range(2):
                        _mm(nc, ps, w_q_sb[kc][:, mt * 128 : (mt + 1) * 128], Qs[s][kc],
                            start=(kc == 0), stop=(kc == 1))
                    qt = temp.tile([128, T], dtm, tag=f"ga_{s}_{mt}", name=f"q_{s}_{mt}")
                    nc.vector.tensor_scalar_add(
                        qt, ps, bias_sb[:, COL_QB + mt : COL_QB + mt + 1])
                    q_sb[s].append(qt)

            # softmax without max-subtraction: scores are O(10), exp is safe in
            # fp32 and softmax is shift-invariant, so this matches the reference.
            aT = {s: [] for s in range(BS)}
            for s in range(BS):
                at_s = temp.tile([128, 4, TE], dt, tag=f"att_{s}", name=f"att_{s}")
                psts = [psum.tile([128, T], F32, tag=f"bank{s * 4 + 2 + sc}",
                                  name=f"tps_{s}_{sc}") for sc in range(2)]
                for tt in range(4):
                    ps = psum.tile([128, TE], F32, tag=f"bank{s * 4 + (tt % 2)}",
                                   name=f"sps_{s}_{tt}")
                    for dc in range(2):
                        _mm(nc, ps, q_sb[s][dc][:, tt * 128 : (tt + 1) * 128],
                            kT_sb[(s, dc)], start=(dc == 0), stop=(dc == 1))
                    st = stat_sb[:, (s * 4 + tt) * 4 : (s * 4 + tt) * 4 + 4]
                    at = at_s[:, tt, :]
                    nc.scalar.activation(at, ps, AF.Exp, accum_out=st[:, 1:2])
                    nc.vector.reciprocal(st[:, 2:3], st[:, 1:2])
                    nc.vector.tensor_scalar_mul(at, at, st[:, 2:3])
                    for sc in range(2):
                        nc.tensor.matmul(
                            psts[sc][:, tt * 128 : (tt + 1) * 128],
                            at[:, sc * 128 : (sc + 1) * 128],
                            ident, is_transpose=True, start=True, stop=True,
                            skip_group_check=True)
                nc.sync.dma_start(
                    d_attn[s].rearrange("(tt p) e -> p tt e", p=128), at_s)
                for sc in range(2):
                    a2 = temp.tile([128, T], dtm, tag=f"gb_{s}_{sc}", name=f"aT_{s}_{sc}")
                    nc.vector.tensor_copy(a2, psts[sc])
                    aT[s].append(a2)

            ctx_sb = {s: [] for s in range(BS)}
            for s in range(BS):
                for dc in range(2):
                    ps = psum.tile([128, T], F32, tag=f"bank{s * 4 + dc}",
                                   name=f"cps_{s}_{dc}")
                    for sc in range(2):
                        _mm(nc, ps, v_sb[(s, sc)][:, dc * 128 : (dc + 1) * 128],
                            aT[s][sc], start=(sc == 0), stop=(sc == 1))
                    ct = temp.tile([128, T], dtm, tag=f"ga_{s}_{dc}", name=f"ctx_{s}_{dc}")
                    nc.vector.tensor_copy(ct, ps)
                    ctx_sb[s].append(ct)

            for mt in range(2):
                for s in range(BS):
                    ps = psum.tile([128, T], F32, tag=f"bank{s * 4 + mt}",
                                   name=f"ops_{s}_{mt}")
                    for dc in range(2):
                        _mm(nc, ps, w_o_sb[dc][:, mt * 128 : (mt + 1) * 128],
                            ctx_sb[s][dc], start=(dc == 0), stop=(dc == 1))
                    tmpo = temp.tile([128, T], dt, tag=f"gb_{s}_{mt}",
                                     name=f"tmpo_{s}_{mt}")
                    nc.vector.tensor_scalar_add(
                        tmpo, ps, bias_sb[:, COL_OB + mt : COL_OB + mt + 1])
                    rq = persist.tile([128, T], dtm, tag=f"rq_{s}_{mt}",
                                      name=f"rq_{s}_{mt}")
                    # Rq = sqrt(.5)*query + out_proj  [scales folded into w_o/b_o]
                    nc.vector.scalar_tensor_tensor(
                        rq, Qs[s][mt], SQ2, tmpo, op0=ALU.mult, op1=ALU.add)
                    Rqs[s].append(rq)

            # ---------- decoder ----------
            dec_in = {s: [Qs[s][0], Qs[s][1], Rqs[s][0], Rqs[s][1]] for s in range(BS)}
            w_dec0_r = [w_dec0_sb[2], w_dec0_sb[3], w_dec0_sb[0], w_dec0_sb[1]]
            xs = conv_block(w_dec0_r, dec_in, False, COL_DEC0, [2, 3, 6, 7],
                            xpool, xtag, bufs=3, uid="xd0")
            highway_layers(10, 16, make_x2(xs, "d0") if use_x2 else None)
            xs = conv_block(w_dec1_sb, xs, True, COL_DEC1, [0, 1, 4, 5], xpool, xtag,
                            bufs=3, uid="xd1")
            xs = conv_block(w_dec2_sb, xs, True, COL_DEC2, [2, 3, 6, 7], xpool, xtag,
                            bufs=3, uid="xd2")
            xs = conv_block(w_dec3_sb, xs, True, COL_DEC3, [0, 1, 4, 5], xpool, xtag,
                            bufs=3, uid="xd3")

            # ---------- final: mel (per-tt sigmoid conv) + done ((1,T) row) ----
            for s in range(BS):
                # done = sigmoid(fc . x) computed as a single-row matmul so the
                # output DMA is one contiguous 2KB write (not a 512-desc scatter)
                psd = psum.tile([1, T], F32, tag=f"bank{s * 4 + 3}", name=f"dps_{s}")
                for dc in range(2):
                    _mm(nc, psd, w_last_sb[dc][:, F : F + 1], xs[s][dc],
                        start=(dc == 0), stop=(dc == 1))
                dn = temp.tile([1, T], dt, tag=f"done_{s}", name=f"done_{s}")
                nc.scalar.activation(dn, psd, AF.Sigmoid, scale=1.0,
                                     bias=blast_sb[0:1, F : F + 1])
                nc.sync.dma_start(d_done[s].rearrange("t o -> o t"), dn)
            for s in range(BS):
                fo = temp.tile([128, 4, F + 2], dt, tag=f"fin_{s}", name=f"fin_{s}")
                for tt in range(4):
                    ps = psum.tile([128, F + 2], F32, tag=f"bank{s * 4 + tt}",
                                   name=f"fps_{s}_{tt}")
                    for dc in range(2):
                        _mm(nc, ps, xs[s][dc][:, tt * 128 : (tt + 1) * 128],
                            w_last_sb[dc], start=(dc == 0), stop=False)
                    _mm(nc, ps, ones_row, blast_sb, start=False, stop=True)
                    nc.scalar.activation(fo[:, tt, :], ps, AF.Sigmoid, scale=1.0)
                    nc.sync.dma_start(d_mel[s, tt * 128 : (tt + 1) * 128, :],
                                      fo[:, tt, 0:F])

    nc.compile()
    return nc


def _prep_host(inputs):
    """Host-side packing: transposes, chunking, and packed const blocks."""
    f32 = np.float32
    mm_np = np.float16 if MM_DT == "f16" else np.float32

    def npm(a):
        return np.ascontiguousarray(np.asarray(a, dtype=f32)).astype(mm_np)

    # x0: (B, T, F) -> pad F to 512 -> (B, 128, 4, T)
    x_t = np.zeros((B, 512, T), f32)
    x_t[:, :F, :] = np.asarray(inputs["inputs"], f32).transpose(0, 2, 1)
    x0 = npm(x_t.reshape(B, 4, 128, T).transpose(0, 2, 1, 3))

    keysT = np.asarray(inputs["keys"], f32).transpose(0, 2, 1)  # (B, D, TE)
    values = np.asarray(inputs["values"], f32)  # (B, TE, D)

    w_all = np.concatenate([np.asarray(inputs["enc_hw_w"]),
                            np.asarray(inputs["dec_hw_w"])], axis=0)  # (16, 512, 256, 3)
    wt = w_all.transpose(0, 2, 1, 3)            # (L, ci, co, k)
    wt = wt.reshape(L, 2, 128, 4, 128, 3)       # (L, kc, p, mt, f, k)
    hw_w = npm(wt.transpose(0, 2, 1, 5, 3, 4))  # (L, 128, kc, k, mt, f)

    def t2(w):  # (O, I, 1) -> (I, O) fp32
        return np.asarray(w, f32)[:, :, 0].T

    # wenc0p: (400, 256) -> pad rows to 512 -> (128, 4, 256)
    we0 = np.zeros((512, D), f32)
    we0[:F] = t2(inputs["enc_w0"])
    wenc0p = npm(we0.reshape(4, 128, D).transpose(1, 0, 2))

    def chunks(w):  # (rows, cols) -> list of (128, cols)
        return [w[c * 128 : (c + 1) * 128] for c in range(w.shape[0] // 128)]

    pack1 = npm(np.concatenate(
        chunks(t2(inputs["enc_w1"])) + chunks(t2(inputs["enc_w2"])), axis=1))

    # wpack: [wq | wo | wdec0 | wdec1 | wdec2 | wdec3 | wlast | kT | v | ones | blast]
    w_q = np.asarray(inputs["attn_q_w"], f32).T
    w_o = np.asarray(inputs["attn_o_w"], f32).T * (math.sqrt(TE) * SQ2)
    w_last = np.concatenate(
        [np.asarray(inputs["last_w"], f32)[:, :, 0].T,
         np.asarray(inputs["fc_w"], f32).T,
         np.zeros((D, 1), f32)], axis=1)  # (256, 402)
    blocks = (chunks(w_q) + chunks(w_o) + chunks(t2(inputs["dec_w0"]))
              + chunks(t2(inputs["dec_w1"])) + chunks(t2(inputs["dec_w2"]))
              + chunks(t2(inputs["dec_w3"])) + chunks(w_last))
    # keys/values are per-core; build the shared prefix once
    prefix = np.concatenate(blocks, axis=1)  # (128, 4388)
    ones_blk = np.zeros((128, 128), f32)
    ones_blk[0, :] = 1.0
    blast_blk = np.zeros((128, F + 2), f32)
    blast_blk[0, :F] = np.asarray(inputs["last_b"], f32)
    blast_blk[0, F] = np.asarray(inputs["fc_b"], f32)[0]

    b_all = np.concatenate([np.asarray(inputs["enc_hw_b"]),
                            np.asarray(inputs["dec_hw_b"])], axis=0)  # (16, 512)
    hw_b = np.asarray(b_all, f32).reshape(L, 4, 128).transpose(2, 0, 1).reshape(128, L * 4)

    def cols(v):  # (256,) -> (128, 2)
        return np.asarray(v, dtype=f32).reshape(2, 128).T

    bias_tbl = np.zeros((128, NB), dtype=f32)
    bias_tbl[:, COL_ENC0:COL_ENC0 + 2] = cols(inputs["enc_b0"])
    bias_tbl[:, COL_ENC1:COL_ENC1 + 2] = cols(inputs["enc_b1"])
    bias_tbl[:, COL_ENC2:COL_ENC2 + 2] = cols(inputs["enc_b2"])
    bias_tbl[:, COL_QB:COL_QB + 2] = cols(inputs["attn_q_b"])
    bias_tbl[:, COL_OB:COL_OB + 2] = cols(np.asarray(inputs["attn_o_b"], f32) * SQ2)
    bias_tbl[:, COL_DEC0:COL_DEC0 + 2] = cols(inputs["dec_b0"])
    bias_tbl[:, COL_DEC1:COL_DEC1 + 2] = cols(inputs["dec_b1"])
    bias_tbl[:, COL_DEC2:COL_DEC2 + 2] = cols(inputs["dec_b2"])
    bias_tbl[:, COL_DEC3:COL_DEC3 + 2] = cols(inputs["dec_b3"])
    bias2 = np.ascontiguousarray(np.concatenate([bias_tbl, hw_b], axis=1))

    shared = dict(hw_w=hw_w, bias2=bias2, wenc0p=wenc0p, pack1=pack1)

    in_maps = []
    for i in range(N_CORES):
        sl = slice(i * BS, (i + 1) * BS)
        kv_blocks = []
        for s in range(BS):
            for c in range(2):
                kv_blocks.append(keysT[i * BS + s, c * 128 : (c + 1) * 128, :])
        for s in range(BS):
            for c in range(2):
                kv_blocks.append(values[i * BS + s, c * 128 : (c + 1) * 128, :])
        wpack = npm(np.concatenate(
            [prefix] + kv_blocks[:4] + kv_blocks[4:] + [ones_blk, blast_blk], axis=1))
        m = dict(shared)
        m["x0"] = np.ascontiguousarray(x0[sl])
        m["wpack"] = wpack
        in_maps.append(m)
    return in_maps


def kernel(**inputs):
    global LAST_EXEC_NS
    if "nc" not in _BUILD_CACHE:
        _BUILD_CACHE["nc"] = _build()
    nc = _BUILD_CACHE["nc"]

    in_maps = _prep_host(inputs)

    trace = os.environ.get("KBENCH_TRACE", "0") == "1"
    if trace:
        _install_ntff_hook()
    res = run_bass_kernel_spmd(nc, in_maps, core_ids=list(range(N_CORES)), trace=trace)
    LAST_EXEC_NS = res.exec_time_ns

    mel = np.concatenate([r["mel"] for r in res.results], axis=0)
    attn = np.concatenate([r["attn"] for r in res.results], axis=0)
    done = np.concatenate([r["done"] for r in res.results], axis=0)
    return mel, attn, done


def _install_ntff_hook():
    """Register the axon NTFF profiling hook (missing from this image's antenv)."""
    import types

    if "antenv.axon_hooks" in sys.modules:
        return
    m = types.ModuleType("antenv.axon_hooks")
    m._h = None
    m.set_axon_ntff_profile_hook = lambda h: setattr(m, "_h", h)
    m.get_axon_ntff_profile_hook = lambda: m._h
    sys.modules["antenv.axon_hooks"] = m
    try:
        import antenv

        antenv.axon_hooks = m
        from trn_agent_boot.trn_boot import _ntff_profile_via_ctypes

        m._h = _ntff_profile_via_ctypes("/opt/axon/libaxon_pjrt.so")
    except Exception:
        m._h = None


# revision 25
# speedup vs baseline: 1.4527x; 1.0158x over previous
"""Trainium2 Bass kernel for nn_Decoder_42417097016016 (DCTTS-style decoder).

Sharding: pure data parallel over batch. B=16 samples -> 8 NeuronCores x 2
samples each; all weights replicated per core.

Layout: activations live on-chip as (channels, time) so every conv1x1 /
causal conv is a PE matmul with channels on partitions.  Causal convs with
dilation d are 3 accumulating matmuls per output tile with column shifts
(0, d, 2d) - left zero-padding falls out of the shifted PSUM accumulation.

The attention block computes scores (t,s), softmax along free dim (ACT Exp
with accum_out row sums), writes the normalized attn output directly, and
PE-transposes it to (s,t) for the context matmul.  mel and done share one
final matmul by concatenating fc_w as a 401st output column (padded to 402).

Matmul precision knob KBENCH_MM: "f16" (default, full PE rate), "f32r"
(reduced-precision fp32, half rate, needs even matmul geometry -> shifted x2
copies for odd-dilation taps), "f32" (exact, quarter rate).
"""

import math
import os
import sys

import numpy as np

for _p in ("/opt/trn_rl_repo", "/root/.axon_site/_ro/trn_rl_repo"):
    if os.path.isdir(_p) and _p not in sys.path:
        sys.path.append(_p)

import concourse.bass as bass
import concourse.tile as tile
from concourse import bacc, mybir
from concourse.bass_utils import run_bass_kernel_spmd

AF = mybir.ActivationFunctionType
ALU = mybir.AluOpType
AX = mybir.AxisListType
F32 = mybir.dt.float32
F32R = mybir.dt.float32r
F16 = mybir.dt.float16

N_CORES = 8
B, T, TE, D, F = 16, 512, 256, 256, 400
BS = B // N_CORES  # samples per core
DIL = [1, 3, 9, 27, 1, 3, 9, 27, 3, 3] + [1, 3, 9, 27, 1, 1]
L = len(DIL)  # 16 highway layers (10 encoder + 6 decoder)
SQ2 = math.sqrt(0.5)

MM_DT = os.environ.get("KBENCH_MM", "f16")

# bias table column assignments
COL_ENC0, COL_ENC1, COL_ENC2 = 0, 2, 4
COL_QB, COL_OB = 6, 8
COL_DEC0, COL_DEC1, COL_DEC2, COL_DEC3 = 10, 12, 14, 16
NB = 18

LAST_EXEC_NS = None
_BUILD_CACHE = {}


def _mm(nc, out, lhsT, rhs, **kw):
    nc.tensor.matmul(out, lhsT, rhs, **kw)


def _build():
    """Build the per-core Bass program (same program on all 8 cores)."""
    from concourse.masks import make_identity

    nc = bacc.Bacc("TRN2", target_bir_lowering=False, debug=False)
    dt = F32
    dtm = {"f16": F16, "f32r": F32R, "f32": F32}[MM_DT]
    use_x2 = MM_DT == "f32r"  # f32r matmuls need even offsets/N

    # ---- DRAM I/O (per-core shard shapes) ----
    # x0: (s, p, c, t) pre-chunked+padded on host -> one DMA per sample
    d_x0 = nc.dram_tensor("x0", [BS, 128, 4, T], dtm, kind="ExternalInput").ap()
    d_hw_w = nc.dram_tensor("hw_w", [L, 128, 2, 3, 4, 128], dtm, kind="ExternalInput").ap()
    # wenc0p: enc_w0^T padded 400->512 rows, chunked (128, 4, 256)
    d_wenc0 = nc.dram_tensor("wenc0p", [128, 4, D], dtm, kind="ExternalInput").ap()
    # pack1: [enc1 c0|c1, enc2 c0|c1] as (128, 4*256)
    d_pack1 = nc.dram_tensor("pack1", [128, 4 * D], dtm, kind="ExternalInput").ap()
    # bias2: [bias_tbl (18) | hw_b (L*4)] fp32
    d_bias2 = nc.dram_tensor("bias2", [128, NB + L * 4], dt, kind="ExternalInput").ap()
    # wpack: all attention/decoder weights + keys/values + ones/blast rows
    WQ_O, WO_O = 0, 512
    WD0_O, WD1_O, WD2_O, WD3_O = 1024, 2048, 2560, 3072
    WLAST_O = 3584
    KT_O = WLAST_O + 2 * (F + 2)          # 4388
    V_O = KT_O + 4 * TE                   # 5412
    ONES_O = V_O + 4 * D                  # 6436
    BLAST_O = ONES_O + 128                # 6564
    WPACK_COLS = BLAST_O + (F + 2)        # 6966
    d_wpack = nc.dram_tensor("wpack", [128, WPACK_COLS], dtm, kind="ExternalInput").ap()

    d_mel = nc.dram_tensor("mel", [BS, T, F], dt, kind="ExternalOutput").ap()
    d_attn = nc.dram_tensor("attn", [BS, T, TE], dt, kind="ExternalOutput").ap()
    d_done = nc.dram_tensor("done", [BS, T, 1], dt, kind="ExternalOutput").ap()

    with tile.TileContext(nc) as tc:
        with (
            tc.tile_pool(name="const", bufs=1) as const,
            tc.tile_pool(name="xpool", bufs=1) as xpool,
            tc.tile_pool(name="persist", bufs=1) as persist,
            tc.tile_pool(name="wstream", bufs=1) as wstream,
            tc.tile_pool(name="temp", bufs=1) as temp,
            tc.tile_pool(name="psum", bufs=1, space="PSUM") as psum,
        ):
            # ---------- startup-critical loads first (DMA queue is FIFO) ----
            wenc0_sb = const.tile([128, 4, D], dtm, name="wenc0_sb")
            nc.sync.dma_start(wenc0_sb, d_wenc0)
            w_enc0_sb = [wenc0_sb[:, c, :] for c in range(4)]
            xin = {}
            for s in range(BS):
                xt = temp.tile([128, 4, T], dtm, tag=f"xin_{s}", name=f"xin_{s}")
                nc.sync.dma_start(xt, d_x0[s])
                xin[s] = [xt[:, c, :] for c in range(4)]
            pack1_sb = const.tile([128, 4 * D], dtm, name="pack1_sb")
            nc.sync.dma_start(pack1_sb, d_pack1)
            w_enc1_sb = [pack1_sb[:, 0:D], pack1_sb[:, D : 2 * D]]
            w_enc2_sb = [pack1_sb[:, 2 * D : 3 * D], pack1_sb[:, 3 * D : 4 * D]]
            bias2_sb = const.tile([128, NB + L * 4], dt, name="bias2_sb")
            nc.sync.dma_start(bias2_sb, d_bias2)
            bias_sb = bias2_sb[:, 0:NB]

            def hwb_ap(l, j):
                c = NB + l * 4 + j
                return bias2_sb[:, c : c + 1]

            ident = const.tile([128, 128], dt, name="ident")
            make_identity(nc, ident)
            stat_sb = const.tile([128, 32], F32, name="stat_sb")
            # HAM warm-up: keep the PE busy during the input DMA wait so the
            # clock gate is already at 8/8 when real matmuls arrive.
            warm_ps = psum.tile([128, 128], F32, tag="bank7", name="warm_ps")
            for _w in range(24):
                nc.tensor.matmul(warm_ps, ident, ident, is_transpose=True,
                                 start=True, stop=True, skip_group_check=True)

            # helper: one conv1x1 block (256 outputs) for all samples.
            # Epilogue on DVE: out = relu?(psum + bias)
            def conv_block(w_tiles, rhs_per_s, relu, bias_col, banks, out_pool,
                           tag_fn, bufs=1, uid=""):
                outs = {s: [None, None] for s in range(BS)}
                pss = {}
                for mt in range(2):
                    for s in range(BS):
                        ps = psum.tile(
                            [128, T], F32,
                            tag=f"bank{banks[s * 2 + mt]}",
                            name=f"ps_{uid}_{s}_{mt}",
                        )
                        nk = len(w_tiles)
                        for c in range(nk):
                            _mm(nc, ps, w_tiles[c][:, mt * 128 : (mt + 1) * 128],
                                rhs_per_s[s][c], start=(c == 0), stop=(c == nk - 1))
                        pss[(s, mt)] = ps
                for mt in range(2):
                    for s in range(BS):
                        ot = out_pool.tile(
                            [128, T], dtm,
                            tag=tag_fn(s, mt), bufs=bufs,
                            name=f"{uid}_{s}_{mt}",
                        )
                        b_ap = bias_sb[:, bias_col + mt : bias_col + mt + 1]
                        if mt == 0:  # split epilogues across DVE and ACT
                            if relu:
                                nc.vector.tensor_scalar(ot, pss[(s, mt)], b_ap, 0.0,
                                                        op0=ALU.add, op1=ALU.max)
                            else:
                                nc.vector.tensor_scalar_add(ot, pss[(s, mt)], b_ap)
                        else:
                            nc.scalar.activation(
                                ot, pss[(s, mt)], AF.Relu if relu else AF.Identity,
                                bias=b_ap, scale=1.0)
                        outs[s][mt] = ot
                return outs

            xtag = lambda s, mt: f"x_{s}_{mt}"
            gatag = lambda s, mt: f"ga_{s}_{mt}"
            gbtag = lambda s, mt: f"gb_{s}_{mt}"

            # ---------- encoder head ----------
            h1 = conv_block(w_enc0_sb, xin, True, COL_ENC0, [0, 1, 4, 5], temp, gatag, uid="h1")
            h2 = conv_block(w_enc1_sb, h1, True, COL_ENC1, [2, 3, 6, 7], temp, gbtag, uid="h2")
            xs = conv_block(w_enc2_sb, h2, False, COL_ENC2, [0, 1, 4, 5], xpool, xtag,
                            bufs=4, uid="xe")

            # remaining const loads are deferred (emitted mid-encoder so the
            # first highway-layer weight DMAs aren't stuck behind them)
            misc = {}

            def load_misc_consts():
                wp = const.tile([128, WPACK_COLS], dtm, name="wpack_sb")
                nc.sync.dma_start(wp, d_wpack)
                misc["w_q"] = [wp[:, WQ_O : WQ_O + D], wp[:, WQ_O + D : WQ_O + 2 * D]]
                misc["w_o"] = [wp[:, WO_O : WO_O + D], wp[:, WO_O + D : WO_O + 2 * D]]
                misc["w_dec0"] = [wp[:, WD0_O + c * D : WD0_O + (c + 1) * D] for c in range(4)]
                misc["w_dec1"] = [wp[:, WD1_O : WD1_O + D], wp[:, WD1_O + D : WD1_O + 2 * D]]
                misc["w_dec2"] = [wp[:, WD2_O : WD2_O + D], wp[:, WD2_O + D : WD2_O + 2 * D]]
                misc["w_dec3"] = [wp[:, WD3_O : WD3_O + D], wp[:, WD3_O + D : WD3_O + 2 * D]]
                misc["w_last"] = [wp[:, WLAST_O : WLAST_O + F + 2],
                                  wp[:, WLAST_O + F + 2 : WLAST_O + 2 * (F + 2)]]
                misc["ones"] = wp[0:2, ONES_O : ONES_O + 128]
                misc["blast"] = wp[0:2, BLAST_O : BLAST_O + F + 2]
                kT_sb, v_sb = {}, {}
                for s in range(BS):
                    for c in range(2):
                        i = s * 2 + c
                        kT_sb[(s, c)] = wp[:, KT_O + i * TE : KT_O + (i + 1) * TE]
                        v_sb[(s, c)] = wp[:, V_O + i * D : V_O + (i + 1) * D]
                misc["kT"] = kT_sb
                misc["v"] = v_sb

            # ---------- highway stack ----------
            def make_x2(xs_cur, uid):
                # right-shifted copy (col0 = 0): makes the odd-dilation middle
                # conv tap even-aligned for the f32r matmul mode.
                out = {s: [None, None] for s in range(BS)}
                for c in range(2):
                    for s in range(BS):
                        x2 = xpool.tile([128, T], dtm, tag=f"x2_{s}_{c}", bufs=2,
                                        name=f"x2_{uid}_{s}_{c}")
                        nc.gpsimd.tensor_scalar_mul(x2[:, 0:1], xs_cur[s][c][:, 0:1], 0.0)
                        nc.gpsimd.tensor_copy(x2[:, 1:T], xs_cur[s][c][:, 0 : T - 1])
                        out[s][c] = x2
                return out

            def highway_layers(l_lo, l_hi, x2s):
                nonlocal xs
                for l in range(l_lo, l_hi):
                    dil = DIL[l]
                    wt = wstream.tile([128, 2, 3, 4, 128], dtm, tag="hww", bufs=2,
                                      name=f"hw_w_{l}")
                    nc.sync.dma_start(wt, d_hw_w[l])
                    last_enc = l == 9
                    ps_all = {}
                    for mt in range(4):
                        for s in range(BS):
                            ps_all[(mt, s)] = psum.tile(
                                [128, T], F32, tag=f"bank{mt * 2 + s}",
                                name=f"hwps_{l}_{mt}_{s}",
                            )
                    # kc-major: ALL kc0 taps (24 matmuls) before any kc1 tap,
                    # giving the previous layer's x'[1] epilogue a ~5us runway.
                    # mt order (2,0,3,1): chunk-0 gate+input banks finish first
                    # so its epilogue chain overlaps the remaining matmuls.
                    seen = {}
                    for kc in range(2):
                        for mt in (2, 0, 3, 1):
                            for k in (2, 1, 0):
                                first = kc == 0 and k == 2
                                last = kc == 1 and k == 0
                                for s in range(BS):
                                    ps = ps_all[(mt, s)]
                                    if k == 2:
                                        _mm(nc, ps, wt[:, kc, k, mt, :], xs[s][kc],
                                            start=first, stop=last)
                                    elif k == 1:
                                        if use_x2:
                                            off = dil - 1
                                            _mm(nc, ps[:, off:T], wt[:, kc, k, mt, :],
                                                x2s[s][kc][:, 0 : T - off],
                                                start=first, stop=last)
                                        else:
                                            _mm(nc, ps[:, dil:T], wt[:, kc, k, mt, :],
                                                xs[s][kc][:, 0 : T - dil],
                                                start=first, stop=last)
                                    else:
                                        sh = 2 * dil
                                        _mm(nc, ps[:, sh:T], wt[:, kc, k, mt, :],
                                            xs[s][kc][:, 0 : T - sh],
                                            start=first, stop=last)
                    # epilogue: x' = x + sigmoid(g) * (a - x)
                    # per chunk c: g = bank (c+2), a = bank c
                    newxs = {s: [None, None] for s in range(BS)}
                    for c in range(2):
                        tgs = {}
                        for s in range(BS):
                            tg = temp.tile([128, T], dtm, tag=f"tg_{s}_{c}", bufs=3,
                                           name=f"tg_{l}_{s}_{c}")
                            nc.scalar.activation(
                                tg, ps_all[(c + 2, s)], AF.Sigmoid,
                                bias=hwb_ap(l, c + 2), scale=1.0)
                            tgs[s] = tg
                        tmps = {}
                        for s in range(BS):
                            tmp = temp.tile([128, T], dtm, tag=f"tmp_{s}_{c}", bufs=3,
                                            name=f"tmp_{l}_{s}_{c}")
                            nc.vector.scalar_tensor_tensor(
                                tmp, ps_all[(c, s)], hwb_ap(l, c),
                                xs[s][c], op0=ALU.add, op1=ALU.subtract)
                            tmps[s] = tmp
                        for s in range(BS):
                            nc.gpsimd.tensor_mul(tmps[s], tgs[s], tmps[s])
                        for s in range(BS):
                            if last_enc:
                                xn = persist.tile([128, T], dtm, tag=f"q_{s}_{c}",
                                                  name=f"Q_{s}_{c}")
                            else:
                                xn = xpool.tile([128, T], dtm, tag=f"x_{s}_{c}", bufs=4,
                                                name=f"x_{l + 1}_{s}_{c}")
                            nc.vector.tensor_add(xn, tmps[s], xs[s][c])
                            newxs[s][c] = xn
                    xs = newxs
                    if use_x2 and l + 1 < l_hi:
                        x2s = make_x2(xs, f"l{l + 1}")
                    if l == 1:
                        load_misc_consts()

            # ---------- encoder highway ----------
            highway_layers(0, 10, make_x2(xs, "e0") if use_x2 else None)
            w_q_sb, w_o_sb = misc["w_q"], misc["w_o"]
            w_dec0_sb, w_dec1_sb = misc["w_dec0"], misc["w_dec1"]
            w_dec2_sb, w_dec3_sb = misc["w_dec2"], misc["w_dec3"]
            w_last_sb, ones_row, blast_sb = misc["w_last"], misc["ones"], misc["blast"]
            kT_sb, v_sb = misc["kT"], misc["v"]
            Qs = xs  # persisted encoder output (D, T) tiles

            # ---------- attention ----------
            Rqs = {s: [] for s in range(BS)}
            q_sb = {s: [] for s in range(BS)}
            for mt in range(2):
                for s in range(BS):
                    ps = psum.tile([128, T], F32, tag=f"bank{s * 4 + mt}",
                                   name=f"qps_{s}_{mt}")
                    for kc in range(2):
                        _mm(nc, ps, w_q_sb[kc][:, mt * 128 : (mt + 1) * 128], Qs[s][kc],
                            start=(kc == 0), stop=(kc == 1))
                    qt = temp.tile([128, T], dtm, tag=f"ga_{s}_{mt}", name=f"q_{s}_{mt}")
                    nc.vector.tensor_scalar_add(
                        qt, ps, bias_sb[:, COL_QB + mt : COL_QB + mt + 1])
                    q_sb[s].append(qt)

            # softmax without max-subtraction: scores are O(10), exp is safe in
            # fp32 and softmax is shift-invariant, so this matches the reference.
            # Stage-major across samples so the PE always has the other
            # sample's matmuls while one sample's softmax chain runs.
            at_tiles, psts, aT = {}, {}, {s: [] for s in range(BS)}
            for s in range(BS):
                at_tiles[s] = temp.tile([128, 4, TE], dt, tag=f"att_{s}",
                                        name=f"att_{s}")
                psts[s] = [psum.tile([128, T], F32, tag=f"bank{s * 4 + 2 + sc}",
                                     name=f"tps_{s}_{sc}") for sc in range(2)]
            for s in range(BS):
                for tt in range(4):
                    ps = psum.tile([128, TE], F32, tag=f"bank{s * 4 + (tt % 2)}",
                                   name=f"sps_{s}_{tt}")
                    for dc in range(2):
                        _mm(nc, ps, q_sb[s][dc][:, tt * 128 : (tt + 1) * 128],
                            kT_sb[(s, dc)], start=(dc == 0), stop=(dc == 1))
                    st = stat_sb[:, (s * 4 + tt) * 4 : (s * 4 + tt) * 4 + 4]
                    at = at_tiles[s][:, tt, :]
                    nc.scalar.activation(at, ps, AF.Exp, accum_out=st[:, 1:2])
                    nc.vector.reciprocal(st[:, 2:3], st[:, 1:2])
                    nc.vector.tensor_scalar_mul(at, at, st[:, 2:3])
            for s in range(BS):
                for tt in range(4):
                    at = at_tiles[s][:, tt, :]
                    for sc in range(2):
                        nc.tensor.matmul(
                            psts[s][sc][:, tt * 128 : (tt + 1) * 128],
                            at[:, sc * 128 : (sc + 1) * 128],
                            ident, is_transpose=True, start=True, stop=True,
                            skip_group_check=True)
                nc.sync.dma_start(
                    d_attn[s].rearrange("(tt p) e -> p tt e", p=128), at_tiles[s])
            for s in range(BS):
                for sc in range(2):
                    a2 = temp.tile([128, T], dtm, tag=f"gb_{s}_{sc}", name=f"aT_{s}_{sc}")
                    nc.vector.tensor_copy(a2, psts[s][sc])
                    aT[s].append(a2)

            ctx_sb = {s: [] for s in range(BS)}
            for s in range(BS):
                for dc in range(2):
                    ps = psum.tile([128, T], F32, tag=f"bank{s * 4 + dc}",
                                   name=f"cps_{s}_{dc}")
                    for sc in range(2):
                        _mm(nc, ps, v_sb[(s, sc)][:, dc * 128 : (dc + 1) * 128],
                            aT[s][sc], start=(sc == 0), stop=(sc == 1))
                    ct = temp.tile([128, T], dtm, tag=f"ga_{s}_{dc}", name=f"ctx_{s}_{dc}")
                    nc.vector.tensor_copy(ct, ps)
                    ctx_sb[s].append(ct)

            for mt in range(2):
                for s in range(BS):
                    ps = psum.tile([128, T], F32, tag=f"bank{s * 4 + mt}",
                                   name=f"ops_{s}_{mt}")
                    for dc in range(2):
                        _mm(nc, ps, w_o_sb[dc][:, mt * 128 : (mt + 1) * 128],
                            ctx_sb[s][dc], start=(dc == 0), stop=(dc == 1))
                    tmpo = temp.tile([128, T], dt, tag=f"gb_{s}_{mt}",
                                     name=f"tmpo_{s}_{mt}")
                    nc.vector.tensor_scalar_add(
                        tmpo, ps, bias_sb[:, COL_OB + mt : COL_OB + mt + 1])
                    rq = persist.tile([128, T], dtm, tag=f"rq_{s}_{mt}",
                                      name=f"rq_{s}_{mt}")
                    # Rq = sqrt(.5)*query + out_proj  [scales folded into w_o/b_o]
                    nc.vector.scalar_tensor_tensor(
                        rq, Qs[s][mt], SQ2, tmpo, op0=ALU.mult, op1=ALU.add)
                    Rqs[s].append(rq)

            # ---------- decoder ----------
            dec_in = {s: [Qs[s][0], Qs[s][1], Rqs[s][0], Rqs[s][1]] for s in range(BS)}
            w_dec0_r = [w_dec0_sb[2], w_dec0_sb[3], w_dec0_sb[0], w_dec0_sb[1]]
            xs = conv_block(w_dec0_r, dec_in, False, COL_DEC0, [2, 3, 6, 7],
                            xpool, xtag, bufs=4, uid="xd0")
            highway_layers(10, 16, make_x2(xs, "d0") if use_x2 else None)
            xs = conv_block(w_dec1_sb, xs, True, COL_DEC1, [0, 1, 4, 5], xpool, xtag,
                            bufs=4, uid="xd1")
            xs = conv_block(w_dec2_sb, xs, True, COL_DEC2, [2, 3, 6, 7], xpool, xtag,
                            bufs=4, uid="xd2")
            xs = conv_block(w_dec3_sb, xs, True, COL_DEC3, [0, 1, 4, 5], xpool, xtag,
                            bufs=4, uid="xd3")

            # ---------- final: mel (per-tt sigmoid conv) + done ((1,T) row) ----
            for s in range(BS):
                # done = sigmoid(fc . x) computed as a single-row matmul so the
                # output DMA is one contiguous 2KB write (not a 512-desc scatter)
                psd = psum.tile([1, T], F32, tag=f"bank{s * 4 + 3}", name=f"dps_{s}")
                for dc in range(2):
                    _mm(nc, psd, w_last_sb[dc][:, F : F + 1], xs[s][dc],
                        start=(dc == 0), stop=(dc == 1))
                dn = temp.tile([1, T], dt, tag=f"done_{s}", name=f"done_{s}")
                nc.scalar.activation(dn, psd, AF.Sigmoid, scale=1.0,
                                     bias=blast_sb[0:1, F : F + 1])
                nc.sync.dma_start(d_done[s].rearrange("t o -> o t"), dn)
            for s in range(BS):
                fo = temp.tile([128, 4, F + 2], dt, tag=f"fin_{s}", name=f"fin_{s}")
                for tt in range(4):
                    ps = psum.tile([128, F + 2], F32, tag=f"bank{s * 4 + tt}",
                                   name=f"fps_{s}_{tt}")
                    for dc in range(2):
                        _mm(nc, ps, xs[s][dc][:, tt * 128 : (tt + 1) * 128],
                            w_last_sb[dc], start=(dc == 0), stop=False)
                    _mm(nc, ps, ones_row, blast_sb, start=False, stop=True)
                    nc.scalar.activation(fo[:, tt, :], ps, AF.Sigmoid, scale=1.0)
                    nc.sync.dma_start(d_mel[s, tt * 128 : (tt + 1) * 128, :],
                                      fo[:, tt, 0:F])

    nc.compile()
    return nc


def _prep_host(inputs):
    """Host-side packing: transposes, chunking, and packed const blocks."""
    f32 = np.float32
    mm_np = np.float16 if MM_DT == "f16" else np.float32

    def npm(a):
        return np.ascontiguousarray(np.asarray(a, dtype=f32)).astype(mm_np)

    # x0: (B, T, F) -> pad F to 512 -> (B, 128, 4, T)
    x_t = np.zeros((B, 512, T), f32)
    x_t[:, :F, :] = np.asarray(inputs["inputs"], f32).transpose(0, 2, 1)
    x0 = npm(x_t.reshape(B, 4, 128, T).transpose(0, 2, 1, 3))

    keysT = np.asarray(inputs["keys"], f32).transpose(0, 2, 1)  # (B, D, TE)
    values = np.asarray(inputs["values"], f32)  # (B, TE, D)

    w_all = np.concatenate([np.asarray(inputs["enc_hw_w"]),
                            np.asarray(inputs["dec_hw_w"])], axis=0)  # (16, 512, 256, 3)
    wt = w_all.transpose(0, 2, 1, 3)            # (L, ci, co, k)
    wt = wt.reshape(L, 2, 128, 4, 128, 3)       # (L, kc, p, mt, f, k)
    hw_w = npm(wt.transpose(0, 2, 1, 5, 3, 4))  # (L, 128, kc, k, mt, f)

    def t2(w):  # (O, I, 1) -> (I, O) fp32
        return np.asarray(w, f32)[:, :, 0].T

    # wenc0p: (400, 256) -> pad rows to 512 -> (128, 4, 256)
    we0 = np.zeros((512, D), f32)
    we0[:F] = t2(inputs["enc_w0"])
    wenc0p = npm(we0.reshape(4, 128, D).transpose(1, 0, 2))

    def chunks(w):  # (rows, cols) -> list of (128, cols)
        return [w[c * 128 : (c + 1) * 128] for c in range(w.shape[0] // 128)]

    pack1 = npm(np.concatenate(
        chunks(t2(inputs["enc_w1"])) + chunks(t2(inputs["enc_w2"])), axis=1))

    # wpack: [wq | wo | wdec0 | wdec1 | wdec2 | wdec3 | wlast | kT | v | ones | blast]
    w_q = np.asarray(inputs["attn_q_w"], f32).T
    w_o = np.asarray(inputs["attn_o_w"], f32).T * (math.sqrt(TE) * SQ2)
    w_last = np.concatenate(
        [np.asarray(inputs["last_w"], f32)[:, :, 0].T,
         np.asarray(inputs["fc_w"], f32).T,
         np.zeros((D, 1), f32)], axis=1)  # (256, 402)
    blocks = (chunks(w_q) + chunks(w_o) + chunks(t2(inputs["dec_w0"]))
              + chunks(t2(inputs["dec_w1"])) + chunks(t2(inputs["dec_w2"]))
              + chunks(t2(inputs["dec_w3"])) + chunks(w_last))
    # keys/values are per-core; build the shared prefix once
    prefix = np.concatenate(blocks, axis=1)  # (128, 4388)
    ones_blk = np.zeros((128, 128), f32)
    ones_blk[0, :] = 1.0
    blast_blk = np.zeros((128, F + 2), f32)
    blast_blk[0, :F] = np.asarray(inputs["last_b"], f32)
    blast_blk[0, F] = np.asarray(inputs["fc_b"], f32)[0]

    b_all = np.concatenate([np.asarray(inputs["enc_hw_b"]),
                            np.asarray(inputs["dec_hw_b"])], axis=0)  # (16, 512)
    hw_b = np.asarray(b_all, f32).reshape(L, 4, 128).transpose(2, 0, 1).reshape(128, L * 4)

    def cols(v):  # (256,) -> (128, 2)
        return np.asarray(v, dtype=f32).reshape(2, 128).T

    bias_tbl = np.zeros((128, NB), dtype=f32)
    bias_tbl[:, COL_ENC0:COL_ENC0 + 2] = cols(inputs["enc_b0"])
    bias_tbl[:, COL_ENC1:COL_ENC1 + 2] = cols(inputs["enc_b1"])
    bias_tbl[:, COL_ENC2:COL_ENC2 + 2] = cols(inputs["enc_b2"])
    bias_tbl[:, COL_QB:COL_QB + 2] = cols(inputs["attn_q_b"])
    bias_tbl[:, COL_OB:COL_OB + 2] = cols(np.asarray(inputs["attn_o_b"], f32) * SQ2)
    bias_tbl[:, COL_DEC0:COL_DEC0 + 2] = cols(inputs["dec_b0"])
    bias_tbl[:, COL_DEC1:COL_DEC1 + 2] = cols(inputs["dec_b1"])
    bias_tbl[:, COL_DEC2:COL_DEC2 + 2] = cols(inputs["dec_b2"])
    bias_tbl[:, COL_DEC3:COL_DEC3 + 2] = cols(inputs["dec_b3"])
    bias2 = np.ascontiguousarray(np.concatenate([bias_tbl, hw_b], axis=1))

    shared = dict(hw_w=hw_w, bias2=bias2, wenc0p=wenc0p, pack1=pack1)

    in_maps = []
    for i in range(N_CORES):
        sl = slice(i * BS, (i + 1) * BS)
        kv_blocks = []
        for s in range(BS):
            for c in range(2):
                kv_blocks.append(keysT[i * BS + s, c * 128 : (c + 1) * 128, :])
        for s in range(BS):
            for c in range(2):
                kv_blocks.append(values[i * BS + s, c * 128 : (c + 1) * 128, :])
        wpack = npm(np.concatenate(
            [prefix] + kv_blocks[:4] + kv_blocks[4:] + [ones_blk, blast_blk], axis=1))
        m = dict(shared)
        m["x0"] = np.ascontiguousarray(x0[sl])
        m["wpack"] = wpack
        in_maps.append(m)
    return in_maps


def kernel(**inputs):
    global LAST_EXEC_NS
    if "nc" not in _BUILD_CACHE:
        _BUILD_CACHE["nc"] = _build()
    nc = _BUILD_CACHE["nc"]

    in_maps = _prep_host(inputs)

    trace = os.environ.get("KBENCH_TRACE", "0") == "1"
    if trace:
        _install_ntff_hook()
    res = run_bass_kernel_spmd(nc, in_maps, core_ids=list(range(N_CORES)), trace=trace)
    LAST_EXEC_NS = res.exec_time_ns

    mel = np.concatenate([r["mel"] for r in res.results], axis=0)
    attn = np.concatenate([r["attn"] for r in res.results], axis=0)
    done = np.concatenate([r["done"] for r in res.results], axis=0)
    return mel, attn, done


def _install_ntff_hook():
    """Register the axon NTFF profiling hook (missing from this image's antenv)."""
    import types

    if "antenv.axon_hooks" in sys.modules:
        return
    m = types.ModuleType("antenv.axon_hooks")
    m._h = None
    m.set_axon_ntff_profile_hook = lambda h: setattr(m, "_h", h)
    m.get_axon_ntff_profile_hook = lambda: m._h
    sys.modules["antenv.axon_hooks"] = m
    try:
        import antenv

        antenv.axon_hooks = m
        from trn_agent_boot.trn_boot import _ntff_profile_via_ctypes

        m._h = _ntff_profile_via_ctypes("/opt/axon/libaxon_pjrt.so")
    except Exception:
        m._h = None


# revision 26
# speedup vs baseline: 1.5948x; 1.0978x over previous
"""Trainium2 Bass kernel for nn_Decoder_42417097016016 (DCTTS-style decoder).

Sharding: pure data parallel over batch. B=16 samples -> 8 NeuronCores x 2
samples each; all weights replicated per core.

Layout: activations live on-chip as (channels, time) so every conv1x1 /
causal conv is a PE matmul with channels on partitions.  Causal convs with
dilation d are 3 accumulating matmuls per output tile with column shifts
(0, d, 2d) - left zero-padding falls out of the shifted PSUM accumulation.

The attention block computes scores (t,s), softmax along free dim (ACT Exp
with accum_out row sums), writes the normalized attn output directly, and
PE-transposes it to (s,t) for the context matmul.  mel and done share one
final matmul by concatenating fc_w as a 401st output column (padded to 402).

Matmul precision knob KBENCH_MM: "f16" (default, full PE rate), "f32r"
(reduced-precision fp32, half rate, needs even matmul geometry -> shifted x2
copies for odd-dilation taps), "f32" (exact, quarter rate).
"""

import math
import os
import sys

import numpy as np

for _p in ("/opt/trn_rl_repo", "/root/.axon_site/_ro/trn_rl_repo"):
    if os.path.isdir(_p) and _p not in sys.path:
        sys.path.append(_p)

import concourse.bass as bass
import concourse.tile as tile
from concourse import bacc, mybir
from concourse.bass_utils import run_bass_kernel_spmd

AF = mybir.ActivationFunctionType
ALU = mybir.AluOpType
AX = mybir.AxisListType
F32 = mybir.dt.float32
F32R = mybir.dt.float32r
F16 = mybir.dt.float16

N_CORES = 8
B, T, TE, D, F = 16, 512, 256, 256, 400
BS = B // N_CORES  # samples per core
DIL = [1, 3, 9, 27, 1, 3, 9, 27, 3, 3] + [1, 3, 9, 27, 1, 1]
L = len(DIL)  # 16 highway layers (10 encoder + 6 decoder)
SQ2 = math.sqrt(0.5)

MM_DT = os.environ.get("KBENCH_MM", "f16")

# bias table column assignments
COL_ENC0, COL_ENC1, COL_ENC2 = 0, 2, 4
COL_QB, COL_OB = 6, 8
COL_DEC0, COL_DEC1, COL_DEC2, COL_DEC3 = 10, 12, 14, 16
NB = 18

LAST_EXEC_NS = None
_BUILD_CACHE = {}


def _mm(nc, out, lhsT, rhs, **kw):
    nc.tensor.matmul(out, lhsT, rhs, **kw)


def _build():
    """Build the per-core Bass program (same program on all 8 cores)."""
    from concourse.masks import make_identity

    nc = bacc.Bacc("TRN2", target_bir_lowering=False, debug=False)
    dt = F32
    dtm = {"f16": F16, "f32r": F32R, "f32": F32}[MM_DT]
    use_x2 = MM_DT == "f32r"  # f32r matmuls need even offsets/N

    # ---- DRAM I/O (per-core shard shapes) ----
    # x0: (s, p, c, t) pre-chunked+padded on host -> one DMA per sample
    d_x0 = nc.dram_tensor("x0", [BS, 128, 4, T], dtm, kind="ExternalInput").ap()
    d_hw_w = nc.dram_tensor("hw_w", [L, 128, 2, 3, 4, 128], dtm, kind="ExternalInput").ap()
    # wenc0p: enc_w0^T padded 400->512 rows, chunked (128, 4, 256)
    d_wenc0 = nc.dram_tensor("wenc0p", [128, 4, D], dtm, kind="ExternalInput").ap()
    # pack1: [enc1 c0|c1, enc2 c0|c1] as (128, 4*256)
    d_pack1 = nc.dram_tensor("pack1", [128, 4 * D], dtm, kind="ExternalInput").ap()
    # bias2: [bias_tbl (18) | hw_b (L*4)] fp32
    d_bias2 = nc.dram_tensor("bias2", [128, NB + L * 4], dt, kind="ExternalInput").ap()
    # wpack: all attention/decoder weights + keys/values + ones/blast rows
    WQ_O, WO_O = 0, 512
    WD0_O, WD1_O, WD2_O, WD3_O = 1024, 2048, 2560, 3072
    WLAST_O = 3584
    KT_O = WLAST_O + 2 * (F + 2)          # 4388
    V_O = KT_O + 4 * TE                   # 5412
    ONES_O = V_O + 4 * D                  # 6436
    BLAST_O = ONES_O + 128                # 6564
    WPACK_COLS = BLAST_O + (F + 2)        # 6966
    d_wpack = nc.dram_tensor("wpack", [128, WPACK_COLS], dtm, kind="ExternalInput").ap()

    d_mel = nc.dram_tensor("mel", [BS, T, F], dt, kind="ExternalOutput").ap()
    d_attn = nc.dram_tensor("attn", [BS, T, TE], dt, kind="ExternalOutput").ap()
    d_done = nc.dram_tensor("done", [BS, T, 1], dt, kind="ExternalOutput").ap()

    with tile.TileContext(nc) as tc:
        with (
            tc.tile_pool(name="const", bufs=1) as const,
            tc.tile_pool(name="xpool", bufs=1) as xpool,
            tc.tile_pool(name="persist", bufs=1) as persist,
            tc.tile_pool(name="wstream", bufs=1) as wstream,
            tc.tile_pool(name="temp", bufs=1) as temp,
            tc.tile_pool(name="psum", bufs=1, space="PSUM") as psum,
        ):
            # ---------- startup-critical loads first (DMA queue is FIFO) ----
            wenc0_sb = const.tile([128, 4, D], dtm, name="wenc0_sb")
            nc.sync.dma_start(wenc0_sb, d_wenc0)
            w_enc0_sb = [wenc0_sb[:, c, :] for c in range(4)]
            xin = {}
            for s in range(BS):
                xt = temp.tile([128, 4, T], dtm, tag=f"xin_{s}", name=f"xin_{s}")
                nc.sync.dma_start(xt, d_x0[s])
                xin[s] = [xt[:, c, :] for c in range(4)]
            pack1_sb = const.tile([128, 4 * D], dtm, name="pack1_sb")
            nc.sync.dma_start(pack1_sb, d_pack1)
            w_enc1_sb = [pack1_sb[:, 0:D], pack1_sb[:, D : 2 * D]]
            w_enc2_sb = [pack1_sb[:, 2 * D : 3 * D], pack1_sb[:, 3 * D : 4 * D]]
            bias2_sb = const.tile([128, NB + L * 4], dt, name="bias2_sb")
            nc.sync.dma_start(bias2_sb, d_bias2)
            bias_sb = bias2_sb[:, 0:NB]

            def hwb_ap(l, j):
                c = NB + l * 4 + j
                return bias2_sb[:, c : c + 1]

            ident = const.tile([128, 128], dt, name="ident")
            make_identity(nc, ident)
            stat_sb = const.tile([128, 32], F32, name="stat_sb")
            # HAM warm-up: keep the PE busy during the input DMA wait so the
            # clock gate is already at 8/8 when real matmuls arrive.
            warm_ps = psum.tile([128, 128], F32, tag="bank7", name="warm_ps")
            for _w in range(24):
                nc.tensor.matmul(warm_ps, ident, ident, is_transpose=True,
                                 start=True, stop=True, skip_group_check=True)

            # helper: one conv1x1 block (256 outputs) for all samples.
            # Epilogue on DVE: out = relu?(psum + bias)
            def conv_block(w_tiles, rhs_per_s, relu, bias_col, banks, out_pool,
                           tag_fn, bufs=1, uid=""):
                outs = {s: [None, None] for s in range(BS)}
                pss = {}
                for mt in range(2):
                    for s in range(BS):
                        ps = psum.tile(
                            [128, T], F32,
                            tag=f"bank{banks[s * 2 + mt]}",
                            name=f"ps_{uid}_{s}_{mt}",
                        )
                        nk = len(w_tiles)
                        for c in range(nk):
                            _mm(nc, ps, w_tiles[c][:, mt * 128 : (mt + 1) * 128],
                                rhs_per_s[s][c], start=(c == 0), stop=(c == nk - 1))
                        pss[(s, mt)] = ps
                for mt in range(2):
                    for s in range(BS):
                        ot = out_pool.tile(
                            [128, T], dtm,
                            tag=tag_fn(s, mt), bufs=bufs,
                            name=f"{uid}_{s}_{mt}",
                        )
                        b_ap = bias_sb[:, bias_col + mt : bias_col + mt + 1]
                        if mt == 0:  # split epilogues across DVE and ACT
                            if relu:
                                nc.vector.tensor_scalar(ot, pss[(s, mt)], b_ap, 0.0,
                                                        op0=ALU.add, op1=ALU.max)
                            else:
                                nc.vector.tensor_scalar_add(ot, pss[(s, mt)], b_ap)
                        else:
                            nc.scalar.activation(
                                ot, pss[(s, mt)], AF.Relu if relu else AF.Identity,
                                bias=b_ap, scale=1.0)
                        outs[s][mt] = ot
                return outs

            xtag = lambda s, mt: f"x_{s}_{mt}"
            gatag = lambda s, mt: f"ga_{s}_{mt}"
            gbtag = lambda s, mt: f"gb_{s}_{mt}"

            # ---------- encoder head ----------
            h1 = conv_block(w_enc0_sb, xin, True, COL_ENC0, [0, 1, 4, 5], temp, gatag, uid="h1")
            h2 = conv_block(w_enc1_sb, h1, True, COL_ENC1, [2, 3, 6, 7], temp, gbtag, uid="h2")
            xs = conv_block(w_enc2_sb, h2, False, COL_ENC2, [0, 1, 4, 5], xpool, xtag,
                            bufs=4, uid="xe")

            # remaining const loads are deferred (emitted mid-encoder so the
            # first highway-layer weight DMAs aren't stuck behind them)
            misc = {}

            def load_misc_consts():
                wp = const.tile([128, WPACK_COLS], dtm, name="wpack_sb")
                nc.sync.dma_start(wp, d_wpack)
                misc["w_q"] = [wp[:, WQ_O : WQ_O + D], wp[:, WQ_O + D : WQ_O + 2 * D]]
                misc["w_o"] = [wp[:, WO_O : WO_O + D], wp[:, WO_O + D : WO_O + 2 * D]]
                misc["w_dec0"] = [wp[:, WD0_O + c * D : WD0_O + (c + 1) * D] for c in range(4)]
                misc["w_dec1"] = [wp[:, WD1_O : WD1_O + D], wp[:, WD1_O + D : WD1_O + 2 * D]]
                misc["w_dec2"] = [wp[:, WD2_O : WD2_O + D], wp[:, WD2_O + D : WD2_O + 2 * D]]
                misc["w_dec3"] = [wp[:, WD3_O : WD3_O + D], wp[:, WD3_O + D : WD3_O + 2 * D]]
                misc["w_last"] = [wp[:, WLAST_O : WLAST_O + F + 2],
                                  wp[:, WLAST_O + F + 2 : WLAST_O + 2 * (F + 2)]]
                misc["ones"] = wp[0:2, ONES_O : ONES_O + 128]
                misc["blast"] = wp[0:2, BLAST_O : BLAST_O + F + 2]
                kT_sb, v_sb = {}, {}
                for s in range(BS):
                    for c in range(2):
                        i = s * 2 + c
                        kT_sb[(s, c)] = wp[:, KT_O + i * TE : KT_O + (i + 1) * TE]
                        v_sb[(s, c)] = wp[:, V_O + i * D : V_O + (i + 1) * D]
                misc["kT"] = kT_sb
                misc["v"] = v_sb

            # ---------- highway stack ----------
            def make_x2(xs_cur, uid):
                # right-shifted copy (col0 = 0): makes the odd-dilation middle
                # conv tap even-aligned for the f32r matmul mode.
                out = {s: [None, None] for s in range(BS)}
                for c in range(2):
                    for s in range(BS):
                        x2 = xpool.tile([128, T], dtm, tag=f"x2_{s}_{c}", bufs=2,
                                        name=f"x2_{uid}_{s}_{c}")
                        nc.gpsimd.tensor_scalar_mul(x2[:, 0:1], xs_cur[s][c][:, 0:1], 0.0)
                        nc.gpsimd.tensor_copy(x2[:, 1:T], xs_cur[s][c][:, 0 : T - 1])
                        out[s][c] = x2
                return out

            def highway_layers(l_lo, l_hi, x2s):
                nonlocal xs
                for l in range(l_lo, l_hi):
                    dil = DIL[l]
                    wt = wstream.tile([128, 2, 3, 4, 128], dtm, tag="hww", bufs=3,
                                      name=f"hw_w_{l}")
                    nc.sync.dma_start(wt, d_hw_w[l])
                    last_enc = l == 9
                    ps_all = {}
                    for mt in range(4):
                        for s in range(BS):
                            ps_all[(mt, s)] = psum.tile(
                                [128, T], F32, tag=f"bank{mt * 2 + s}",
                                name=f"hwps_{l}_{mt}_{s}",
                            )
                    # kc-major: ALL kc0 taps (24 matmuls) before any kc1 tap,
                    # giving the previous layer's x'[1] epilogue a ~5us runway.
                    # mt order (2,0,3,1): chunk-0 gate+input banks finish first
                    # so its epilogue chain overlaps the remaining matmuls.
                    seen = {}
                    for kc in range(2):
                        for mt in (2, 0, 3, 1):
                            for k in (2, 1, 0):
                                first = kc == 0 and k == 2
                                last = kc == 1 and k == 0
                                for s in range(BS):
                                    ps = ps_all[(mt, s)]
                                    if k == 2:
                                        _mm(nc, ps, wt[:, kc, k, mt, :], xs[s][kc],
                                            start=first, stop=last)
                                    elif k == 1:
                                        if use_x2:
                                            off = dil - 1
                                            _mm(nc, ps[:, off:T], wt[:, kc, k, mt, :],
                                                x2s[s][kc][:, 0 : T - off],
                                                start=first, stop=last)
                                        else:
                                            _mm(nc, ps[:, dil:T], wt[:, kc, k, mt, :],
                                                xs[s][kc][:, 0 : T - dil],
                                                start=first, stop=last)
                                    else:
                                        sh = 2 * dil
                                        _mm(nc, ps[:, sh:T], wt[:, kc, k, mt, :],
                                            xs[s][kc][:, 0 : T - sh],
                                            start=first, stop=last)
                    # epilogue: x' = x + sigmoid(g) * (a - x)
                    # per chunk c: g = bank (c+2), a = bank c
                    newxs = {s: [None, None] for s in range(BS)}
                    for c in range(2):
                        tgs = {}
                        for s in range(BS):
                            tg = temp.tile([128, T], dtm, tag=f"tg_{s}_{c}", bufs=3,
                                           name=f"tg_{l}_{s}_{c}")
                            nc.scalar.activation(
                                tg, ps_all[(c + 2, s)], AF.Sigmoid,
                                bias=hwb_ap(l, c + 2), scale=1.0)
                            tgs[s] = tg
                        tmps = {}
                        for s in range(BS):
                            tmp = temp.tile([128, T], dtm, tag=f"tmp_{s}_{c}", bufs=3,
                                            name=f"tmp_{l}_{s}_{c}")
                            nc.vector.scalar_tensor_tensor(
                                tmp, ps_all[(c, s)], hwb_ap(l, c),
                                xs[s][c], op0=ALU.add, op1=ALU.subtract)
                            tmps[s] = tmp
                        for s in range(BS):
                            nc.vector.tensor_mul(tmps[s], tgs[s], tmps[s])
                        for s in range(BS):
                            if last_enc:
                                xn = persist.tile([128, T], dtm, tag=f"q_{s}_{c}",
                                                  name=f"Q_{s}_{c}")
                            else:
                                xn = xpool.tile([128, T], dtm, tag=f"x_{s}_{c}", bufs=4,
                                                name=f"x_{l + 1}_{s}_{c}")
                            nc.vector.tensor_add(xn, tmps[s], xs[s][c])
                            newxs[s][c] = xn
                    xs = newxs
                    if use_x2 and l + 1 < l_hi:
                        x2s = make_x2(xs, f"l{l + 1}")
                    if l == 1:
                        load_misc_consts()

            # ---------- encoder highway ----------
            highway_layers(0, 10, make_x2(xs, "e0") if use_x2 else None)
            w_q_sb, w_o_sb = misc["w_q"], misc["w_o"]
            w_dec0_sb, w_dec1_sb = misc["w_dec0"], misc["w_dec1"]
            w_dec2_sb, w_dec3_sb = misc["w_dec2"], misc["w_dec3"]
            w_last_sb, ones_row, blast_sb = misc["w_last"], misc["ones"], misc["blast"]
            kT_sb, v_sb = misc["kT"], misc["v"]
            Qs = xs  # persisted encoder output (D, T) tiles

            # ---------- attention ----------
            Rqs = {s: [] for s in range(BS)}
            q_sb = {s: [] for s in range(BS)}
            for mt in range(2):
                for s in range(BS):
                    ps = psum.tile([128, T], F32, tag=f"bank{s * 4 + mt}",
                                   name=f"qps_{s}_{mt}")
                    for kc in range(2):
                        _mm(nc, ps, w_q_sb[kc][:, mt * 128 : (mt + 1) * 128], Qs[s][kc],
                            start=(kc == 0), stop=(kc == 1))
                    qt = temp.tile([128, T], dtm, tag=f"ga_{s}_{mt}", name=f"q_{s}_{mt}")
                    nc.vector.tensor_scalar_add(
                        qt, ps, bias_sb[:, COL_QB + mt : COL_QB + mt + 1])
                    q_sb[s].append(qt)

            # softmax without max-subtraction: scores are O(10), exp is safe in
            # fp32 and softmax is shift-invariant, so this matches the reference.
            # Stage-major across samples so the PE always has the other
            # sample's matmuls while one sample's softmax chain runs.
            at_tiles, psts, aT = {}, {}, {s: [] for s in range(BS)}
            for s in range(BS):
                at_tiles[s] = temp.tile([128, 4, TE], dt, tag=f"att_{s}",
                                        name=f"att_{s}")
                psts[s] = [psum.tile([128, T], F32, tag=f"bank{s * 4 + 2 + sc}",
                                     name=f"tps_{s}_{sc}") for sc in range(2)]
            for s in range(BS):
                for tt in range(4):
                    ps = psum.tile([128, TE], F32, tag=f"bank{s * 4 + (tt % 2)}",
                                   name=f"sps_{s}_{tt}")
                    for dc in range(2):
                        _mm(nc, ps, q_sb[s][dc][:, tt * 128 : (tt + 1) * 128],
                            kT_sb[(s, dc)], start=(dc == 0), stop=(dc == 1))
                    st = stat_sb[:, (s * 4 + tt) * 4 : (s * 4 + tt) * 4 + 4]
                    at = at_tiles[s][:, tt, :]
                    nc.scalar.activation(at, ps, AF.Exp, accum_out=st[:, 1:2])
                    nc.vector.reciprocal(st[:, 2:3], st[:, 1:2])
                    nc.vector.tensor_scalar_mul(at, at, st[:, 2:3])
            for s in range(BS):
                for tt in range(4):
                    at = at_tiles[s][:, tt, :]
                    for sc in range(2):
                        nc.tensor.matmul(
                            psts[s][sc][:, tt * 128 : (tt + 1) * 128],
                            at[:, sc * 128 : (sc + 1) * 128],
                            ident, is_transpose=True, start=True, stop=True,
                            skip_group_check=True)
                nc.sync.dma_start(
                    d_attn[s].rearrange("(tt p) e -> p tt e", p=128), at_tiles[s])
            for s in range(BS):
                for sc in range(2):
                    a2 = temp.tile([128, T], dtm, tag=f"gb_{s}_{sc}", name=f"aT_{s}_{sc}")
                    nc.vector.tensor_copy(a2, psts[s][sc])
                    aT[s].append(a2)

            ctx_sb = {s: [] for s in range(BS)}
            for s in range(BS):
                for dc in range(2):
                    ps = psum.tile([128, T], F32, tag=f"bank{s * 4 + dc}",
                                   name=f"cps_{s}_{dc}")
                    for sc in range(2):
                        _mm(nc, ps, v_sb[(s, sc)][:, dc * 128 : (dc + 1) * 128],
                            aT[s][sc], start=(sc == 0), stop=(sc == 1))
                    ct = temp.tile([128, T], dtm, tag=f"ga_{s}_{dc}", name=f"ctx_{s}_{dc}")
                    nc.vector.tensor_copy(ct, ps)
                    ctx_sb[s].append(ct)

            for mt in range(2):
                for s in range(BS):
                    ps = psum.tile([128, T], F32, tag=f"bank{s * 4 + mt}",
                                   name=f"ops_{s}_{mt}")
                    for dc in range(2):
                        _mm(nc, ps, w_o_sb[dc][:, mt * 128 : (mt + 1) * 128],
                            ctx_sb[s][dc], start=(dc == 0), stop=(dc == 1))
                    tmpo = temp.tile([128, T], dt, tag=f"gb_{s}_{mt}",
                                     name=f"tmpo_{s}_{mt}")
                    nc.vector.tensor_scalar_add(
                        tmpo, ps, bias_sb[:, COL_OB + mt : COL_OB + mt + 1])
                    rq = persist.tile([128, T], dtm, tag=f"rq_{s}_{mt}",
                                      name=f"rq_{s}_{mt}")
                    # Rq = sqrt(.5)*query + out_proj  [scales folded into w_o/b_o]
                    nc.vector.scalar_tensor_tensor(
                        rq, Qs[s][mt], SQ2, tmpo, op0=ALU.mult, op1=ALU.add)
                    Rqs[s].append(rq)

            # ---------- decoder ----------
            dec_in = {s: [Qs[s][0], Qs[s][1], Rqs[s][0], Rqs[s][1]] for s in range(BS)}
            w_dec0_r = [w_dec0_sb[2], w_dec0_sb[3], w_dec0_sb[0], w_dec0_sb[1]]
            xs = conv_block(w_dec0_r, dec_in, False, COL_DEC0, [2, 3, 6, 7],
                            xpool, xtag, bufs=4, uid="xd0")
            highway_layers(10, 16, make_x2(xs, "d0") if use_x2 else None)
            xs = conv_block(w_dec1_sb, xs, True, COL_DEC1, [0, 1, 4, 5], xpool, xtag,
                            bufs=4, uid="xd1")
            xs = conv_block(w_dec2_sb, xs, True, COL_DEC2, [2, 3, 6, 7], xpool, xtag,
                            bufs=4, uid="xd2")
            xs = conv_block(w_dec3_sb, xs, True, COL_DEC3, [0, 1, 4, 5], xpool, xtag,
                            bufs=4, uid="xd3")

            # ---------- final: mel (per-tt sigmoid conv) + done ((1,T) row) ----
            for s in range(BS):
                # done = sigmoid(fc . x) computed as a single-row matmul so the
                # output DMA is one contiguous 2KB write (not a 512-desc scatter)
                psd = psum.tile([1, T], F32, tag=f"bank{s * 4 + 3}", name=f"dps_{s}")
                for dc in range(2):
                    _mm(nc, psd, w_last_sb[dc][:, F : F + 1], xs[s][dc],
                        start=(dc == 0), stop=(dc == 1))
                dn = temp.tile([1, T], dt, tag=f"done_{s}", name=f"done_{s}")
                nc.scalar.activation(dn, psd, AF.Sigmoid, scale=1.0,
                                     bias=blast_sb[0:1, F : F + 1])
                nc.sync.dma_start(d_done[s].rearrange("t o -> o t"), dn)
            for s in range(BS):
                fo = temp.tile([128, 4, F + 2], dt, tag=f"fin_{s}", name=f"fin_{s}")
                for tt in range(4):
                    ps = psum.tile([128, F + 2], F32, tag=f"bank{s * 4 + tt}",
                                   name=f"fps_{s}_{tt}")
                    for dc in range(2):
                        _mm(nc, ps, xs[s][dc][:, tt * 128 : (tt + 1) * 128],
                            w_last_sb[dc], start=(dc == 0), stop=False)
                    _mm(nc, ps, ones_row, blast_sb, start=False, stop=True)
                    nc.scalar.activation(fo[:, tt, :], ps, AF.Sigmoid, scale=1.0)
                    nc.sync.dma_start(d_mel[s, tt * 128 : (tt + 1) * 128, :],
                                      fo[:, tt, 0:F])

    nc.compile()
    return nc


def _prep_host(inputs):
    """Host-side packing: transposes, chunking, and packed const blocks."""
    f32 = np.float32
    mm_np = np.float16 if MM_DT == "f16" else np.float32

    def npm(a):
        return np.ascontiguousarray(np.asarray(a, dtype=f32)).astype(mm_np)

    # x0: (B, T, F) -> pad F to 512 -> (B, 128, 4, T)
    x_t = np.zeros((B, 512, T), f32)
    x_t[:, :F, :] = np.asarray(inputs["inputs"], f32).transpose(0, 2, 1)
    x0 = npm(x_t.reshape(B, 4, 128, T).transpose(0, 2, 1, 3))

    keysT = np.asarray(inputs["keys"], f32).transpose(0, 2, 1)  # (B, D, TE)
    values = np.asarray(inputs["values"], f32)  # (B, TE, D)

    w_all = np.concatenate([np.asarray(inputs["enc_hw_w"]),
                            np.asarray(inputs["dec_hw_w"])], axis=0)  # (16, 512, 256, 3)
    wt = w_all.transpose(0, 2, 1, 3)            # (L, ci, co, k)
    wt = wt.reshape(L, 2, 128, 4, 128, 3)       # (L, kc, p, mt, f, k)
    hw_w = npm(wt.transpose(0, 2, 1, 5, 3, 4))  # (L, 128, kc, k, mt, f)

    def t2(w):  # (O, I, 1) -> (I, O) fp32
        return np.asarray(w, f32)[:, :, 0].T

    # wenc0p: (400, 256) -> pad rows to 512 -> (128, 4, 256)
    we0 = np.zeros((512, D), f32)
    we0[:F] = t2(inputs["enc_w0"])
    wenc0p = npm(we0.reshape(4, 128, D).transpose(1, 0, 2))

    def chunks(w):  # (rows, cols) -> list of (128, cols)
        return [w[c * 128 : (c + 1) * 128] for c in range(w.shape[0] // 128)]

    pack1 = npm(np.concatenate(
        chunks(t2(inputs["enc_w1"])) + chunks(t2(inputs["enc_w2"])), axis=1))

    # wpack: [wq | wo | wdec0 | wdec1 | wdec2 | wdec3 | wlast | kT | v | ones | blast]
    w_q = np.asarray(inputs["attn_q_w"], f32).T
    w_o = np.asarray(inputs["attn_o_w"], f32).T * (math.sqrt(TE) * SQ2)
    w_last = np.concatenate(
        [np.asarray(inputs["last_w"], f32)[:, :, 0].T,
         np.asarray(inputs["fc_w"], f32).T,
         np.zeros((D, 1), f32)], axis=1)  # (256, 402)
    blocks = (chunks(w_q) + chunks(w_o) + chunks(t2(inputs["dec_w0"]))
              + chunks(t2(inputs["dec_w1"])) + chunks(t2(inputs["dec_w2"]))
              + chunks(t2(inputs["dec_w3"])) + chunks(w_last))
    # keys/values are per-core; build the shared prefix once
    prefix = np.concatenate(blocks, axis=1)  # (128, 4388)
    ones_blk = np.zeros((128, 128), f32)
    ones_blk[0, :] = 1.0
    blast_blk = np.zeros((128, F + 2), f32)
    blast_blk[0, :F] = np.asarray(inputs["last_b"], f32)
    blast_blk[0, F] = np.asarray(inputs["fc_b"], f32)[0]

    b_all = np.concatenate([np.asarray(inputs["enc_hw_b"]),
                            np.asarray(inputs["dec_hw_b"])], axis=0)  # (16, 512)
    hw_b = np.asarray(b_all, f32).reshape(L, 4, 128).transpose(2, 0, 1).reshape(128, L * 4)

    def cols(v):  # (256,) -> (128, 2)
        return np.asarray(v, dtype=f32).reshape(2, 128).T

    bias_tbl = np.zeros((128, NB), dtype=f32)
    bias_tbl[:, COL_ENC0:COL_ENC0 + 2] = cols(inputs["enc_b0"])
    bias_tbl[:, COL_ENC1:COL_ENC1 + 2] = cols(inputs["enc_b1"])
    bias_tbl[:, COL_ENC2:COL_ENC2 + 2] = cols(inputs["enc_b2"])
    bias_tbl[:, COL_QB:COL_QB + 2] = cols(inputs["attn_q_b"])
    bias_tbl[:, COL_OB:COL_OB + 2] = cols(np.asarray(inputs["attn_o_b"], f32) * SQ2)
    bias_tbl[:, COL_DEC0:COL_DEC0 + 2] = cols(inputs["dec_b0"])
    bias_tbl[:, COL_DEC1:COL_DEC1 + 2] = cols(inputs["dec_b1"])
    bias_tbl[:, COL_DEC2:COL_DEC2 + 2] = cols(inputs["dec_b2"])
    bias_tbl[:, COL_DEC3:COL_DEC3 + 2] = cols(inputs["dec_b3"])
    bias2 = np.ascontiguousarray(np.concatenate([bias_tbl, hw_b], axis=1))

    shared = dict(hw_w=hw_w, bias2=bias2, wenc0p=wenc0p, pack1=pack1)

    in_maps = []
    for i in range(N_CORES):
        sl = slice(i * BS, (i + 1) * BS)
        kv_blocks = []
        for s in range(BS):
            for c in range(2):
                kv_blocks.append(keysT[i * BS + s, c * 128 : (c + 1) * 128, :])
        for s in range(BS):
            for c in range(2):
                kv_blocks.append(values[i * BS + s, c * 128 : (c + 1) * 128, :])
        wpack = npm(np.concatenate(
            [prefix] + kv_blocks[:4] + kv_blocks[4:] + [ones_blk, blast_blk], axis=1))
        m = dict(shared)
        m["x0"] = np.ascontiguousarray(x0[sl])
        m["wpack"] = wpack
        in_maps.append(m)
    return in_maps


def kernel(**inputs):
    global LAST_EXEC_NS
    if "nc" not in _BUILD_CACHE:
        _BUILD_CACHE["nc"] = _build()
    nc = _BUILD_CACHE["nc"]

    in_maps = _prep_host(inputs)

    trace = os.environ.get("KBENCH_TRACE", "0") == "1"
    if trace:
        _install_ntff_hook()
    res = run_bass_kernel_spmd(nc, in_maps, core_ids=list(range(N_CORES)), trace=trace)
    LAST_EXEC_NS = res.exec_time_ns

    mel = np.concatenate([r["mel"] for r in res.results], axis=0)
    attn = np.concatenate([r["attn"] for r in res.results], axis=0)
    done = np.concatenate([r["done"] for r in res.results], axis=0)
    return mel, attn, done


def _install_ntff_hook():
    """Register the axon NTFF profiling hook (missing from this image's antenv)."""
    import types

    if "antenv.axon_hooks" in sys.modules:
        return
    m = types.ModuleType("antenv.axon_hooks")
    m._h = None
    m.set_axon_ntff_profile_hook = lambda h: setattr(m, "_h", h)
    m.get_axon_ntff_profile_hook = lambda: m._h
    sys.modules["antenv.axon_hooks"] = m
    try:
        import antenv

        antenv.axon_hooks = m
        from trn_agent_boot.trn_boot import _ntff_profile_via_ctypes

        m._h = _ntff_profile_via_ctypes("/opt/axon/libaxon_pjrt.so")
    except Exception:
        m._h = None
